# revision 1
# baseline (speedup 1.0000x reference)
"""Trainium2 Bass kernel for nn_Block_42460046688864 (dense transformer block).

Reference math (B=2, T=2048, C=2048, H=16, HD=128):
    n1  = rmsnorm(x) * norm1_w
    qkv = n1 @ attn_w.T ; q,k,v per head ; q,k = rope(q,k) ; phi = elu(.)+1
    w   = (phi_q . phi_k) * scale * tril ; w /= sum(w) ; y = w @ v
    h   = y @ proj_w.T ; x2 = x + h
    ffn = gelu(rmsnorm(x2)*norm2_w @ fc_w.T) @ mlp_proj_w.T ; out = x2 + ffn

Distribution (8 NeuronCores, one NEFF, sequence-parallel Megatron):
  - rows (b*T+t, 4096 total) sharded 512/core for norms/residuals/output
  - attention head-sharded (2 heads/core) after an AllGather of n1^T
  - proj/mlp_proj row-parallel with ReduceScatter of partial sums
  - fc column-parallel (1024 hidden/core) after an AllGather of n2^T
  Attention itself is computed as *chunked linear attention*: the causal
  mask is exactly tril and elu+1 is positive, so sum-normalized masked
  scores equal prefix-state linear attention (the 1/sqrt(HD) scale and
  the 1e-8 epsilon cancel to ~1e-9 relative).

Notes:
  - norm weights are folded into attn_w / fc_w on the host (exact algebra).
  - matmul operands are bf16 (fp32 PSUM accumulation); norms, rope, elu,
    residuals and collective partial sums stay fp32.
  - TileContext's tail drain is patched to split its semaphore waits:
    this walrus build rejects >2 sync waits on one TPB_CTRL instruction.
"""

from contextlib import ExitStack

import numpy as np
import ml_dtypes

import concourse.bass as bass
import concourse.mybir as mybir
import concourse.tile as tile
from concourse.bass_utils import run_bass_kernel_spmd
from concourse.masks import make_identity
from bass_rust import ScopedClock

F32 = mybir.dt.float32
BF16 = mybir.dt.bfloat16
AF = mybir.ActivationFunctionType

N_CORES = 8
B, T, C, H, HD = 2, 2048, 2048, 16, 128
R = B * T                 # 4096 flattened rows (b-major)
R_LOC = R // N_CORES      # 512 rows per core
H_LOC = H // N_CORES      # 2 heads per core
F_LOC = (4 * C) // N_CORES  # 1024 mlp hidden per core
P = 128
EPS_NORM = 1e-5
N_RT = R_LOC // P         # 4 local row tiles
N_KC = C // P             # 16 contraction tiles over C
N_NB = R // 512           # 8 column blocks over flattened rows
N_CH = T // P             # 16 causal chunks per sequence


_MAX_WAITS = 1  # this walrus build rejects multi-wait instructions


def _split_excess_waits(nc):
    """Move excess semaphore waits onto same-engine NoOps ahead of the op."""
    for fn in nc.m.functions:
        for bb in fn.blocks:
            insts = list(bb.instructions)
            out = []
            for ins in insts:
                si = getattr(ins, "sync_info", None)
                waits = list(si.on_wait) if si and si.on_wait else []
                sem_waits = [w for w in waits if w.sync_type == "semaphore"]
                if len(sem_waits) > _MAX_WAITS:
                    keep = [w for w in waits if w.sync_type != "semaphore"]
                    keep += sem_waits[: _MAX_WAITS - 1] if _MAX_WAITS > 1 else []
                    extra = sem_waits[_MAX_WAITS - 1:] if _MAX_WAITS > 1 else sem_waits
                    for j in range(0, len(extra), _MAX_WAITS):
                        chunk = extra[j:j + _MAX_WAITS]
                        nop = mybir.InstNoOp(
                            name=nc.get_next_instruction_name(), ins=[], outs=[]
                        )
                        nop.engine = ins.engine
                        nop.sync_info = mybir.SyncInfo(on_wait=chunk, on_update=[])
                        out.append(nop)
                    si.on_wait[:] = keep
                out.append(ins)
            if len(out) != len(insts):
                bb.instructions[:] = out


class _TC(tile.TileContext):
    """TileContext whose tail drain splits sem waits one-per-NOP."""

    def schedule_and_allocate(self):
        ret = super().schedule_and_allocate()
        _split_excess_waits(self.nc)
        return ret

    def _drain_and_barrier(self, tick_clock, wait_clock):
        probe = self.nc.sync.nop(nofuse=True, hint="drain_waits")
        wait_clock.add_sem_waits(
            probe.ins, ScopedClock({None: tick_clock.global_clock})
        )
        si = probe.ins.sync_info
        waits = list(si.on_wait) if si and si.on_wait else []
        if len(waits) > 1:
            si.on_wait[:] = waits[:1]
            for w in waits[1:]:
                extra = self.nc.sync.nop(nofuse=True, hint="drain_waits")
                extra.ins.sync_info = mybir.SyncInfo(on_wait=[w], on_update=[])
        self.nc.sync.drain()
        self.nc.all_engine_barrier()
        popped = self.nc._tile_sem_poison_stack.pop()
        assert popped is self._sem_poison
        self.nc.clear_and_free_semaphores(list(self.sems.allocated().values()))
        self.nc.all_engine_barrier()


from contextlib import contextmanager


@contextmanager
def _low_priority(tc, offset=50000):
    tc.cur_priority += offset
    try:
        yield
    finally:
        tc.cur_priority -= offset


def _rmsnorm_transpose(nc, tc, pools, src_tiles, dstT_dram, ident_f32, eps_t):
    """rmsnorm rows of 4x[128,C] fp32 tiles -> bf16 transposed [C, 512] DRAM."""
    sq_pool, st_pool, n_pool, trp_pool, trc_pool = pools
    for i in range(N_RT):
        x_t = src_tiles[i]
        sq = sq_pool.tile([P, C], F32, name=f"sq{i}", tag="sq")
        ss = st_pool.tile([P, 1], F32, name=f"ss{i}", tag="ss")
        nc.scalar.activation(sq[:], x_t[:], AF.Square, accum_out=ss[:])
        rms = st_pool.tile([P, 1], F32, name=f"rms{i}", tag="rms")
        nc.scalar.activation(rms[:], ss[:], AF.Sqrt, bias=eps_t[:], scale=1.0 / C)
        inv = st_pool.tile([P, 1], F32, name=f"inv{i}", tag="inv")
        nc.vector.reciprocal(inv[:], rms[:])
        n_t = n_pool.tile([P, C], F32, name=f"n{i}", tag="n")
        nc.vector.tensor_scalar_mul(n_t[:], x_t[:], inv[:])
        for j in range(N_KC):
            ps = trp_pool.tile([P, P], F32, name=f"trp{i}_{j}", tag="trp")
            nc.tensor.transpose(ps[:], n_t[:, j * P:(j + 1) * P], ident_f32[:])
            cp = trc_pool.tile([P, P], BF16, name=f"trc{i}_{j}", tag="trc")
            nc.scalar.copy(cp[:], ps[:])
            nc.sync.dma_start(
                out=dstT_dram[j * P:(j + 1) * P, i * P:(i + 1) * P], in_=cp[:]
            )


def build_nc():
    nc = bass.Bass(target_bir_lowering=False)

    x_loc = nc.declare_dram_parameter("x_loc", [R_LOC, C], F32, isOutput=False)
    cosT = nc.declare_dram_parameter("cosT", [HD // 2, R], F32, isOutput=False)
    sinT = nc.declare_dram_parameter("sinT", [HD // 2, R], F32, isOutput=False)
    maskT = nc.declare_dram_parameter("maskT", [P, P], F32, isOutput=False)
    attn_wT = nc.declare_dram_parameter("attn_wT", [C, 3 * HD * H_LOC], BF16, isOutput=False)
    projwT = nc.declare_dram_parameter("projwT", [HD * H_LOC, C], BF16, isOutput=False)
    fcwT = nc.declare_dram_parameter("fcwT", [C, F_LOC], BF16, isOutput=False)
    mlpw = nc.declare_dram_parameter("mlpw", [F_LOC, C], BF16, isOutput=False)
    out_loc = nc.declare_dram_parameter("out_loc", [R_LOC, C], F32, isOutput=True)

    n1T_loc = nc.dram_tensor("n1T_loc", [C, R_LOC], BF16)
    n1T_all = nc.dram_tensor("n1T_all", [N_CORES, C, R_LOC], BF16, addr_space="Shared")
    h_part = nc.dram_tensor("h_part", [R, C], BF16)
    h_loc = nc.dram_tensor("h_loc", [R_LOC, C], BF16)
    n2T_loc = nc.dram_tensor("n2T_loc", [C, R_LOC], BF16)
    n2T_all = nc.dram_tensor("n2T_all", [N_CORES, C, R_LOC], BF16, addr_space="Shared")
    ffn_part = nc.dram_tensor("ffn_part", [R, C], BF16)
    ffn_loc = nc.dram_tensor("ffn_loc", [R_LOC, C], BF16)

    groups = [list(range(N_CORES))]

    with _TC(nc) as tc:
        with (
            tc.tile_pool(name="const", bufs=1) as const,
            tc.tile_pool(name="yT", bufs=1) as yT_pool,
        ):
            ident_f32 = const.tile([P, P], F32)
            make_identity(nc, ident_f32)
            ident_bf = const.tile([P, P], BF16)
            make_identity(nc, ident_bf)
            mask_sb = const.tile([P, P], F32)
            nc.sync.dma_start(out=mask_sb[:], in_=maskT[:, :])
            eps_t = const.tile([P, 1], F32)
            nc.vector.memset(eps_t[:], EPS_NORM)

            # per-t-chunk tiles so proj deps are precise (proj overlaps attention)
            yT = [
                [yT_pool.tile([P, P], BF16, name=f"yT{h}_{m}") for m in range(R // P)]
                for h in range(H_LOC)
            ]

            # ---- phase 0: rmsnorm(x_loc) -> n1T_loc; AllGather -> n1T_all
            with (
                tc.tile_pool(name="p0x", bufs=2) as p0x,
                tc.tile_pool(name="p0sq", bufs=2) as p0sq,
                tc.tile_pool(name="p0st", bufs=8) as p0st,
                tc.tile_pool(name="p0n", bufs=4) as p0n,
                tc.tile_pool(name="p0trp", bufs=4, space="PSUM") as p0trp,
                tc.tile_pool(name="p0trc", bufs=8) as p0trc,
            ):
                x_tiles = []
                for i in range(N_RT):
                    x_t = p0x.tile([P, C], F32, name=f"x{i}", tag=f"x{i}")
                    nc.sync.dma_start(out=x_t[:], in_=x_loc[i * P:(i + 1) * P, :])
                    x_tiles.append(x_t)
                _rmsnorm_transpose(
                    nc, tc, (p0sq, p0st, p0n, p0trp, p0trc), x_tiles, n1T_loc, ident_f32, eps_t
                )
                nc.gpsimd.collective_compute(
                    "AllGather",
                    mybir.AluOpType.bypass,
                    ins=[n1T_loc.ap().opt()],
                    outs=[n1T_all.ap().opt()],
                    replica_groups=groups,
                )

            # ---- phase 1: qkv^T for 2 local heads + rope + elu+1 -> Q/K/V
            # resident [128, 4096] bf16 per (head, comp)
            with tc.tile_pool(name="qkvres", bufs=1) as qkv_pool:
                qres = [qkv_pool.tile([P, R], BF16, name=f"q{h}") for h in range(H_LOC)]
                kres = [qkv_pool.tile([P, R], BF16, name=f"k{h}") for h in range(H_LOC)]
                vres = [qkv_pool.tile([P, R], BF16, name=f"v{h}") for h in range(H_LOC)]

                with (
                    tc.tile_pool(name="p1w", bufs=1) as p1w,
                    tc.tile_pool(name="p1cs", bufs=1) as p1cs,
                    tc.tile_pool(name="p1rhs", bufs=18) as p1rhs,
                    tc.tile_pool(name="p1ps", bufs=4, space="PSUM") as p1ps,
                    tc.tile_pool(name="p1rp", bufs=4) as p1rp,
                ):
                    cos_sb = p1cs.tile([HD // 2, R], F32, name="cos_sb")
                    sin_sb = p1cs.tile([HD // 2, R], F32, name="sin_sb")
                    nc.sync.dma_start(out=cos_sb[:], in_=cosT[:, :])
                    nc.sync.dma_start(out=sin_sb[:], in_=sinT[:, :])
                    aw = []
                    for k in range(N_KC):
                        w_t = p1w.tile([P, 3 * HD * H_LOC], BF16, name=f"aw{k}", tag=f"aw{k}")
                        nc.sync.dma_start(
                            out=w_t[:], in_=attn_wT[k * P:(k + 1) * P, :]
                        )
                        aw.append(w_t)

                    for nb in range(N_NB):
                        rhs = []
                        for k in range(N_KC):
                            r_t = p1rhs.tile([P, 512], BF16, name=f"n1r{nb}_{k}", tag="n1r")
                            nc.sync.dma_start(
                                out=r_t[:],
                                in_=n1T_all[nb, k * P:(k + 1) * P, :],
                            )
                            rhs.append(r_t)
                        ncol = slice(nb * 512, (nb + 1) * 512)
                        for h in range(H_LOC):
                            for comp in range(3):
                                j = h * 3 + comp
                                ps = p1ps.tile([P, 512], F32, name=f"qkvp{nb}_{j}", tag="qkvp")
                                for k in range(N_KC):
                                    nc.tensor.matmul(
                                        ps[:],
                                        aw[k][:, j * P:(j + 1) * P],
                                        rhs[k][:],
                                        start=(k == 0),
                                        stop=(k == N_KC - 1),
                                    )
                                if comp == 2:
                                    nc.scalar.copy(vres[h][:, ncol], ps[:])
                                else:
                                    dst = qres[h] if comp == 0 else kres[h]
                                    HF = HD // 2
                                    ro = p1rp.tile([P, 512], F32, name=f"ro{nb}_{j}", tag="ro")
                                    s1 = p1rp.tile([HF, 512], F32, name=f"s1{nb}_{j}", tag="s1")
                                    s2 = p1rp.tile([HF, 512], F32, name=f"s2{nb}_{j}", tag="s2")
                                    # rope: out[0:64] = a1*cos - a2*sin ; out[64:128] = a1*sin + a2*cos
                                    nc.vector.tensor_mul(s1[:], ps[0:HF, :], cos_sb[:, ncol])
                                    nc.vector.tensor_mul(s2[:], ps[HF:P, :], sin_sb[:, ncol])
                                    nc.vector.tensor_sub(ro[0:HF, :], s1[:], s2[:])
                                    nc.vector.tensor_mul(s1[:], ps[0:HF, :], sin_sb[:, ncol])
                                    nc.vector.tensor_mul(s2[:], ps[HF:P, :], cos_sb[:, ncol])
                                    nc.vector.tensor_add(ro[HF:P, :], s1[:], s2[:])
                                    # phi = elu(ro)+1 = relu(ro) + exp(ro - relu(ro))
                                    rl = p1rp.tile([P, 512], F32, name=f"rl{nb}_{j}", tag="rl")
                                    nc.scalar.activation(rl[:], ro[:], AF.Relu)
                                    dmin = p1rp.tile([P, 512], F32, name=f"dm{nb}_{j}", tag="dm")
                                    nc.vector.tensor_sub(dmin[:], ro[:], rl[:])
                                    ex = p1rp.tile([P, 512], F32, name=f"ex{nb}_{j}", tag="ex")
                                    nc.scalar.activation(ex[:], dmin[:], AF.Exp)
                                    nc.vector.tensor_add(dst[:, ncol], rl[:], ex[:])

                # ---- phase 2: chunked linear attention per (head, b)
                with (
                    tc.tile_pool(name="p2st", bufs=1) as p2st,
                    tc.tile_pool(name="p2sbf", bufs=3) as p2sbf,
                    tc.tile_pool(name="p2sb", bufs=8) as p2sb,
                    tc.tile_pool(name="p2psA", bufs=3, space="PSUM") as p2psA,
                    tc.tile_pool(name="p2psY", bufs=3, space="PSUM") as p2psY,
                    tc.tile_pool(name="p2psS", bufs=2, space="PSUM") as p2psS,
                ):
                    s_sb_d = {}
                    s_bf_d = {}
                    for h in range(H_LOC):
                        for b in range(B):
                            s_sb = p2st.tile([P, HD + 1], F32, name=f"S{h}_{b}")
                            nc.vector.memset(s_sb[:], 0.0)
                            s_bf = p2sbf.tile([P, HD + 1], BF16, name=f"Sb{h}_{b}_init", tag=f"sbf{h}{b}")
                            nc.vector.memset(s_bf[:], 0.0)
                            s_sb_d[(h, b)] = s_sb
                            s_bf_d[(h, b)] = s_bf
                    for i in range(N_CH):
                        for h in range(H_LOC):
                            for b in range(B):
                                s_sb = s_sb_d[(h, b)]
                                s_bf = s_bf_d[(h, b)]
                                t0 = b * T + i * P
                                tcol = slice(t0, t0 + P)
                                # A^T[s,t] = sum_d K^T[d,s] Q^T[d,t]
                                a_ps = p2psA.tile([P, P], F32, name=f"A{h}{b}{i}", tag="A")
                                nc.tensor.matmul(
                                    a_ps[:], kres[h][:, tcol], qres[h][:, tcol],
                                    start=True, stop=True,
                                )
                                am = p2sb.tile([P, P], BF16, name=f"Am{h}{b}{i}", tag="Am")
                                nc.vector.tensor_mul(am[:], a_ps[:], mask_sb[:])
                                # V' = [V_chunk | 1], K_chunk row-major via DMA transpose
                                vp = p2sb.tile([P, HD + 1], BF16, name=f"Vp{h}{b}{i}", tag="Vp")
                                nc.vector.memset(vp[:, HD:HD + 1], 1.0)
                                nc.sync.dma_start_transpose(vp[:, 0:HD], vres[h][:, tcol])
                                kp = p2sb.tile([P, P], BF16, name=f"Kp{h}{b}{i}", tag="Kp")
                                nc.sync.dma_start_transpose(kp[:], kres[h][:, tcol])
                                # Y = Q_chunk @ S' + Am^T @ V'  (last col = denominator)
                                y_ps = p2psY.tile([P, HD + 1], F32, name=f"Y{h}{b}{i}", tag="Y")
                                nc.tensor.matmul(
                                    y_ps[:], qres[h][:, tcol], s_bf[:],
                                    start=True, stop=False,
                                )
                                nc.tensor.matmul(
                                    y_ps[:], am[:], vp[:], start=False, stop=True
                                )
                                # state += K_chunk^T-outer-V'
                                sd_ps = p2psS.tile([P, HD + 1], F32, name=f"Sd{h}{b}{i}", tag="Sd")
                                nc.tensor.matmul(
                                    sd_ps[:], kp[:], vp[:], start=True, stop=True
                                )
                                nc.vector.tensor_add(s_sb[:], s_sb[:], sd_ps[:])
                                s_bf = p2sbf.tile([P, HD + 1], BF16, name=f"Sb{h}_{b}_{i}", tag=f"sbf{h}{b}")
                                nc.scalar.copy(s_bf[:], s_sb[:])
                                s_bf_d[(h, b)] = s_bf
                                # y = num/den ; write y^T
                                rec = p2sb.tile([P, 1], F32, name=f"rec{h}{b}{i}", tag="rec")
                                nc.vector.reciprocal(rec[:], y_ps[:, HD:HD + 1])
                                y_sb = p2sb.tile([P, HD], BF16, name=f"y{h}{b}{i}", tag="y")
                                nc.vector.tensor_scalar_mul(y_sb[:], y_ps[:, 0:HD], rec[:])
                                nc.sync.dma_start_transpose(
                                    yT[h][b * N_CH + i][:], y_sb[:]
                                )

            # ---- phase 3: h_part = y^T.T @ projwT (row-parallel partial)
            with (
                tc.tile_pool(name="p3w", bufs=1) as p3w,
                tc.tile_pool(name="p3ps", bufs=4, space="PSUM") as p3ps,
                tc.tile_pool(name="p3ev", bufs=8) as p3ev,
            ):
                pw = []
                for kd in range(H_LOC):
                    w_t = p3w.tile([P, C], BF16, name=f"pw{kd}", tag=f"pw{kd}")
                    nc.sync.dma_start(out=w_t[:], in_=projwT[kd * P:(kd + 1) * P, :])
                    pw.append(w_t)
                for mt in range(R // P):
                    mcol = slice(mt * P, (mt + 1) * P)
                    for ont in range(C // 512):
                        ps = p3ps.tile([P, 512], F32, name=f"hp{mt}_{ont}", tag="hp")
                        for kd in range(H_LOC):
                            nc.tensor.matmul(
                                ps[:],
                                yT[kd][mt][:],
                                pw[kd][:, ont * 512:(ont + 1) * 512],
                                start=(kd == 0),
                                stop=(kd == H_LOC - 1),
                            )
                        ev = p3ev.tile([P, 512], BF16, name=f"he{mt}_{ont}", tag="he")
                        nc.scalar.copy(ev[:], ps[:])
                        nc.sync.dma_start(
                            out=h_part[mt * P:(mt + 1) * P, ont * 512:(ont + 1) * 512],
                            in_=ev[:],
                        )
                nc.gpsimd.collective_compute(
                    "ReduceScatter",
                    mybir.AluOpType.add,
                    ins=[h_part.ap().opt()],
                    outs=[h_loc.ap().opt()],
                    replica_groups=groups,
                )

            # ---- phase 4: x2 = x + h (own rows); rmsnorm2 -> n2T; AllGather
            x2_ctx = ExitStack()
            x2_pool = x2_ctx.enter_context(tc.tile_pool(name="x2res", bufs=1))
            x2_res = [x2_pool.tile([P, C], F32, name=f"x2_{i}") for i in range(N_RT)]
            with (
                tc.tile_pool(name="p4h", bufs=4) as p4h,
                tc.tile_pool(name="p4sq", bufs=2) as p4sq,
                tc.tile_pool(name="p4st", bufs=8) as p4st,
                tc.tile_pool(name="p4n", bufs=4) as p4n,
                tc.tile_pool(name="p4trp", bufs=4, space="PSUM") as p4trp,
                tc.tile_pool(name="p4trc", bufs=8) as p4trc,
            ):
                for i in range(N_RT):
                    hb_t = p4h.tile([P, C], BF16, name=f"hb{i}", tag="hb")
                    nc.sync.dma_start(out=hb_t[:], in_=h_loc[i * P:(i + 1) * P, :])
                    h_t = p4h.tile([P, C], F32, name=f"h{i}", tag="h")
                    nc.scalar.copy(h_t[:], hb_t[:])
                    x_t = p4h.tile([P, C], F32, name=f"x4_{i}", tag="x4")
                    nc.sync.dma_start(out=x_t[:], in_=x_loc[i * P:(i + 1) * P, :])
                    nc.vector.tensor_add(x2_res[i][:], x_t[:], h_t[:])
                _rmsnorm_transpose(
                    nc, tc, (p4sq, p4st, p4n, p4trp, p4trc), x2_res, n2T_loc, ident_f32, eps_t
                )
                nc.gpsimd.collective_compute(
                    "AllGather",
                    mybir.AluOpType.bypass,
                    ins=[n2T_loc.ap().opt()],
                    outs=[n2T_all.ap().opt()],
                    replica_groups=groups,
                )

            # ---- phase 5: gT = gelu(fcwT.T @ n2T); ffn_part = gT.T @ mlpw
            with (
                tc.tile_pool(name="p5fw", bufs=1) as p5fw,
                tc.tile_pool(name="p5mw", bufs=1) as p5mw,
                tc.tile_pool(name="p5rhs", bufs=18) as p5rhs,
                tc.tile_pool(name="p5g", bufs=1) as p5g,
                tc.tile_pool(name="p5ps", bufs=3, space="PSUM") as p5ps,
                tc.tile_pool(name="p5ps2", bufs=3, space="PSUM") as p5ps2,
                tc.tile_pool(name="p5ev", bufs=4) as p5ev,
            ):
                fw = []
                for k in range(N_KC):
                    w_t = p5fw.tile([P, F_LOC], BF16, name=f"fw{k}", tag=f"fw{k}")
                    nc.sync.dma_start(out=w_t[:], in_=fcwT[k * P:(k + 1) * P, :])
                    fw.append(w_t)
                mw = []
                for k in range(F_LOC // P):
                    w_t = p5mw.tile([P, C], BF16, name=f"mw{k}", tag=f"mw{k}")
                    nc.sync.dma_start(out=w_t[:], in_=mlpw[k * P:(k + 1) * P, :])
                    mw.append(w_t)

                gk = [None] * (F_LOC // P)
                for nb in range(N_NB):
                    rhs = []
                    for k in range(N_KC):
                        r_t = p5rhs.tile([P, 512], BF16, name=f"n2r{nb}_{k}", tag="n2r")
                        nc.sync.dma_start(
                            out=r_t[:], in_=n2T_all[nb, k * P:(k + 1) * P, :]
                        )
                        rhs.append(r_t)
                    for mf in range(F_LOC // P):
                        ps = p5ps.tile([P, 512], F32, name=f"gp{nb}_{mf}", tag="gp")
                        for k in range(N_KC):
                            nc.tensor.matmul(
                                ps[:],
                                fw[k][:, mf * P:(mf + 1) * P],
                                rhs[k][:],
                                start=(k == 0),
                                stop=(k == N_KC - 1),
                            )
                        g_t = p5g.tile([P, 512], BF16, name=f"g{nb}_{mf}", tag=f"g{mf}", bufs=2)
                        nc.scalar.activation(g_t[:], ps[:], AF.Gelu)
                        gk[mf] = g_t
                    for mt in range(4):
                        mcol = slice(mt * P, (mt + 1) * P)
                        row0 = nb * 512 + mt * P
                        for ont in range(C // 512):
                            ps2 = p5ps2.tile([P, 512], F32, name=f"fp{nb}_{mt}_{ont}", tag="fp")
                            for kf in range(F_LOC // P):
                                nc.tensor.matmul(
                                    ps2[:],
                                    gk[kf][:, mcol],
                                    mw[kf][:, ont * 512:(ont + 1) * 512],
                                    start=(kf == 0),
                                    stop=(kf == F_LOC // P - 1),
                                )
                            ev = p5ev.tile([P, 512], BF16, name=f"fe{nb}_{mt}_{ont}", tag="fe")
                            nc.scalar.copy(ev[:], ps2[:])
                            nc.sync.dma_start(
                                out=ffn_part[row0:row0 + P, ont * 512:(ont + 1) * 512],
                                in_=ev[:],
                            )
                nc.gpsimd.collective_compute(
                    "ReduceScatter",
                    mybir.AluOpType.add,
                    ins=[ffn_part.ap().opt()],
                    outs=[ffn_loc.ap().opt()],
                    replica_groups=groups,
                )

            # ---- phase 6: out = x2 + ffn (own rows)
            with tc.tile_pool(name="p6", bufs=2) as p6:
                for i in range(N_RT):
                    fb_t = p6.tile([P, C], BF16, name=f"fb{i}", tag="fb")
                    nc.sync.dma_start(out=fb_t[:], in_=ffn_loc[i * P:(i + 1) * P, :])
                    f_t = p6.tile([P, C], F32, name=f"f{i}", tag="f")
                    nc.scalar.copy(f_t[:], fb_t[:])
                    o_t = p6.tile([P, C], F32, name=f"o{i}", tag="o")
                    nc.vector.tensor_add(o_t[:], x2_res[i][:], f_t[:])
                    nc.sync.dma_start(out=out_loc[i * P:(i + 1) * P, :], in_=o_t[:])
            x2_ctx.close()

    return nc


_NC_CACHE = None


def _get_nc():
    global _NC_CACHE
    if _NC_CACHE is None:
        _NC_CACHE = build_nc()
    return _NC_CACHE


def _prep_inputs(x, cos, sin, attention_bias, norm1_w, norm2_w, attn_w, proj_w,
                 fc_w, mlp_proj_w):
    bf = ml_dtypes.bfloat16
    xf = np.asarray(x, np.float32).reshape(R, C)
    cosT = np.ascontiguousarray(
        np.concatenate([np.asarray(cos, np.float32).T] * B, axis=1)
    )
    sinT = np.ascontiguousarray(
        np.concatenate([np.asarray(sin, np.float32).T] * B, axis=1)
    )
    # mask[s, t] = 1 iff s <= t  (transposed causal tril)
    maskT = np.triu(np.ones((P, P), np.float32))
    w1 = np.asarray(norm1_w, np.float32)
    w2 = np.asarray(norm2_w, np.float32)
    aw = np.asarray(attn_w, np.float32).reshape(H, 3, HD, C)
    pw = np.asarray(proj_w, np.float32)
    fw = np.asarray(fc_w, np.float32)
    mw = np.asarray(mlp_proj_w, np.float32)

    in_maps = []
    for c in range(N_CORES):
        aw_c = (aw[2 * c:2 * c + 2].reshape(3 * HD * H_LOC, C) * w1[None, :])
        fw_c = fw[F_LOC * c:F_LOC * (c + 1)] * w2[None, :]
        in_maps.append({
            "x_loc": np.ascontiguousarray(xf[R_LOC * c:R_LOC * (c + 1)]),
            "cosT": cosT,
            "sinT": sinT,
            "maskT": maskT,
            "attn_wT": np.ascontiguousarray(aw_c.T).astype(bf),
            "projwT": np.ascontiguousarray(
                pw[:, HD * H_LOC * c:HD * H_LOC * (c + 1)].T
            ).astype(bf),
            "fcwT": np.ascontiguousarray(fw_c.T).astype(bf),
            "mlpw": np.ascontiguousarray(
                mw[:, F_LOC * c:F_LOC * (c + 1)].T
            ).astype(bf),
        })
    return in_maps


def kernel(**inputs):
    nc = _get_nc()
    in_maps = _prep_inputs(**inputs)
    res = run_bass_kernel_spmd(nc, in_maps, list(range(N_CORES)))
    out = np.concatenate(
        [np.asarray(res.results[c]["out_loc"], np.float32) for c in range(N_CORES)],
        axis=0,
    )
    return out.reshape(B, T, C)



# revision 7
# speedup vs baseline: 1.8073x; 1.8073x over previous
"""Trainium2 Bass kernel for nn_Block_42460046688864 (dense transformer block).

Reference math (B=2, T=2048, C=2048, H=16, HD=128):
    n1  = rmsnorm(x) * norm1_w
    qkv = n1 @ attn_w.T ; q,k,v per head ; q,k = rope(q,k) ; phi = elu(.)+1
    w   = (phi_q . phi_k) * scale * tril ; w /= sum(w) ; y = w @ v
    h   = y @ proj_w.T ; x2 = x + h
    ffn = gelu(rmsnorm(x2)*norm2_w @ fc_w.T) @ mlp_proj_w.T ; out = x2 + ffn

Distribution (8 NeuronCores, one NEFF, fully data-parallel):
  - rows (b*T+t, 4096 total) sharded 512/core; every core streams the FULL
    weights from its own HBM (no activation collectives at all).
  - attention is chunked linear attention (causal tril + positive elu+1
    features == prefix-state form; scale and eps cancel to ~1e-9 rel).
    The only cross-core dependency is the causal prefix state: each core's
    segment state S_seg[h] = sum_t k_t (x) [v_t | 1] is exchanged with ONE
    small ReduceScatter. Core j writes S_seg * mask[j<s, same-seq] into
    slot s, so after the add-RS core s holds exactly the sum of its
    same-sequence predecessors' states (its causal init state). The RS is
    issued right after K/V are ready and overlaps the Q projection; the
    correction q @ S_init is fused into each chunk's PSUM accumulation.
  - V is computed directly in [token, dim] layout by using n1^T chunks as
    the stationary matmul operand (no V transposes); K additionally needs
    [token, dim] for the state outer products -> 64 small DMA transposes.

Notes:
  - norm weights are folded into attn_w / fc_w on the host (exact algebra).
  - matmul operands are bf16 (fp32 PSUM accumulation); norms, residuals and
    attention numerators/denominators stay fp32 (psum) end to end.
  - weights are pre-tiled on the host into [128 x N] DMA slabs so every
    weight DMA is one contiguous >=2KB-per-partition block.
  - SBUF pools are strict LIFO per side; long-lived attention tiles live on
    the left stack, y^T on the right stack so lifetimes nest.
  - TileContext's tail drain is patched to split its semaphore waits:
    this walrus build rejects >2 sync waits on one TPB_CTRL instruction.
"""

from contextlib import ExitStack

import numpy as np
import ml_dtypes

import concourse.bass as bass
import concourse.mybir as mybir
import concourse.tile as tile
from concourse.bass_utils import run_bass_kernel_spmd
from concourse.masks import make_identity
from bass_rust import ScopedClock

F32 = mybir.dt.float32
BF16 = mybir.dt.bfloat16
AF = mybir.ActivationFunctionType

N_CORES = 8
B, T, C, H, HD = 2, 2048, 2048, 16, 128
F = 4 * C                  # 8192 mlp hidden
R = B * T                  # 4096 flattened rows (b-major)
R_LOC = R // N_CORES       # 512 rows per core
P = 128
EPS_NORM = 1e-5
N_RT = R_LOC // P          # 4 local row tiles
N_KC = C // P              # 16 contraction tiles over C
N_CH = N_RT                # 4 local causal chunks
N_MF = F // P              # 64 mlp-hidden tiles
HD1 = HD + 1               # state cols: [v dims | 1]
SAW = H * HD1              # 2064 = all-head state cols
MLP_CC = 256               # mlp output col-chunk
N_MCH = C // MLP_CC        # 8 col chunks

_MAX_WAITS = 1  # this walrus build rejects multi-wait instructions


def _split_excess_waits(nc):
    """Move excess semaphore waits onto same-engine NoOps ahead of the op."""
    for fn in nc.m.functions:
        for bb in fn.blocks:
            insts = list(bb.instructions)
            out = []
            for ins in insts:
                si = getattr(ins, "sync_info", None)
                waits = list(si.on_wait) if si and si.on_wait else []
                sem_waits = [w for w in waits if w.sync_type == "semaphore"]
                if len(sem_waits) > _MAX_WAITS:
                    keep = [w for w in waits if w.sync_type != "semaphore"]
                    keep += sem_waits[: _MAX_WAITS - 1] if _MAX_WAITS > 1 else []
                    extra = sem_waits[_MAX_WAITS - 1:] if _MAX_WAITS > 1 else sem_waits
                    for j in range(0, len(extra), _MAX_WAITS):
                        chunk = extra[j:j + _MAX_WAITS]
                        nop = mybir.InstNoOp(
                            name=nc.get_next_instruction_name(), ins=[], outs=[]
                        )
                        nop.engine = ins.engine
                        nop.sync_info = mybir.SyncInfo(on_wait=chunk, on_update=[])
                        out.append(nop)
                    si.on_wait[:] = keep
                out.append(ins)
            if len(out) != len(insts):
                bb.instructions[:] = out


class _TC(tile.TileContext):
    """TileContext whose tail drain splits sem waits one-per-NOP."""

    def schedule_and_allocate(self):
        ret = super().schedule_and_allocate()
        _split_excess_waits(self.nc)
        return ret

    def _drain_and_barrier(self, tick_clock, wait_clock):
        probe = self.nc.sync.nop(nofuse=True, hint="drain_waits")
        wait_clock.add_sem_waits(
            probe.ins, ScopedClock({None: tick_clock.global_clock})
        )
        si = probe.ins.sync_info
        waits = list(si.on_wait) if si and si.on_wait else []
        if len(waits) > 1:
            si.on_wait[:] = waits[:1]
            for w in waits[1:]:
                extra = self.nc.sync.nop(nofuse=True, hint="drain_waits")
                extra.ins.sync_info = mybir.SyncInfo(on_wait=[w], on_update=[])
        self.nc.sync.drain()
        self.nc.all_engine_barrier()
        popped = self.nc._tile_sem_poison_stack.pop()
        assert popped is self._sem_poison
        self.nc.clear_and_free_semaphores(list(self.sems.allocated().values()))
        self.nc.all_engine_barrier()


def build_nc():
    nc = bass.Bass(target_bir_lowering=False)

    x_loc = nc.declare_dram_parameter("x_loc", [R_LOC, C], F32, isOutput=False)
    cosr = nc.declare_dram_parameter("cosr", [HD // 2, R_LOC], BF16, isOutput=False)
    sinr = nc.declare_dram_parameter("sinr", [HD // 2, R_LOC], BF16, isOutput=False)
    maskT = nc.declare_dram_parameter("maskT", [P, P], F32, isOutput=False)
    smask = nc.declare_dram_parameter("smask", [P, N_CORES], F32, isOutput=False)
    # pre-tiled weight slabs (see _prep_inputs for layouts)
    qkw = nc.declare_dram_parameter("qkw", [2 * H, P, C], BF16, isOutput=False)
    vw = nc.declare_dram_parameter("vw", [2, N_KC, P, C // 2], BF16, isOutput=False)
    pw = nc.declare_dram_parameter("pw", [N_KC, P, C], BF16, isOutput=False)
    fcw = nc.declare_dram_parameter("fcw", [N_MF, P, C], BF16, isOutput=False)
    mww = nc.declare_dram_parameter(
        "mww", [N_MCH, P, N_MF * MLP_CC], BF16, isOutput=False
    )
    out_loc = nc.declare_dram_parameter("out_loc", [R_LOC, C], F32, isOutput=True)

    rs_in = nc.dram_tensor("rs_in", [N_CORES, P, SAW], BF16)
    rs_out = nc.dram_tensor("rs_out", [P, SAW], BF16)

    groups = [list(range(N_CORES))]

    with _TC(nc) as tc:
        stk = ExitStack()
        const = stk.enter_context(tc.tile_pool(name="const", bufs=1))
        ident_f32 = const.tile([P, P], F32)
        make_identity(nc, ident_f32)
        mask_sb = const.tile([P, P], F32)
        nc.sync.dma_start(out=mask_sb[:], in_=maskT[:, :])
        smask_sb = const.tile([P, N_CORES], F32)
        nc.sync.dma_start(out=smask_sb[:], in_=smask[:, :])
        eps_t = const.tile([P, 1], F32)
        nc.vector.memset(eps_t[:], EPS_NORM)
        cos_sb = const.tile([HD // 2, R_LOC], BF16)
        sin_sb = const.tile([HD // 2, R_LOC], BF16)
        nc.sync.dma_start(out=cos_sb[:], in_=cosr[:, :])
        nc.sync.dma_start(out=sin_sb[:], in_=sinr[:, :])

        # y^T on the right stack: outlives the attention residents (left).
        yT_ctx = ExitStack()
        yT_pool = yT_ctx.enter_context(tc.tile_pool(name="yT", bufs=1, side="right"))
        yT = [
            [yT_pool.tile([P, P], BF16, name=f"yT{h}_{i}") for i in range(N_CH)]
            for h in range(H)
        ]

        # attention residents (left): released together after phase 4.
        att_ctx = ExitStack()
        n1T_pool = att_ctx.enter_context(tc.tile_pool(name="n1T", bufs=1))
        qk_pool = att_ctx.enter_context(tc.tile_pool(name="qkres", bufs=1))
        vp_pool = att_ctx.enter_context(tc.tile_pool(name="vpres", bufs=1))
        sbf_pool = att_ctx.enter_context(tc.tile_pool(name="sbfres", bufs=1))
        n1T = [n1T_pool.tile([P, R_LOC], BF16, name=f"n1T{k}") for k in range(N_KC)]
        kres = [qk_pool.tile([P, R_LOC], BF16, name=f"k{h}") for h in range(H)]
        qres = [qk_pool.tile([P, R_LOC], BF16, name=f"q{h}") for h in range(H)]
        vp = [
            [vp_pool.tile([P, HD1], BF16, name=f"vp{h}_{i}") for i in range(N_CH)]
            for h in range(H)
        ]

        # ---- phase 0: rmsnorm(x) -> n1T (SBUF-resident, transposed) ----
        with (
            tc.tile_pool(name="p0x", bufs=2) as p0x,
            tc.tile_pool(name="p0sq", bufs=1) as p0sq,
            tc.tile_pool(name="p0st", bufs=8) as p0st,
            tc.tile_pool(name="p0n", bufs=2) as p0n,
            tc.tile_pool(name="p0tr", bufs=4, space="PSUM") as p0tr,
        ):
            for i in range(N_RT):
                x_t = p0x.tile([P, C], F32, name=f"x{i}", tag="x")
                nc.sync.dma_start(out=x_t[:], in_=x_loc[i * P:(i + 1) * P, :])
                sq = p0sq.tile([P, C], F32, name=f"sq{i}", tag="sq")
                ss = p0st.tile([P, 1], F32, name=f"ss{i}", tag="ss")
                nc.scalar.activation(sq[:], x_t[:], AF.Square, accum_out=ss[:])
                rms = p0st.tile([P, 1], F32, name=f"rms{i}", tag="rms")
                nc.scalar.activation(
                    rms[:], ss[:], AF.Sqrt, bias=eps_t[:], scale=1.0 / C
                )
                inv = p0st.tile([P, 1], F32, name=f"inv{i}", tag="inv")
                nc.vector.reciprocal(inv[:], rms[:])
                n_t = p0n.tile([P, C], F32, name=f"n{i}", tag="n")
                nc.vector.tensor_scalar_mul(n_t[:], x_t[:], inv[:])
                for k in range(N_KC):
                    ps = p0tr.tile([P, P], F32, name=f"tr{i}_{k}", tag="tr")
                    nc.tensor.transpose(ps[:], n_t[:, k * P:(k + 1) * P], ident_f32[:])
                    nc.scalar.copy(n1T[k][:, i * P:(i + 1) * P], ps[:])

        # ---- phase 1: K then V then Q projections --------------------
        HF = HD // 2

        def rope_elu(dst, ps, pool, uid):
            """psum [128 x 512] (d-major head tile) -> phi(rope(.)) bf16."""
            raw = pool.tile([P, R_LOC], BF16, name=f"raw{uid}", tag="raw")
            nc.scalar.copy(raw[:], ps[:])
            s1 = pool.tile([HF, R_LOC], BF16, name=f"s1{uid}", tag="s1")
            s2 = pool.tile([HF, R_LOC], BF16, name=f"s2{uid}", tag="s2")
            ro = pool.tile([P, R_LOC], BF16, name=f"ro{uid}", tag="ro")
            nc.vector.tensor_mul(s1[:], raw[0:HF, :], cos_sb[:])
            nc.vector.tensor_mul(s2[:], raw[HF:P, :], sin_sb[:])
            nc.vector.tensor_sub(ro[0:HF, :], s1[:], s2[:])
            nc.vector.tensor_mul(s1[:], raw[0:HF, :], sin_sb[:])
            nc.vector.tensor_mul(s2[:], raw[HF:P, :], cos_sb[:])
            nc.vector.tensor_add(ro[HF:P, :], s1[:], s2[:])
            # phi = elu(ro)+1 = relu(ro) + exp(ro - relu(ro))
            rl = pool.tile([P, R_LOC], BF16, name=f"rl{uid}", tag="rl")
            nc.scalar.activation(rl[:], ro[:], AF.Relu)
            dm = pool.tile([P, R_LOC], BF16, name=f"dm{uid}", tag="dm")
            nc.vector.tensor_sub(dm[:], ro[:], rl[:])
            ex = pool.tile([P, R_LOC], BF16, name=f"ex{uid}", tag="ex")
            nc.scalar.activation(ex[:], dm[:], AF.Exp)
            nc.vector.tensor_add(dst[:], rl[:], ex[:])

        with (
            tc.tile_pool(name="p1w", bufs=3) as p1w,
            tc.tile_pool(name="p1vw", bufs=1) as p1vw,
            tc.tile_pool(name="p1ps", bufs=3, space="PSUM") as p1ps,
            tc.tile_pool(name="p1r", bufs=2) as p1r,
        ):
            # K projections (j = 0..15), rope+elu on DVE as tiles land
            for j in range(H):
                w_t = p1w.tile([P, C], BF16, name=f"kw{j}", tag="qkw")
                nc.sync.dma_start(out=w_t[:], in_=qkw[j, :, :])
                ps = p1ps.tile([P, R_LOC], F32, name=f"kp{j}", tag="qkp")
                for k in range(N_KC):
                    nc.tensor.matmul(
                        ps[:], w_t[:, k * P:(k + 1) * P], n1T[k][:],
                        start=(k == 0), stop=(k == N_KC - 1),
                    )
                rope_elu(kres[j], ps, p1r, f"k{j}")

            # V projections, directly transposed: [t x hv] chunks
            with tc.tile_pool(name="p1vps", bufs=3, space="PSUM") as p1vps:
                for half in range(2):
                    vw_sb = []
                    for k in range(N_KC):
                        w_t = p1vw.tile(
                            [P, C // 2], BF16, name=f"vw{half}_{k}", tag=f"vw{k}"
                        )
                        nc.sync.dma_start(out=w_t[:], in_=vw[half, k, :, :])
                        vw_sb.append(w_t)
                    for vb in range(2):
                        vcol = slice(vb * 512, (vb + 1) * 512)
                        for i in range(N_CH):
                            icol = slice(i * P, (i + 1) * P)
                            ps = p1vps.tile(
                                [P, 512], F32, name=f"v{half}_{vb}_{i}", tag="vps"
                            )
                            for k in range(N_KC):
                                nc.tensor.matmul(
                                    ps[:], n1T[k][:, icol], vw_sb[k][:, vcol],
                                    start=(k == 0), stop=(k == N_KC - 1),
                                )
                            for hs in range(4):
                                h = half * 8 + vb * 4 + hs
                                nc.scalar.copy(
                                    vp[h][i][:, 0:HD], ps[:, hs * P:(hs + 1) * P]
                                )
                                nc.vector.memset(vp[h][i][:, HD:HD1], 1.0)

            # ---- phase 2: segment states + masked RS exchange --------
            # (emitted before Q so the collective overlaps Q/scores)
            s_bf = [[None] * N_CH for _ in range(H)]
            with (
                tc.tile_pool(name="p2kp", bufs=4) as p2kp,
                tc.tile_pool(name="p2ps", bufs=2, space="PSUM") as p2ps,
                tc.tile_pool(name="p2run", bufs=1) as p2run,
                tc.tile_pool(name="p2all", bufs=1) as p2all,
                tc.tile_pool(name="p2msk", bufs=2) as p2msk,
            ):
                s_all = p2all.tile([P, SAW], BF16, name="s_all")
                for h in range(H):
                    s_run = p2run.tile([P, HD1], F32, name=f"srun{h}")
                    for i in range(N_CH):
                        tcol = slice(i * P, (i + 1) * P)
                        kp = p2kp.tile([P, P], BF16, name=f"kp{h}_{i}", tag="kp")
                        nc.sync.dma_start_transpose(kp[:], kres[h][:, tcol])
                        sd = p2ps.tile([P, HD1], F32, name=f"sd{h}_{i}", tag="sd")
                        nc.tensor.matmul(sd[:], kp[:], vp[h][i][:], start=True, stop=True)
                        if i == 0:
                            nc.scalar.copy(s_run[:], sd[:])
                        else:
                            nc.vector.tensor_add(s_run[:], s_run[:], sd[:])
                        if i < N_CH - 1:
                            sb = sbf_pool.tile([P, HD1], BF16, name=f"sbf{h}_{i}")
                            nc.scalar.copy(sb[:], s_run[:])
                            s_bf[h][i + 1] = sb
                    nc.scalar.copy(s_all[:, h * HD1:(h + 1) * HD1], s_run[:])
                for s in range(N_CORES):
                    ms = p2msk.tile([P, SAW], BF16, name=f"ms{s}", tag="ms")
                    nc.vector.tensor_scalar_mul(ms[:], s_all[:], smask_sb[:, s:s + 1])
                    nc.sync.dma_start(out=rs_in[s, :, :], in_=ms[:])
                nc.gpsimd.collective_compute(
                    "ReduceScatter",
                    mybir.AluOpType.add,
                    ins=[rs_in.ap().opt()],
                    outs=[rs_out.ap().opt()],
                    replica_groups=groups,
                )

            # Q projections (j = 16..31), overlap the collective
            for j in range(H):
                w_t = p1w.tile([P, C], BF16, name=f"qw{j}", tag="qkw")
                nc.sync.dma_start(out=w_t[:], in_=qkw[H + j, :, :])
                ps = p1ps.tile([P, R_LOC], F32, name=f"qp{j}", tag="qkp")
                for k in range(N_KC):
                    nc.tensor.matmul(
                        ps[:], w_t[:, k * P:(k + 1) * P], n1T[k][:],
                        start=(k == 0), stop=(k == N_KC - 1),
                    )
                rope_elu(qres[j], ps, p1r, f"q{j}")

        # ---- phase 3: local masked scores (no state dependency) ------
        am_ctx = ExitStack()
        am_pool = am_ctx.enter_context(tc.tile_pool(name="amres", bufs=1))
        am = [[None] * N_CH for _ in range(H)]
        with tc.tile_pool(name="p3ps", bufs=3, space="PSUM") as p3ps:
            for h in range(H):
                for i in range(N_CH):
                    tcol = slice(i * P, (i + 1) * P)
                    a_ps = p3ps.tile([P, P], F32, name=f"a{h}_{i}", tag="a")
                    nc.tensor.matmul(
                        a_ps[:], kres[h][:, tcol], qres[h][:, tcol],
                        start=True, stop=True,
                    )
                    am_t = am_pool.tile([P, P], BF16, name=f"am{h}_{i}")
                    nc.vector.tensor_mul(am_t[:], a_ps[:], mask_sb[:])
                    am[h][i] = am_t

        # ---- phase 4: y = (q@S_loc + Am^T@V' + q@S_init) / den; y^T --
        with (
            tc.tile_pool(name="p4si", bufs=1) as p4si,
            tc.tile_pool(name="p4ps", bufs=4, space="PSUM") as p4ps,
            tc.tile_pool(name="p4y", bufs=4) as p4y,
        ):
            sinit = p4si.tile([P, SAW], BF16, name="sinit")
            nc.sync.dma_start(out=sinit[:], in_=rs_out[:, :])
            for h in range(H):
                hcol = slice(h * HD1, (h + 1) * HD1)
                for i in range(N_CH):
                    tcol = slice(i * P, (i + 1) * P)
                    y_ps = p4ps.tile([P, HD1], F32, name=f"y{h}_{i}", tag="y")
                    if i > 0:
                        nc.tensor.matmul(
                            y_ps[:], qres[h][:, tcol], s_bf[h][i][:],
                            start=True, stop=False,
                        )
                    nc.tensor.matmul(
                        y_ps[:], am[h][i][:], vp[h][i][:],
                        start=(i == 0), stop=False,
                    )
                    nc.tensor.matmul(
                        y_ps[:], qres[h][:, tcol], sinit[:, hcol],
                        start=False, stop=True,
                    )
                    rec = p4y.tile([P, 1], F32, name=f"rec{h}_{i}", tag="rec")
                    nc.vector.reciprocal(rec[:], y_ps[:, HD:HD1])
                    yb = p4y.tile([P, HD], BF16, name=f"yb{h}_{i}", tag="yb")
                    nc.vector.tensor_scalar_mul(yb[:], y_ps[:, 0:HD], rec[:])
                    nc.sync.dma_start_transpose(yT[h][i][:], yb[:])
        am_ctx.close()
        att_ctx.close()

        # ---- phase 5: proj, residual, rmsnorm2 -> n2T ----------------
        x2_ctx = ExitStack()
        x2_pool = x2_ctx.enter_context(tc.tile_pool(name="x2res", bufs=1))
        x2_res = [x2_pool.tile([P, C], F32, name=f"x2_{i}") for i in range(N_RT)]
        n2T_ctx = ExitStack()
        n2T_pool = n2T_ctx.enter_context(tc.tile_pool(name="n2T", bufs=1))
        n2T = [n2T_pool.tile([P, R_LOC], BF16, name=f"n2T{k}") for k in range(N_KC)]
        with (
            tc.tile_pool(name="p5w", bufs=4) as p5w,
            tc.tile_pool(name="p5ps", bufs=5, space="PSUM") as p5ps,
            tc.tile_pool(name="p5sq", bufs=1) as p5sq,
            tc.tile_pool(name="p5st", bufs=8) as p5st,
            tc.tile_pool(name="p5n", bufs=2) as p5n,
            tc.tile_pool(name="p5tr", bufs=2, space="PSUM") as p5tr,
        ):
            for mt in range(N_RT):
                nc.sync.dma_start(
                    out=x2_res[mt][:], in_=x_loc[mt * P:(mt + 1) * P, :]
                )
            for ont in range(4):
                ocol = slice(ont * 512, (ont + 1) * 512)
                ps_mt = [
                    p5ps.tile([P, 512], F32, name=f"h{ont}_{mt}", tag="h")
                    for mt in range(N_RT)
                ]
                for kd in range(N_KC):
                    w_t = p5w.tile([P, 512], BF16, name=f"pw{ont}_{kd}", tag="pw")
                    nc.sync.dma_start(out=w_t[:], in_=pw[kd, :, ocol])
                    for mt in range(N_RT):
                        nc.tensor.matmul(
                            ps_mt[mt][:], yT[kd][mt][:], w_t[:],
                            start=(kd == 0), stop=(kd == N_KC - 1),
                        )
                for mt in range(N_RT):
                    nc.vector.tensor_add(
                        x2_res[mt][:, ocol], x2_res[mt][:, ocol], ps_mt[mt][:]
                    )
            for mt in range(N_RT):
                sq = p5sq.tile([P, C], F32, name=f"sq2_{mt}", tag="sq2")
                ss = p5st.tile([P, 1], F32, name=f"ss2_{mt}", tag="ss2")
                nc.scalar.activation(sq[:], x2_res[mt][:], AF.Square, accum_out=ss[:])
                rms = p5st.tile([P, 1], F32, name=f"rms2_{mt}", tag="rms2")
                nc.scalar.activation(
                    rms[:], ss[:], AF.Sqrt, bias=eps_t[:], scale=1.0 / C
                )
                inv = p5st.tile([P, 1], F32, name=f"inv2_{mt}", tag="inv2")
                nc.vector.reciprocal(inv[:], rms[:])
                n_t = p5n.tile([P, C], F32, name=f"n2_{mt}", tag="n2")
                nc.vector.tensor_scalar_mul(n_t[:], x2_res[mt][:], inv[:])
                for k in range(N_KC):
                    ps = p5tr.tile([P, P], F32, name=f"tr2_{mt}_{k}", tag="tr2")
                    nc.tensor.transpose(ps[:], n_t[:, k * P:(k + 1) * P], ident_f32[:])
                    nc.scalar.copy(n2T[k][:, mt * P:(mt + 1) * P], ps[:])
        yT_ctx.close()

        # ---- phase 6: fc + gelu -> gT (resident) ---------------------
        gT_ctx = ExitStack()
        gT_pool = gT_ctx.enter_context(tc.tile_pool(name="gT", bufs=1))
        gT = [gT_pool.tile([P, R_LOC], BF16, name=f"gT{mf}") for mf in range(N_MF)]
        with (
            tc.tile_pool(name="p6w", bufs=3) as p6w,
            tc.tile_pool(name="p6ps", bufs=3, space="PSUM") as p6ps,
        ):
            for mf in range(N_MF):
                w_t = p6w.tile([P, C], BF16, name=f"fcw{mf}", tag="fcw")
                nc.sync.dma_start(out=w_t[:], in_=fcw[mf, :, :])
                ps = p6ps.tile([P, R_LOC], F32, name=f"g{mf}", tag="g")
                for k in range(N_KC):
                    nc.tensor.matmul(
                        ps[:], w_t[:, k * P:(k + 1) * P], n2T[k][:],
                        start=(k == 0), stop=(k == N_KC - 1),
                    )
                nc.scalar.activation(gT[mf][:], ps[:], AF.Gelu)

        # ---- phase 7: mlp proj + residual -> out ---------------------
        with (
            tc.tile_pool(name="p7w", bufs=2) as p7w,
            tc.tile_pool(name="p7ps", bufs=4, space="PSUM") as p7ps,
            tc.tile_pool(name="p7o", bufs=4) as p7o,
        ):
            for ch in range(N_MCH):
                w_t = p7w.tile([P, N_MF * MLP_CC], BF16, name=f"mw{ch}", tag="mw")
                nc.sync.dma_start(out=w_t[:], in_=mww[ch, :, :])
                for mt in range(N_RT):
                    mcol = slice(mt * P, (mt + 1) * P)
                    ps = p7ps.tile([P, MLP_CC], F32, name=f"f{ch}_{mt}", tag="f")
                    for kf in range(N_MF):
                        nc.tensor.matmul(
                            ps[:],
                            gT[kf][:, mcol],
                            w_t[:, kf * MLP_CC:(kf + 1) * MLP_CC],
                            start=(kf == 0), stop=(kf == N_MF - 1),
                        )
                    o_t = p7o.tile([P, MLP_CC], F32, name=f"o{ch}_{mt}", tag="o")
                    nc.vector.tensor_add(
                        o_t[:],
                        x2_res[mt][:, ch * MLP_CC:(ch + 1) * MLP_CC],
                        ps[:],
                    )
                    nc.scalar.dma_start(
                        out=out_loc[
                            mt * P:(mt + 1) * P,
                            ch * MLP_CC:(ch + 1) * MLP_CC,
                        ],
                        in_=o_t[:],
                    )
        gT_ctx.close()
        n2T_ctx.close()
        x2_ctx.close()
        stk.close()

    return nc


_NC_CACHE = None


def _get_nc():
    global _NC_CACHE
    if _NC_CACHE is None:
        _NC_CACHE = build_nc()
    return _NC_CACHE


def _prep_inputs(x, cos, sin, attention_bias, norm1_w, norm2_w, attn_w, proj_w,
                 fc_w, mlp_proj_w):
    bf = ml_dtypes.bfloat16
    xf = np.asarray(x, np.float32).reshape(R, C)
    w1 = np.asarray(norm1_w, np.float32)
    w2 = np.asarray(norm2_w, np.float32)
    aw = np.asarray(attn_w, np.float32) * w1[None, :]      # [3C, C] (norm folded)
    pwf = np.asarray(proj_w, np.float32)                   # [C, C]
    fwf = np.asarray(fc_w, np.float32) * w2[None, :]       # [F, C]
    mwf = np.asarray(mlp_proj_w, np.float32)               # [C, F]
    cosf = np.asarray(cos, np.float32)                     # [T, 64]
    sinf = np.asarray(sin, np.float32)

    awr = aw.reshape(H, 3, HD, C)
    # qkw[j<H] = K-weights of head j; qkw[j>=H] = Q-weights of head j-H.
    # qkw[j, p, k*128+m] = awr[h, comp, m, k*128+p]
    qk = np.empty((2 * H, P, C), np.float32)
    for h in range(H):
        qk[h] = awr[h, 1].T.reshape(N_KC, P, HD).transpose(1, 0, 2).reshape(P, C)
        qk[H + h] = awr[h, 0].T.reshape(N_KC, P, HD).transpose(1, 0, 2).reshape(P, C)
    # vw[half, k, p, (h-8*half)*128+d] = awr[h, 2, d, k*128+p]
    vwt = (
        awr[:, 2].reshape(H * HD, C).T.reshape(N_KC, P, 2, C // 2)
        .transpose(2, 0, 1, 3)
    )
    # pw[kd, p, co] = proj_w[co, kd*128+p]
    pwt = pwf.T.reshape(N_KC, P, C)
    # fcw[mf, p, k*128+f] = fwf[mf*128+f, k*128+p]
    fct = np.ascontiguousarray(
        fwf.reshape(N_MF, P, N_KC, P).transpose(0, 3, 2, 1)
    ).reshape(N_MF, P, C)
    # mww[ch, p, kf*CC+c] = mwf[ch*CC+c, kf*128+p]
    mwt = np.ascontiguousarray(
        mwf.reshape(N_MCH, MLP_CC, N_MF, P).transpose(0, 3, 2, 1)
    ).reshape(N_MCH, P, N_MF * MLP_CC)

    qk_b = np.ascontiguousarray(qk).astype(bf)
    vw_b = np.ascontiguousarray(vwt).astype(bf)
    pw_b = np.ascontiguousarray(pwt).astype(bf)
    fc_b = fct.astype(bf)
    mw_b = mwt.astype(bf)
    # mask[s, t] = 1 iff s <= t  (transposed causal tril)
    maskT = np.triu(np.ones((P, P), np.float32))

    in_maps = []
    for c in range(N_CORES):
        t0 = (c % (N_CORES // B)) * R_LOC
        sm = np.zeros((P, N_CORES), np.float32)
        for s in range(N_CORES):
            if s // (N_CORES // B) == c // (N_CORES // B) and s > c:
                sm[:, s] = 1.0
        in_maps.append({
            "x_loc": np.ascontiguousarray(xf[R_LOC * c:R_LOC * (c + 1)]),
            "cosr": np.ascontiguousarray(cosf[t0:t0 + R_LOC].T).astype(bf),
            "sinr": np.ascontiguousarray(sinf[t0:t0 + R_LOC].T).astype(bf),
            "maskT": maskT,
            "smask": sm,
            "qkw": qk_b,
            "vw": vw_b,
            "pw": pw_b,
            "fcw": fc_b,
            "mww": mw_b,
        })
    return in_maps


def kernel(**inputs):
    nc = _get_nc()
    in_maps = _prep_inputs(**inputs)
    res = run_bass_kernel_spmd(nc, in_maps, list(range(N_CORES)))
    out = np.concatenate(
        [np.asarray(res.results[c]["out_loc"], np.float32) for c in range(N_CORES)],
        axis=0,
    )
    return out.reshape(B, T, C)


# revision 9
# speedup vs baseline: 1.8465x; 1.0217x over previous
"""Trainium2 Bass kernel for nn_Block_42460046688864 (dense transformer block).

Reference math (B=2, T=2048, C=2048, H=16, HD=128):
    n1  = rmsnorm(x) * norm1_w
    qkv = n1 @ attn_w.T ; q,k,v per head ; q,k = rope(q,k) ; phi = elu(.)+1
    w   = (phi_q . phi_k) * scale * tril ; w /= sum(w) ; y = w @ v
    h   = y @ proj_w.T ; x2 = x + h
    ffn = gelu(rmsnorm(x2)*norm2_w @ fc_w.T) @ mlp_proj_w.T ; out = x2 + ffn

Distribution (8 NeuronCores, one NEFF, fully data-parallel):
  - rows (b*T+t, 4096 total) sharded 512/core; every core streams the FULL
    weights from its own HBM (no activation collectives at all).
  - attention is chunked linear attention (causal tril + positive elu+1
    features == prefix-state form; scale and eps cancel to ~1e-9 rel).
    The only cross-core dependency is the causal prefix state: each core's
    segment state S_seg[h] = sum_t k_t (x) [v_t | 1] is exchanged with ONE
    small ReduceScatter. Core j writes S_seg * mask[j<s, same-seq] into
    slot s, so after the add-RS core s holds exactly the sum of its
    same-sequence predecessors' states (its causal init state). The RS is
    issued right after K/V are ready and overlaps the Q projection; the
    correction q @ S_init is fused into each chunk's PSUM accumulation.
  - V is computed directly in [token, dim] layout by using n1^T chunks as
    the stationary matmul operand (no V transposes); K additionally needs
    [token, dim] for the state outer products -> 64 small DMA transposes.

Notes:
  - norm weights are folded into attn_w / fc_w on the host (exact algebra).
  - matmul operands are bf16 (fp32 PSUM accumulation); norms, residuals and
    attention numerators/denominators stay fp32 (psum) end to end.
  - weights are pre-tiled on the host into [128 x N] DMA slabs so every
    weight DMA is one contiguous >=2KB-per-partition block.
  - SBUF pools are strict LIFO per side; long-lived attention tiles live on
    the left stack, y^T on the right stack so lifetimes nest.
  - TileContext's tail drain is patched to split its semaphore waits:
    this walrus build rejects >2 sync waits on one TPB_CTRL instruction.
"""

from contextlib import ExitStack

import numpy as np
import ml_dtypes

import concourse.bass as bass
import concourse.mybir as mybir
import concourse.tile as tile
from concourse.bass_utils import run_bass_kernel_spmd
from concourse.masks import make_identity
from bass_rust import ScopedClock

F32 = mybir.dt.float32
BF16 = mybir.dt.bfloat16
AF = mybir.ActivationFunctionType

N_CORES = 8
B, T, C, H, HD = 2, 2048, 2048, 16, 128
F = 4 * C                  # 8192 mlp hidden
R = B * T                  # 4096 flattened rows (b-major)
R_LOC = R // N_CORES       # 512 rows per core
P = 128
EPS_NORM = 1e-5
N_RT = R_LOC // P          # 4 local row tiles
N_KC = C // P              # 16 contraction tiles over C
N_CH = N_RT                # 4 local causal chunks
N_MF = F // P              # 64 mlp-hidden tiles
HD1 = HD + 1               # state cols: [v dims | 1]
SAW = H * HD1              # 2064 = all-head state cols
MLP_CC = 256               # mlp output col-chunk
N_MCH = C // MLP_CC        # 8 col chunks

_MAX_WAITS = 1  # this walrus build rejects multi-wait instructions


def _split_excess_waits(nc):
    """Move excess semaphore waits onto same-engine NoOps ahead of the op."""
    for fn in nc.m.functions:
        for bb in fn.blocks:
            insts = list(bb.instructions)
            out = []
            for ins in insts:
                si = getattr(ins, "sync_info", None)
                waits = list(si.on_wait) if si and si.on_wait else []
                sem_waits = [w for w in waits if w.sync_type == "semaphore"]
                if len(sem_waits) > _MAX_WAITS:
                    keep = [w for w in waits if w.sync_type != "semaphore"]
                    keep += sem_waits[: _MAX_WAITS - 1] if _MAX_WAITS > 1 else []
                    extra = sem_waits[_MAX_WAITS - 1:] if _MAX_WAITS > 1 else sem_waits
                    for j in range(0, len(extra), _MAX_WAITS):
                        chunk = extra[j:j + _MAX_WAITS]
                        nop = mybir.InstNoOp(
                            name=nc.get_next_instruction_name(), ins=[], outs=[]
                        )
                        nop.engine = ins.engine
                        nop.sync_info = mybir.SyncInfo(on_wait=chunk, on_update=[])
                        out.append(nop)
                    si.on_wait[:] = keep
                out.append(ins)
            if len(out) != len(insts):
                bb.instructions[:] = out


class _TC(tile.TileContext):
    """TileContext whose tail drain splits sem waits one-per-NOP."""

    def schedule_and_allocate(self):
        ret = super().schedule_and_allocate()
        _split_excess_waits(self.nc)
        return ret

    def _drain_and_barrier(self, tick_clock, wait_clock):
        probe = self.nc.sync.nop(nofuse=True, hint="drain_waits")
        wait_clock.add_sem_waits(
            probe.ins, ScopedClock({None: tick_clock.global_clock})
        )
        si = probe.ins.sync_info
        waits = list(si.on_wait) if si and si.on_wait else []
        if len(waits) > 1:
            si.on_wait[:] = waits[:1]
            for w in waits[1:]:
                extra = self.nc.sync.nop(nofuse=True, hint="drain_waits")
                extra.ins.sync_info = mybir.SyncInfo(on_wait=[w], on_update=[])
        self.nc.sync.drain()
        self.nc.all_engine_barrier()
        popped = self.nc._tile_sem_poison_stack.pop()
        assert popped is self._sem_poison
        self.nc.clear_and_free_semaphores(list(self.sems.allocated().values()))
        self.nc.all_engine_barrier()


def build_nc():
    nc = bass.Bass(target_bir_lowering=False)

    x_loc = nc.declare_dram_parameter("x_loc", [R_LOC, C], F32, isOutput=False)
    cosr = nc.declare_dram_parameter("cosr", [HD // 2, R_LOC], BF16, isOutput=False)
    sinr = nc.declare_dram_parameter("sinr", [HD // 2, R_LOC], BF16, isOutput=False)
    maskT = nc.declare_dram_parameter("maskT", [P, P], F32, isOutput=False)
    smask = nc.declare_dram_parameter("smask", [P, N_CORES], F32, isOutput=False)
    # pre-tiled weight slabs (see _prep_inputs for layouts)
    qkw = nc.declare_dram_parameter("qkw", [2 * H, P, C], BF16, isOutput=False)
    vw = nc.declare_dram_parameter("vw", [2, N_KC, P, C // 2], BF16, isOutput=False)
    pw = nc.declare_dram_parameter("pw", [4, P, N_KC * 512], BF16, isOutput=False)
    fcw = nc.declare_dram_parameter("fcw", [N_MF, P, C], BF16, isOutput=False)
    mww = nc.declare_dram_parameter(
        "mww", [N_MCH, P, N_MF * MLP_CC], BF16, isOutput=False
    )
    out_loc = nc.declare_dram_parameter("out_loc", [R_LOC, C], F32, isOutput=True)

    rs_in = nc.dram_tensor("rs_in", [N_CORES, P, SAW], BF16)
    rs_out = nc.dram_tensor("rs_out", [P, SAW], BF16)

    groups = [list(range(N_CORES))]

    with _TC(nc) as tc:
        stk = ExitStack()
        const = stk.enter_context(tc.tile_pool(name="const", bufs=1))
        ident_f32 = const.tile([P, P], F32)
        make_identity(nc, ident_f32)
        ident_bf = const.tile([P, P], BF16)
        make_identity(nc, ident_bf)
        mask_sb = const.tile([P, P], F32)
        nc.sync.dma_start(out=mask_sb[:], in_=maskT[:, :])
        smask_sb = const.tile([P, N_CORES], F32)
        nc.sync.dma_start(out=smask_sb[:], in_=smask[:, :])
        eps_t = const.tile([P, 1], F32)
        nc.vector.memset(eps_t[:], EPS_NORM)
        cos_sb = const.tile([HD // 2, R_LOC], BF16)
        sin_sb = const.tile([HD // 2, R_LOC], BF16)
        nc.sync.dma_start(out=cos_sb[:], in_=cosr[:, :])
        nc.sync.dma_start(out=sin_sb[:], in_=sinr[:, :])

        # y^T on the right stack: outlives the attention residents (left).
        yT_ctx = ExitStack()
        yT_pool = yT_ctx.enter_context(tc.tile_pool(name="yT", bufs=1, side="right"))
        yT = [
            [yT_pool.tile([P, P], BF16, name=f"yT{h}_{i}") for i in range(N_CH)]
            for h in range(H)
        ]

        # attention residents (left): released together after phase 4.
        att_ctx = ExitStack()
        n1T_pool = att_ctx.enter_context(tc.tile_pool(name="n1T", bufs=1))
        qk_pool = att_ctx.enter_context(tc.tile_pool(name="qkres", bufs=1))
        vp_pool = att_ctx.enter_context(tc.tile_pool(name="vpres", bufs=1))
        sbf_pool = att_ctx.enter_context(tc.tile_pool(name="sbfres", bufs=1))
        n1T = [n1T_pool.tile([P, R_LOC], BF16, name=f"n1T{k}") for k in range(N_KC)]
        kres = [qk_pool.tile([P, R_LOC], BF16, name=f"k{h}") for h in range(H)]
        qres = [qk_pool.tile([P, R_LOC], BF16, name=f"q{h}") for h in range(H)]
        vp = [
            [vp_pool.tile([P, HD1], BF16, name=f"vp{h}_{i}") for i in range(N_CH)]
            for h in range(H)
        ]

        # ---- phase 0: rmsnorm(x) -> n1T (SBUF-resident, transposed) ----
        with (
            tc.tile_pool(name="p0x", bufs=2) as p0x,
            tc.tile_pool(name="p0sq", bufs=1) as p0sq,
            tc.tile_pool(name="p0st", bufs=8) as p0st,
            tc.tile_pool(name="p0n", bufs=1) as p0n,
            tc.tile_pool(name="p0tr", bufs=2, space="PSUM") as p0tr,
        ):
            n_ts = []
            for i in range(N_RT):
                x_t = p0x.tile([P, C], F32, name=f"x{i}", tag="x")
                nc.sync.dma_start(out=x_t[:], in_=x_loc[i * P:(i + 1) * P, :])
                sq = p0sq.tile([P, C], F32, name=f"sq{i}", tag="sq")
                ss = p0st.tile([P, 1], F32, name=f"ss{i}", tag="ss")
                nc.scalar.activation(sq[:], x_t[:], AF.Square, accum_out=ss[:])
                rms = p0st.tile([P, 1], F32, name=f"rms{i}", tag="rms")
                nc.scalar.activation(
                    rms[:], ss[:], AF.Sqrt, bias=eps_t[:], scale=1.0 / C
                )
                inv = p0st.tile([P, 1], F32, name=f"inv{i}", tag="inv")
                nc.vector.reciprocal(inv[:], rms[:])
                n_t = p0n.tile([P, C], F32, name=f"n{i}", tag=f"n{i}")
                nc.vector.tensor_scalar_mul(n_t[:], x_t[:], inv[:])
                n_ts.append(n_t)
            for k in range(N_KC):
                ps = p0tr.tile([P, R_LOC], F32, name=f"tr{k}", tag="tr")
                for i in range(N_RT):
                    nc.tensor.transpose(
                        ps[:, i * P:(i + 1) * P],
                        n_ts[i][:, k * P:(k + 1) * P], ident_f32[:],
                    )
                nc.scalar.copy(n1T[k][:], ps[:])

        # ---- phase 1: K then V then Q projections --------------------
        HF = HD // 2

        def rope_elu(dst, ps, pool, uid):
            """psum [128 x 512] (d-major head tile) -> phi(rope(.)) bf16."""
            raw = pool.tile([P, R_LOC], BF16, name=f"raw{uid}", tag="raw")
            nc.scalar.copy(raw[:], ps[:])
            s1 = pool.tile([HF, R_LOC], BF16, name=f"s1{uid}", tag="s1")
            s2 = pool.tile([HF, R_LOC], BF16, name=f"s2{uid}", tag="s2")
            ro = pool.tile([P, R_LOC], BF16, name=f"ro{uid}", tag="ro")
            nc.vector.tensor_mul(s1[:], raw[0:HF, :], cos_sb[:])
            nc.vector.tensor_mul(s2[:], raw[HF:P, :], sin_sb[:])
            nc.vector.tensor_sub(ro[0:HF, :], s1[:], s2[:])
            nc.vector.tensor_mul(s1[:], raw[0:HF, :], sin_sb[:])
            nc.vector.tensor_mul(s2[:], raw[HF:P, :], cos_sb[:])
            nc.vector.tensor_add(ro[HF:P, :], s1[:], s2[:])
            # phi = elu(ro)+1 = relu(ro) + exp(ro - relu(ro))
            rl = pool.tile([P, R_LOC], BF16, name=f"rl{uid}", tag="rl")
            nc.scalar.activation(rl[:], ro[:], AF.Relu)
            dm = pool.tile([P, R_LOC], BF16, name=f"dm{uid}", tag="dm")
            nc.vector.tensor_sub(dm[:], ro[:], rl[:])
            ex = pool.tile([P, R_LOC], BF16, name=f"ex{uid}", tag="ex")
            nc.scalar.activation(ex[:], dm[:], AF.Exp)
            nc.vector.tensor_add(dst[:], rl[:], ex[:])

        with (
            tc.tile_pool(name="p1w", bufs=3) as p1w,
            tc.tile_pool(name="p1vw", bufs=1) as p1vw,
            tc.tile_pool(name="p1ps", bufs=5, space="PSUM") as p1ps,
            tc.tile_pool(name="p1r", bufs=3) as p1r,
        ):
            # K projections (j = 0..15), rope+elu on DVE as tiles land
            for j in range(H):
                w_t = p1w.tile([P, C], BF16, name=f"kw{j}", tag="qkw")
                nc.sync.dma_start(out=w_t[:], in_=qkw[j, :, :])
                ps = p1ps.tile([P, R_LOC], F32, name=f"kp{j}", tag="qkp")
                for k in range(N_KC):
                    nc.tensor.matmul(
                        ps[:], w_t[:, k * P:(k + 1) * P], n1T[k][:],
                        start=(k == 0), stop=(k == N_KC - 1),
                    )
                rope_elu(kres[j], ps, p1r, f"k{j}")

            # V projections, directly transposed: [t x hv] chunks
            with tc.tile_pool(name="p1vps", bufs=3, space="PSUM") as p1vps:
                for half in range(2):
                    vw_sb = []
                    for k in range(N_KC):
                        w_t = p1vw.tile(
                            [P, C // 2], BF16, name=f"vw{half}_{k}", tag=f"vw{k}"
                        )
                        nc.sync.dma_start(out=w_t[:], in_=vw[half, k, :, :])
                        vw_sb.append(w_t)
                    for vb in range(2):
                        vcol = slice(vb * 512, (vb + 1) * 512)
                        for i in range(N_CH):
                            icol = slice(i * P, (i + 1) * P)
                            ps = p1vps.tile(
                                [P, 512], F32, name=f"v{half}_{vb}_{i}", tag="vps"
                            )
                            for k in range(N_KC):
                                nc.tensor.matmul(
                                    ps[:], n1T[k][:, icol], vw_sb[k][:, vcol],
                                    start=(k == 0), stop=(k == N_KC - 1),
                                )
                            for hs in range(4):
                                h = half * 8 + vb * 4 + hs
                                nc.scalar.copy(
                                    vp[h][i][:, 0:HD], ps[:, hs * P:(hs + 1) * P]
                                )
                                nc.vector.memset(vp[h][i][:, HD:HD1], 1.0)

            # ---- phase 2: segment states + masked RS exchange --------
            # (emitted before Q so the collective overlaps Q/scores)
            s_bf = [[None] * N_CH for _ in range(H)]
            with (
                tc.high_priority(),
                tc.tile_pool(name="p2kp", bufs=4) as p2kp,
                tc.tile_pool(name="p2ps", bufs=2, space="PSUM") as p2ps,
                tc.tile_pool(name="p2run", bufs=1) as p2run,
                tc.tile_pool(name="p2all", bufs=1) as p2all,
                tc.tile_pool(name="p2msk", bufs=2) as p2msk,
            ):
                s_all = p2all.tile([P, SAW], BF16, name="s_all")
                for h in range(H):
                    s_run = p2run.tile([P, HD1], F32, name=f"srun{h}")
                    for i in range(N_CH):
                        tcol = slice(i * P, (i + 1) * P)
                        kp = p2kp.tile([P, P], BF16, name=f"kp{h}_{i}", tag="kp")
                        nc.sync.dma_start_transpose(kp[:], kres[h][:, tcol])
                        sd = p2ps.tile([P, HD1], F32, name=f"sd{h}_{i}", tag="sd")
                        nc.tensor.matmul(sd[:], kp[:], vp[h][i][:], start=True, stop=True)
                        if i == 0:
                            nc.scalar.copy(s_run[:], sd[:])
                        else:
                            nc.vector.tensor_add(s_run[:], s_run[:], sd[:])
                        if i < N_CH - 1:
                            sb = sbf_pool.tile([P, HD1], BF16, name=f"sbf{h}_{i}")
                            nc.scalar.copy(sb[:], s_run[:])
                            s_bf[h][i + 1] = sb
                    nc.scalar.copy(s_all[:, h * HD1:(h + 1) * HD1], s_run[:])
                for s in range(N_CORES):
                    ms = p2msk.tile([P, SAW], BF16, name=f"ms{s}", tag="ms")
                    nc.vector.tensor_scalar_mul(ms[:], s_all[:], smask_sb[:, s:s + 1])
                    nc.sync.dma_start(out=rs_in[s, :, :], in_=ms[:])
                nc.gpsimd.collective_compute(
                    "ReduceScatter",
                    mybir.AluOpType.add,
                    ins=[rs_in.ap().opt()],
                    outs=[rs_out.ap().opt()],
                    replica_groups=groups,
                )

            # Q projections (j = 16..31), overlap the collective
            for j in range(H):
                w_t = p1w.tile([P, C], BF16, name=f"qw{j}", tag="qkw")
                nc.sync.dma_start(out=w_t[:], in_=qkw[H + j, :, :])
                ps = p1ps.tile([P, R_LOC], F32, name=f"qp{j}", tag="qkp")
                for k in range(N_KC):
                    nc.tensor.matmul(
                        ps[:], w_t[:, k * P:(k + 1) * P], n1T[k][:],
                        start=(k == 0), stop=(k == N_KC - 1),
                    )
                rope_elu(qres[j], ps, p1r, f"q{j}")

        # ---- phase 3: local masked scores (no state dependency) ------
        am_ctx = ExitStack()
        am_pool = am_ctx.enter_context(tc.tile_pool(name="amres", bufs=1))
        am = [[None] * N_CH for _ in range(H)]
        with tc.tile_pool(name="p3ps", bufs=3, space="PSUM") as p3ps:
            for h in range(H):
                for i in range(N_CH):
                    tcol = slice(i * P, (i + 1) * P)
                    a_ps = p3ps.tile([P, P], F32, name=f"a{h}_{i}", tag="a")
                    nc.tensor.matmul(
                        a_ps[:], kres[h][:, tcol], qres[h][:, tcol],
                        start=True, stop=True,
                    )
                    am_t = am_pool.tile([P, P], BF16, name=f"am{h}_{i}")
                    nc.vector.tensor_mul(am_t[:], a_ps[:], mask_sb[:])
                    am[h][i] = am_t

        # ---- phase 4: y = (q@S_loc + Am^T@V' + q@S_init) / den; y^T --
        with (
            tc.tile_pool(name="p4si", bufs=1) as p4si,
            tc.tile_pool(name="p4ps", bufs=4, space="PSUM") as p4ps,
            tc.tile_pool(name="p4tr", bufs=3, space="PSUM") as p4tr,
            tc.tile_pool(name="p4y", bufs=4) as p4y,
        ):
            sinit = p4si.tile([P, SAW], BF16, name="sinit")
            with tc.high_priority():
                nc.sync.dma_start(out=sinit[:], in_=rs_out[:, :])
            for h in range(H):
                hcol = slice(h * HD1, (h + 1) * HD1)
                for i in range(N_CH):
                    tcol = slice(i * P, (i + 1) * P)
                    y_ps = p4ps.tile([P, HD1], F32, name=f"y{h}_{i}", tag="y")
                    if i > 0:
                        nc.tensor.matmul(
                            y_ps[:], qres[h][:, tcol], s_bf[h][i][:],
                            start=True, stop=False,
                        )
                    nc.tensor.matmul(
                        y_ps[:], am[h][i][:], vp[h][i][:],
                        start=(i == 0), stop=False,
                    )
                    nc.tensor.matmul(
                        y_ps[:], qres[h][:, tcol], sinit[:, hcol],
                        start=False, stop=True,
                    )
                    rec = p4y.tile([P, 1], F32, name=f"rec{h}_{i}", tag="rec")
                    nc.vector.reciprocal(rec[:], y_ps[:, HD:HD1])
                    yb = p4y.tile([P, HD], BF16, name=f"yb{h}_{i}", tag="yb")
                    nc.scalar.activation(
                        yb[:], y_ps[:, 0:HD], AF.Identity, scale=rec[:]
                    )
                    tr = p4tr.tile([P, P], BF16, name=f"ytr{h}_{i}", tag="ytr")
                    nc.tensor.transpose(tr[:], yb[:], ident_bf[:])
                    nc.scalar.copy(yT[h][i][:], tr[:])
        am_ctx.close()
        att_ctx.close()

        # ---- phase 5: proj, residual, rmsnorm2 -> n2T ----------------
        x2_ctx = ExitStack()
        x2_pool = x2_ctx.enter_context(tc.tile_pool(name="x2res", bufs=1))
        x2_res = [x2_pool.tile([P, C], F32, name=f"x2_{i}") for i in range(N_RT)]
        n2T_ctx = ExitStack()
        n2T_pool = n2T_ctx.enter_context(tc.tile_pool(name="n2T", bufs=1))
        n2T = [n2T_pool.tile([P, R_LOC], BF16, name=f"n2T{k}") for k in range(N_KC)]
        with (
            tc.tile_pool(name="p5w", bufs=2) as p5w,
            tc.tile_pool(name="p5ps", bufs=5, space="PSUM") as p5ps,
            tc.tile_pool(name="p5sq", bufs=1) as p5sq,
            tc.tile_pool(name="p5st", bufs=8) as p5st,
            tc.tile_pool(name="p5n", bufs=1) as p5n,
            tc.tile_pool(name="p5tr", bufs=2, space="PSUM") as p5tr,
        ):
            for mt in range(N_RT):
                nc.sync.dma_start(
                    out=x2_res[mt][:], in_=x_loc[mt * P:(mt + 1) * P, :]
                )
            for ont in range(4):
                ocol = slice(ont * 512, (ont + 1) * 512)
                w_t = p5w.tile([P, N_KC * 512], BF16, name=f"pw{ont}", tag="pw")
                nc.sync.dma_start(out=w_t[:], in_=pw[ont, :, :])
                ps_mt = [
                    p5ps.tile([P, 512], F32, name=f"h{ont}_{mt}", tag="h")
                    for mt in range(N_RT)
                ]
                for kd in range(N_KC):
                    for mt in range(N_RT):
                        nc.tensor.matmul(
                            ps_mt[mt][:], yT[kd][mt][:],
                            w_t[:, kd * 512:(kd + 1) * 512],
                            start=(kd == 0), stop=(kd == N_KC - 1),
                        )
                for mt in range(N_RT):
                    nc.vector.tensor_add(
                        x2_res[mt][:, ocol], x2_res[mt][:, ocol], ps_mt[mt][:]
                    )
            n2_ts = []
            for mt in range(N_RT):
                sq = p5sq.tile([P, C], F32, name=f"sq2_{mt}", tag="sq2")
                ss = p5st.tile([P, 1], F32, name=f"ss2_{mt}", tag="ss2")
                nc.scalar.activation(sq[:], x2_res[mt][:], AF.Square, accum_out=ss[:])
                rms = p5st.tile([P, 1], F32, name=f"rms2_{mt}", tag="rms2")
                nc.scalar.activation(
                    rms[:], ss[:], AF.Sqrt, bias=eps_t[:], scale=1.0 / C
                )
                inv = p5st.tile([P, 1], F32, name=f"inv2_{mt}", tag="inv2")
                nc.vector.reciprocal(inv[:], rms[:])
                n_t = p5n.tile([P, C], F32, name=f"n2_{mt}", tag=f"n2_{mt}")
                nc.vector.tensor_scalar_mul(n_t[:], x2_res[mt][:], inv[:])
                n2_ts.append(n_t)
            for k in range(N_KC):
                ps = p5tr.tile([P, R_LOC], F32, name=f"tr2_{k}", tag="tr2")
                for mt in range(N_RT):
                    nc.tensor.transpose(
                        ps[:, mt * P:(mt + 1) * P],
                        n2_ts[mt][:, k * P:(k + 1) * P], ident_f32[:],
                    )
                nc.scalar.copy(n2T[k][:], ps[:])
        yT_ctx.close()

        # ---- phase 6: fc + gelu -> gT (resident) ---------------------
        gT_ctx = ExitStack()
        gT_pool = gT_ctx.enter_context(tc.tile_pool(name="gT", bufs=1))
        gT = [gT_pool.tile([P, R_LOC], BF16, name=f"gT{mf}") for mf in range(N_MF)]
        p7w_ctx = ExitStack()
        p7w = p7w_ctx.enter_context(tc.tile_pool(name="p7w", bufs=2))
        with (
            tc.tile_pool(name="p6w", bufs=2) as p6w,
            tc.tile_pool(name="p6ps", bufs=3, space="PSUM") as p6ps,
        ):
            for mf in range(N_MF):
                w_t = p6w.tile([P, C], BF16, name=f"fcw{mf}", tag="fcw")
                nc.sync.dma_start(out=w_t[:], in_=fcw[mf, :, :])
                ps = p6ps.tile([P, R_LOC], F32, name=f"g{mf}", tag="g")
                for k in range(N_KC):
                    nc.tensor.matmul(
                        ps[:], w_t[:, k * P:(k + 1) * P], n2T[k][:],
                        start=(k == 0), stop=(k == N_KC - 1),
                    )
                nc.scalar.activation(gT[mf][:], ps[:], AF.Gelu)

        # ---- phase 7: mlp proj + residual -> out ---------------------
        with (
            tc.tile_pool(name="p7ps", bufs=4, space="PSUM") as p7ps,
            tc.tile_pool(name="p7o", bufs=4) as p7o,
        ):
            for ch in range(N_MCH):
                w_t = p7w.tile([P, N_MF * MLP_CC], BF16, name=f"mw{ch}", tag="mw")
                nc.sync.dma_start(out=w_t[:], in_=mww[ch, :, :])
                for mt in range(N_RT):
                    mcol = slice(mt * P, (mt + 1) * P)
                    ps = p7ps.tile([P, MLP_CC], F32, name=f"f{ch}_{mt}", tag="f")
                    for kf in range(N_MF):
                        nc.tensor.matmul(
                            ps[:],
                            gT[kf][:, mcol],
                            w_t[:, kf * MLP_CC:(kf + 1) * MLP_CC],
                            start=(kf == 0), stop=(kf == N_MF - 1),
                        )
                    o_t = p7o.tile([P, MLP_CC], F32, name=f"o{ch}_{mt}", tag="o")
                    nc.vector.tensor_add(
                        o_t[:],
                        x2_res[mt][:, ch * MLP_CC:(ch + 1) * MLP_CC],
                        ps[:],
                    )
                    nc.scalar.dma_start(
                        out=out_loc[
                            mt * P:(mt + 1) * P,
                            ch * MLP_CC:(ch + 1) * MLP_CC,
                        ],
                        in_=o_t[:],
                    )
        p7w_ctx.close()
        gT_ctx.close()
        n2T_ctx.close()
        x2_ctx.close()
        stk.close()

    return nc


_NC_CACHE = None


def _get_nc():
    global _NC_CACHE
    if _NC_CACHE is None:
        _NC_CACHE = build_nc()
    return _NC_CACHE


def _prep_inputs(x, cos, sin, attention_bias, norm1_w, norm2_w, attn_w, proj_w,
                 fc_w, mlp_proj_w):
    bf = ml_dtypes.bfloat16
    xf = np.asarray(x, np.float32).reshape(R, C)
    w1 = np.asarray(norm1_w, np.float32)
    w2 = np.asarray(norm2_w, np.float32)
    aw = np.asarray(attn_w, np.float32) * w1[None, :]      # [3C, C] (norm folded)
    pwf = np.asarray(proj_w, np.float32)                   # [C, C]
    fwf = np.asarray(fc_w, np.float32) * w2[None, :]       # [F, C]
    mwf = np.asarray(mlp_proj_w, np.float32)               # [C, F]
    cosf = np.asarray(cos, np.float32)                     # [T, 64]
    sinf = np.asarray(sin, np.float32)

    awr = aw.reshape(H, 3, HD, C)
    # qkw[j<H] = K-weights of head j; qkw[j>=H] = Q-weights of head j-H.
    # qkw[j, p, k*128+m] = awr[h, comp, m, k*128+p]
    qk = np.empty((2 * H, P, C), np.float32)
    for h in range(H):
        qk[h] = awr[h, 1].T.reshape(N_KC, P, HD).transpose(1, 0, 2).reshape(P, C)
        qk[H + h] = awr[h, 0].T.reshape(N_KC, P, HD).transpose(1, 0, 2).reshape(P, C)
    # vw[half, k, p, (h-8*half)*128+d] = awr[h, 2, d, k*128+p]
    vwt = (
        awr[:, 2].reshape(H * HD, C).T.reshape(N_KC, P, 2, C // 2)
        .transpose(2, 0, 1, 3)
    )
    # pw[ont, p, kd*512+co] = proj_w[ont*512+co, kd*128+p]
    pwt = np.ascontiguousarray(
        pwf.reshape(4, 512, N_KC, P).transpose(0, 3, 2, 1)
    ).reshape(4, P, N_KC * 512)
    # fcw[mf, p, k*128+f] = fwf[mf*128+f, k*128+p]
    fct = np.ascontiguousarray(
        fwf.reshape(N_MF, P, N_KC, P).transpose(0, 3, 2, 1)
    ).reshape(N_MF, P, C)
    # mww[ch, p, kf*CC+c] = mwf[ch*CC+c, kf*128+p]
    mwt = np.ascontiguousarray(
        mwf.reshape(N_MCH, MLP_CC, N_MF, P).transpose(0, 3, 2, 1)
    ).reshape(N_MCH, P, N_MF * MLP_CC)

    qk_b = np.ascontiguousarray(qk).astype(bf)
    vw_b = np.ascontiguousarray(vwt).astype(bf)
    pw_b = np.ascontiguousarray(pwt).astype(bf)
    fc_b = fct.astype(bf)
    mw_b = mwt.astype(bf)
    # mask[s, t] = 1 iff s <= t  (transposed causal tril)
    maskT = np.triu(np.ones((P, P), np.float32))

    in_maps = []
    for c in range(N_CORES):
        t0 = (c % (N_CORES // B)) * R_LOC
        sm = np.zeros((P, N_CORES), np.float32)
        for s in range(N_CORES):
            if s // (N_CORES // B) == c // (N_CORES // B) and s > c:
                sm[:, s] = 1.0
        in_maps.append({
            "x_loc": np.ascontiguousarray(xf[R_LOC * c:R_LOC * (c + 1)]),
            "cosr": np.ascontiguousarray(cosf[t0:t0 + R_LOC].T).astype(bf),
            "sinr": np.ascontiguousarray(sinf[t0:t0 + R_LOC].T).astype(bf),
            "maskT": maskT,
            "smask": sm,
            "qkw": qk_b,
            "vw": vw_b,
            "pw": pw_b,
            "fcw": fc_b,
            "mww": mw_b,
        })
    return in_maps


def kernel(**inputs):
    nc = _get_nc()
    in_maps = _prep_inputs(**inputs)
    res = run_bass_kernel_spmd(nc, in_maps, list(range(N_CORES)))
    out = np.concatenate(
        [np.asarray(res.results[c]["out_loc"], np.float32) for c in range(N_CORES)],
        axis=0,
    )
    return out.reshape(B, T, C)


# revision 10
# speedup vs baseline: 1.9076x; 1.0331x over previous
"""Trainium2 Bass kernel for nn_Block_42460046688864 (dense transformer block).

Reference math (B=2, T=2048, C=2048, H=16, HD=128):
    n1  = rmsnorm(x) * norm1_w
    qkv = n1 @ attn_w.T ; q,k,v per head ; q,k = rope(q,k) ; phi = elu(.)+1
    w   = (phi_q . phi_k) * scale * tril ; w /= sum(w) ; y = w @ v
    h   = y @ proj_w.T ; x2 = x + h
    ffn = gelu(rmsnorm(x2)*norm2_w @ fc_w.T) @ mlp_proj_w.T ; out = x2 + ffn

Distribution (8 NeuronCores, one NEFF, fully data-parallel):
  - rows (b*T+t, 4096 total) sharded 512/core; every core streams the FULL
    weights from its own HBM (no activation collectives at all).
  - attention is chunked linear attention (causal tril + positive elu+1
    features == prefix-state form; scale and eps cancel to ~1e-9 rel).
    The only cross-core dependency is the causal prefix state: each core's
    segment state S_seg[h] = sum_t k_t (x) [v_t | 1] is exchanged with ONE
    small ReduceScatter. Core j writes S_seg * mask[j<s, same-seq] into
    slot s, so after the add-RS core s holds exactly the sum of its
    same-sequence predecessors' states (its causal init state). The RS is
    issued right after K/V are ready and overlaps the Q projection; the
    correction q @ S_init is fused into each chunk's PSUM accumulation.
  - V is computed directly in [token, dim] layout by using n1^T chunks as
    the stationary matmul operand (no V transposes); K additionally needs
    [token, dim] for the state outer products -> 64 small DMA transposes.

Notes:
  - norm weights are folded into attn_w / fc_w on the host (exact algebra).
  - matmul operands are bf16 (fp32 PSUM accumulation); norms, residuals and
    attention numerators/denominators stay fp32 (psum) end to end.
  - weights are pre-tiled on the host into [128 x N] DMA slabs so every
    weight DMA is one contiguous >=2KB-per-partition block.
  - SBUF pools are strict LIFO per side; long-lived attention tiles live on
    the left stack, y^T on the right stack so lifetimes nest.
  - TileContext's tail drain is patched to split its semaphore waits:
    this walrus build rejects >2 sync waits on one TPB_CTRL instruction.
"""

from contextlib import ExitStack

import numpy as np
import ml_dtypes

import concourse.bass as bass
import concourse.mybir as mybir
import concourse.tile as tile
from concourse.bass_utils import run_bass_kernel_spmd
from concourse.masks import make_identity
from bass_rust import ScopedClock

F32 = mybir.dt.float32
BF16 = mybir.dt.bfloat16
AF = mybir.ActivationFunctionType

N_CORES = 8
B, T, C, H, HD = 2, 2048, 2048, 16, 128
F = 4 * C                  # 8192 mlp hidden
R = B * T                  # 4096 flattened rows (b-major)
R_LOC = R // N_CORES       # 512 rows per core
P = 128
EPS_NORM = 1e-5
N_RT = R_LOC // P          # 4 local row tiles
N_KC = C // P              # 16 contraction tiles over C
N_CH = N_RT                # 4 local causal chunks
N_MF = F // P              # 64 mlp-hidden tiles
HD1 = HD + 1               # state cols: [v dims | 1]
SAW = H * HD1              # 2064 = all-head state cols
MLP_CC = 256               # mlp output col-chunk
N_MCH = C // MLP_CC        # 8 col chunks

_MAX_WAITS = 1  # this walrus build rejects multi-wait instructions


def _split_excess_waits(nc):
    """Move excess semaphore waits onto same-engine NoOps ahead of the op."""
    for fn in nc.m.functions:
        for bb in fn.blocks:
            insts = list(bb.instructions)
            out = []
            for ins in insts:
                si = getattr(ins, "sync_info", None)
                waits = list(si.on_wait) if si and si.on_wait else []
                sem_waits = [w for w in waits if w.sync_type == "semaphore"]
                if len(sem_waits) > _MAX_WAITS:
                    keep = [w for w in waits if w.sync_type != "semaphore"]
                    keep += sem_waits[: _MAX_WAITS - 1] if _MAX_WAITS > 1 else []
                    extra = sem_waits[_MAX_WAITS - 1:] if _MAX_WAITS > 1 else sem_waits
                    for j in range(0, len(extra), _MAX_WAITS):
                        chunk = extra[j:j + _MAX_WAITS]
                        nop = mybir.InstNoOp(
                            name=nc.get_next_instruction_name(), ins=[], outs=[]
                        )
                        nop.engine = ins.engine
                        nop.sync_info = mybir.SyncInfo(on_wait=chunk, on_update=[])
                        out.append(nop)
                    si.on_wait[:] = keep
                out.append(ins)
            if len(out) != len(insts):
                bb.instructions[:] = out


class _TC(tile.TileContext):
    """TileContext whose tail drain splits sem waits one-per-NOP."""

    def schedule_and_allocate(self):
        ret = super().schedule_and_allocate()
        _split_excess_waits(self.nc)
        return ret

    def _drain_and_barrier(self, tick_clock, wait_clock):
        probe = self.nc.sync.nop(nofuse=True, hint="drain_waits")
        wait_clock.add_sem_waits(
            probe.ins, ScopedClock({None: tick_clock.global_clock})
        )
        si = probe.ins.sync_info
        waits = list(si.on_wait) if si and si.on_wait else []
        if len(waits) > 1:
            si.on_wait[:] = waits[:1]
            for w in waits[1:]:
                extra = self.nc.sync.nop(nofuse=True, hint="drain_waits")
                extra.ins.sync_info = mybir.SyncInfo(on_wait=[w], on_update=[])
        self.nc.sync.drain()
        self.nc.all_engine_barrier()
        popped = self.nc._tile_sem_poison_stack.pop()
        assert popped is self._sem_poison
        self.nc.clear_and_free_semaphores(list(self.sems.allocated().values()))
        self.nc.all_engine_barrier()


def build_nc():
    nc = bass.Bass(target_bir_lowering=False)

    x_loc = nc.declare_dram_parameter("x_loc", [R_LOC, C], F32, isOutput=False)
    cosr = nc.declare_dram_parameter("cosr", [HD // 2, R_LOC], BF16, isOutput=False)
    sinr = nc.declare_dram_parameter("sinr", [HD // 2, R_LOC], BF16, isOutput=False)
    maskT = nc.declare_dram_parameter("maskT", [P, P], F32, isOutput=False)
    smask = nc.declare_dram_parameter("smask", [P, N_CORES], F32, isOutput=False)
    # pre-tiled weight slabs (see _prep_inputs for layouts)
    qkw = nc.declare_dram_parameter("qkw", [2 * H, P, C], BF16, isOutput=False)
    vw = nc.declare_dram_parameter("vw", [2, N_KC, P, C // 2], BF16, isOutput=False)
    pw = nc.declare_dram_parameter("pw", [4, P, N_KC * 512], BF16, isOutput=False)
    fcw = nc.declare_dram_parameter("fcw", [N_MF, P, C], BF16, isOutput=False)
    mww = nc.declare_dram_parameter(
        "mww", [N_MCH, P, N_MF * MLP_CC], BF16, isOutput=False
    )
    out_loc = nc.declare_dram_parameter("out_loc", [R_LOC, C], F32, isOutput=True)

    rs_in = nc.dram_tensor("rs_in", [N_CORES, P, SAW], BF16)
    rs_out = nc.dram_tensor("rs_out", [P, SAW], BF16)

    groups = [list(range(N_CORES))]

    with _TC(nc) as tc:
        stk = ExitStack()
        const = stk.enter_context(tc.tile_pool(name="const", bufs=1))
        psum = stk.enter_context(tc.tile_pool(name="psum", bufs=1, space="PSUM"))
        def ps_big(name):
            return psum.tile([P, 512], F32, name=name, tag="big", bufs=4)
        def ps_sm(name, cols=HD1, dtype=F32):
            return psum.tile([P, cols], dtype, name=name, tag="sm",
                             padded_shape=[P, 256], bufs=4)
        ident_f32 = const.tile([P, P], F32)
        make_identity(nc, ident_f32)
        ident_bf = const.tile([P, P], BF16)
        make_identity(nc, ident_bf)
        mask_sb = const.tile([P, P], F32)
        nc.sync.dma_start(out=mask_sb[:], in_=maskT[:, :])
        smask_sb = const.tile([P, N_CORES], F32)
        nc.sync.dma_start(out=smask_sb[:], in_=smask[:, :])
        eps_t = const.tile([P, 1], F32)
        nc.vector.memset(eps_t[:], EPS_NORM)
        cos_sb = const.tile([HD // 2, R_LOC], BF16)
        sin_sb = const.tile([HD // 2, R_LOC], BF16)
        nc.sync.dma_start(out=cos_sb[:], in_=cosr[:, :])
        nc.sync.dma_start(out=sin_sb[:], in_=sinr[:, :])

        # y^T on the right stack: outlives the attention residents (left).
        yT_ctx = ExitStack()
        yT_pool = yT_ctx.enter_context(tc.tile_pool(name="yT", bufs=1, side="right"))
        yT = [
            [yT_pool.tile([P, P], BF16, name=f"yT{h}_{i}") for i in range(N_CH)]
            for h in range(H)
        ]

        # attention residents (left): released together after phase 4.
        att_ctx = ExitStack()
        n1T_pool = att_ctx.enter_context(tc.tile_pool(name="n1T", bufs=1))
        qk_pool = att_ctx.enter_context(tc.tile_pool(name="qkres", bufs=1))
        vp_pool = att_ctx.enter_context(tc.tile_pool(name="vpres", bufs=1))
        sbf_pool = att_ctx.enter_context(tc.tile_pool(name="sbfres", bufs=1))
        n1T = [n1T_pool.tile([P, R_LOC], BF16, name=f"n1T{k}") for k in range(N_KC)]
        kres = [qk_pool.tile([P, R_LOC], BF16, name=f"k{h}") for h in range(H)]
        qres = [qk_pool.tile([P, R_LOC], BF16, name=f"q{h}") for h in range(H)]
        vp = [
            [vp_pool.tile([P, HD1], BF16, name=f"vp{h}_{i}") for i in range(N_CH)]
            for h in range(H)
        ]

        # ---- phase 0: rmsnorm(x) -> n1T (SBUF-resident, transposed) ----
        with (
            tc.tile_pool(name="p0x", bufs=2) as p0x,
            tc.tile_pool(name="p0sq", bufs=1) as p0sq,
            tc.tile_pool(name="p0st", bufs=8) as p0st,
            tc.tile_pool(name="p0n", bufs=1) as p0n,
        ):
            n_ts = []
            for i in range(N_RT):
                x_t = p0x.tile([P, C], F32, name=f"x{i}", tag="x")
                nc.sync.dma_start(out=x_t[:], in_=x_loc[i * P:(i + 1) * P, :])
                sq = p0sq.tile([P, C], F32, name=f"sq{i}", tag="sq")
                ss = p0st.tile([P, 1], F32, name=f"ss{i}", tag="ss")
                nc.scalar.activation(sq[:], x_t[:], AF.Square, accum_out=ss[:])
                rms = p0st.tile([P, 1], F32, name=f"rms{i}", tag="rms")
                nc.scalar.activation(
                    rms[:], ss[:], AF.Sqrt, bias=eps_t[:], scale=1.0 / C
                )
                inv = p0st.tile([P, 1], F32, name=f"inv{i}", tag="inv")
                nc.vector.reciprocal(inv[:], rms[:])
                n_t = p0n.tile([P, C], F32, name=f"n{i}", tag=f"n{i}")
                nc.vector.tensor_scalar_mul(n_t[:], x_t[:], inv[:])
                n_ts.append(n_t)
            for k in range(N_KC):
                ps = ps_big(f"tr{k}")
                for i in range(N_RT):
                    nc.tensor.transpose(
                        ps[:, i * P:(i + 1) * P],
                        n_ts[i][:, k * P:(k + 1) * P], ident_f32[:],
                    )
                nc.scalar.copy(n1T[k][:], ps[:])

        # ---- phase 1: K then V then Q projections --------------------
        HF = HD // 2

        def rope_elu(dst, ps, pool, uid):
            """psum [128 x 512] (d-major head tile) -> phi(rope(.)) bf16."""
            raw = pool.tile([P, R_LOC], BF16, name=f"raw{uid}", tag="raw")
            nc.scalar.copy(raw[:], ps[:])
            s1 = pool.tile([HF, R_LOC], BF16, name=f"s1{uid}", tag="s1")
            s2 = pool.tile([HF, R_LOC], BF16, name=f"s2{uid}", tag="s2")
            ro = pool.tile([P, R_LOC], BF16, name=f"ro{uid}", tag="ro")
            nc.vector.tensor_mul(s1[:], raw[0:HF, :], cos_sb[:])
            nc.vector.tensor_mul(s2[:], raw[HF:P, :], sin_sb[:])
            nc.vector.tensor_sub(ro[0:HF, :], s1[:], s2[:])
            nc.vector.tensor_mul(s1[:], raw[0:HF, :], sin_sb[:])
            nc.vector.tensor_mul(s2[:], raw[HF:P, :], cos_sb[:])
            nc.vector.tensor_add(ro[HF:P, :], s1[:], s2[:])
            # phi = elu(ro)+1 = relu(ro) + exp(ro - relu(ro))
            rl = pool.tile([P, R_LOC], BF16, name=f"rl{uid}", tag="rl")
            nc.scalar.activation(rl[:], ro[:], AF.Relu)
            dm = pool.tile([P, R_LOC], BF16, name=f"dm{uid}", tag="dm")
            nc.vector.tensor_sub(dm[:], ro[:], rl[:])
            ex = pool.tile([P, R_LOC], BF16, name=f"ex{uid}", tag="ex")
            nc.scalar.activation(ex[:], dm[:], AF.Exp)
            nc.vector.tensor_add(dst[:], rl[:], ex[:])

        with (
            tc.tile_pool(name="p1w", bufs=5) as p1w,
            tc.tile_pool(name="p1vw", bufs=1) as p1vw,
            tc.tile_pool(name="p1r", bufs=4) as p1r,
        ):
            # K projections (j = 0..15), rope+elu on DVE as tiles land
            for j in range(H):
                w_t = p1w.tile([P, C], BF16, name=f"kw{j}", tag="qkw")
                nc.sync.dma_start(out=w_t[:], in_=qkw[j, :, :])
                ps = ps_big(f"kps{j}")
                for k in range(N_KC):
                    nc.tensor.matmul(
                        ps[:], w_t[:, k * P:(k + 1) * P], n1T[k][:],
                        start=(k == 0), stop=(k == N_KC - 1),
                    )
                rope_elu(kres[j], ps, p1r, f"k{j}")

            # V projections, directly transposed: [t x hv] chunks
            if True:
                for half in range(2):
                    vw_sb = []
                    for k in range(N_KC):
                        w_t = p1vw.tile(
                            [P, C // 2], BF16, name=f"vw{half}_{k}", tag=f"vw{k}"
                        )
                        nc.sync.dma_start(out=w_t[:], in_=vw[half, k, :, :])
                        vw_sb.append(w_t)
                    for vb in range(2):
                        vcol = slice(vb * 512, (vb + 1) * 512)
                        for i in range(N_CH):
                            icol = slice(i * P, (i + 1) * P)
                            ps = ps_big(f"v{half}_{vb}_{i}")
                            for k in range(N_KC):
                                nc.tensor.matmul(
                                    ps[:], n1T[k][:, icol], vw_sb[k][:, vcol],
                                    start=(k == 0), stop=(k == N_KC - 1),
                                )
                            for hs in range(4):
                                h = half * 8 + vb * 4 + hs
                                nc.scalar.copy(
                                    vp[h][i][:, 0:HD], ps[:, hs * P:(hs + 1) * P]
                                )
                                nc.vector.memset(vp[h][i][:, HD:HD1], 1.0)

            # ---- phase 2: segment states + masked RS exchange --------
            # (emitted before Q so the collective overlaps Q/scores)
            s_bf = [[None] * N_CH for _ in range(H)]
            with (
                tc.high_priority(),
                tc.tile_pool(name="p2kp", bufs=4) as p2kp,
                tc.tile_pool(name="p2run", bufs=1) as p2run,
                tc.tile_pool(name="p2all", bufs=1) as p2all,
                tc.tile_pool(name="p2msk", bufs=2) as p2msk,
            ):
                s_all = p2all.tile([P, SAW], BF16, name="s_all")
                for h in range(H):
                    s_run = p2run.tile([P, HD1], F32, name=f"srun{h}")
                    for i in range(N_CH):
                        tcol = slice(i * P, (i + 1) * P)
                        kp = p2kp.tile([P, P], BF16, name=f"kp{h}_{i}", tag="kp")
                        nc.sync.dma_start_transpose(kp[:], kres[h][:, tcol])
                        sd = ps_sm(f"sd{h}_{i}")
                        nc.tensor.matmul(sd[:], kp[:], vp[h][i][:], start=True, stop=True)
                        if i == 0:
                            nc.scalar.copy(s_run[:], sd[:])
                        else:
                            nc.vector.tensor_add(s_run[:], s_run[:], sd[:])
                        if i < N_CH - 1:
                            sb = sbf_pool.tile([P, HD1], BF16, name=f"sbf{h}_{i}")
                            nc.scalar.copy(sb[:], s_run[:])
                            s_bf[h][i + 1] = sb
                    nc.scalar.copy(s_all[:, h * HD1:(h + 1) * HD1], s_run[:])
                for s in range(N_CORES):
                    ms = p2msk.tile([P, SAW], BF16, name=f"ms{s}", tag="ms")
                    nc.vector.tensor_scalar_mul(ms[:], s_all[:], smask_sb[:, s:s + 1])
                    nc.sync.dma_start(out=rs_in[s, :, :], in_=ms[:])
                nc.gpsimd.collective_compute(
                    "ReduceScatter",
                    mybir.AluOpType.add,
                    ins=[rs_in.ap().opt()],
                    outs=[rs_out.ap().opt()],
                    replica_groups=groups,
                )

            # Q projections (j = 16..31), overlap the collective
            for j in range(H):
                w_t = p1w.tile([P, C], BF16, name=f"qw{j}", tag="qkw")
                nc.sync.dma_start(out=w_t[:], in_=qkw[H + j, :, :])
                ps = ps_big(f"qps{j}")
                for k in range(N_KC):
                    nc.tensor.matmul(
                        ps[:], w_t[:, k * P:(k + 1) * P], n1T[k][:],
                        start=(k == 0), stop=(k == N_KC - 1),
                    )
                rope_elu(qres[j], ps, p1r, f"q{j}")

        # ---- phase 3: local masked scores (no state dependency) ------
        am_ctx = ExitStack()
        am_pool = am_ctx.enter_context(tc.tile_pool(name="amres", bufs=1))
        am = [[None] * N_CH for _ in range(H)]
        if True:
            for h in range(H):
                for i in range(N_CH):
                    tcol = slice(i * P, (i + 1) * P)
                    a_ps = ps_sm(f"a{h}_{i}", cols=P)
                    nc.tensor.matmul(
                        a_ps[:], kres[h][:, tcol], qres[h][:, tcol],
                        start=True, stop=True,
                    )
                    am_t = am_pool.tile([P, P], BF16, name=f"am{h}_{i}")
                    nc.vector.tensor_mul(am_t[:], a_ps[:], mask_sb[:])
                    am[h][i] = am_t

        # ---- phase 4: y = (q@S_loc + Am^T@V' + q@S_init) / den; y^T --
        with (
            tc.tile_pool(name="p4si", bufs=1) as p4si,
            tc.tile_pool(name="p4y", bufs=4) as p4y,
        ):
            sinit = p4si.tile([P, SAW], BF16, name="sinit")
            with tc.high_priority():
                nc.sync.dma_start(out=sinit[:], in_=rs_out[:, :])
            for h in range(H):
                hcol = slice(h * HD1, (h + 1) * HD1)
                for i in range(N_CH):
                    tcol = slice(i * P, (i + 1) * P)
                    y_ps = ps_sm(f"y{h}_{i}")
                    if i > 0:
                        nc.tensor.matmul(
                            y_ps[:], qres[h][:, tcol], s_bf[h][i][:],
                            start=True, stop=False,
                        )
                    nc.tensor.matmul(
                        y_ps[:], am[h][i][:], vp[h][i][:],
                        start=(i == 0), stop=False,
                    )
                    nc.tensor.matmul(
                        y_ps[:], qres[h][:, tcol], sinit[:, hcol],
                        start=False, stop=True,
                    )
                    rec = p4y.tile([P, 1], F32, name=f"rec{h}_{i}", tag="rec")
                    nc.vector.reciprocal(rec[:], y_ps[:, HD:HD1])
                    yb = p4y.tile([P, HD], BF16, name=f"yb{h}_{i}", tag="yb")
                    nc.scalar.activation(
                        yb[:], y_ps[:, 0:HD], AF.Identity, scale=rec[:]
                    )
                    tr = ps_sm(f"ytr{h}_{i}", cols=P, dtype=BF16)
                    nc.tensor.transpose(tr[:], yb[:], ident_bf[:])
                    nc.scalar.copy(yT[h][i][:], tr[:])
        am_ctx.close()
        att_ctx.close()

        # ---- phase 5: proj, residual, rmsnorm2 -> n2T ----------------
        x2_ctx = ExitStack()
        x2_pool = x2_ctx.enter_context(tc.tile_pool(name="x2res", bufs=1))
        x2_res = [x2_pool.tile([P, C], F32, name=f"x2_{i}") for i in range(N_RT)]
        n2T_ctx = ExitStack()
        n2T_pool = n2T_ctx.enter_context(tc.tile_pool(name="n2T", bufs=1))
        n2T = [n2T_pool.tile([P, R_LOC], BF16, name=f"n2T{k}") for k in range(N_KC)]
        with (
            tc.tile_pool(name="p5w", bufs=2) as p5w,
            tc.tile_pool(name="p5sq", bufs=1) as p5sq,
            tc.tile_pool(name="p5st", bufs=8) as p5st,
            tc.tile_pool(name="p5n", bufs=1) as p5n,
        ):
            for mt in range(N_RT):
                nc.sync.dma_start(
                    out=x2_res[mt][:], in_=x_loc[mt * P:(mt + 1) * P, :]
                )
            for ont in range(4):
                ocol = slice(ont * 512, (ont + 1) * 512)
                w_t = p5w.tile([P, N_KC * 512], BF16, name=f"pw{ont}", tag="pw")
                nc.sync.dma_start(out=w_t[:], in_=pw[ont, :, :])
                ps_mt = [ps_big(f"h{ont}_{mt}") for mt in range(N_RT)]
                for kd in range(N_KC):
                    for mt in range(N_RT):
                        nc.tensor.matmul(
                            ps_mt[mt][:], yT[kd][mt][:],
                            w_t[:, kd * 512:(kd + 1) * 512],
                            start=(kd == 0), stop=(kd == N_KC - 1),
                        )
                for mt in range(N_RT):
                    nc.vector.tensor_add(
                        x2_res[mt][:, ocol], x2_res[mt][:, ocol], ps_mt[mt][:]
                    )
            n2_ts = []
            for mt in range(N_RT):
                sq = p5sq.tile([P, C], F32, name=f"sq2_{mt}", tag="sq2")
                ss = p5st.tile([P, 1], F32, name=f"ss2_{mt}", tag="ss2")
                nc.scalar.activation(sq[:], x2_res[mt][:], AF.Square, accum_out=ss[:])
                rms = p5st.tile([P, 1], F32, name=f"rms2_{mt}", tag="rms2")
                nc.scalar.activation(
                    rms[:], ss[:], AF.Sqrt, bias=eps_t[:], scale=1.0 / C
                )
                inv = p5st.tile([P, 1], F32, name=f"inv2_{mt}", tag="inv2")
                nc.vector.reciprocal(inv[:], rms[:])
                n_t = p5n.tile([P, C], F32, name=f"n2_{mt}", tag=f"n2_{mt}")
                nc.vector.tensor_scalar_mul(n_t[:], x2_res[mt][:], inv[:])
                n2_ts.append(n_t)
            for k in range(N_KC):
                ps = ps_big(f"tr2_{k}")
                for mt in range(N_RT):
                    nc.tensor.transpose(
                        ps[:, mt * P:(mt + 1) * P],
                        n2_ts[mt][:, k * P:(k + 1) * P], ident_f32[:],
                    )
                nc.scalar.copy(n2T[k][:], ps[:])
        yT_ctx.close()

        # ---- phase 6: fc + gelu -> gT (resident) ---------------------
        gT_ctx = ExitStack()
        gT_pool = gT_ctx.enter_context(tc.tile_pool(name="gT", bufs=1))
        gT = [gT_pool.tile([P, R_LOC], BF16, name=f"gT{mf}") for mf in range(N_MF)]
        p7w_ctx = ExitStack()
        p7w = p7w_ctx.enter_context(tc.tile_pool(name="p7w", bufs=2))
        with (
            tc.tile_pool(name="p6w", bufs=3) as p6w,
        ):
            for mf in range(N_MF):
                w_t = p6w.tile([P, C], BF16, name=f"fcw{mf}", tag="fcw")
                nc.sync.dma_start(out=w_t[:], in_=fcw[mf, :, :])
                ps = ps_big(f"g{mf}")
                for k in range(N_KC):
                    nc.tensor.matmul(
                        ps[:], w_t[:, k * P:(k + 1) * P], n2T[k][:],
                        start=(k == 0), stop=(k == N_KC - 1),
                    )
                nc.scalar.activation(gT[mf][:], ps[:], AF.Gelu)

        # ---- phase 7: mlp proj + residual -> out ---------------------
        with (
            tc.tile_pool(name="p7o", bufs=4) as p7o,
        ):
            for ch in range(N_MCH):
                w_t = p7w.tile([P, N_MF * MLP_CC], BF16, name=f"mw{ch}", tag="mw")
                nc.sync.dma_start(out=w_t[:], in_=mww[ch, :, :])
                for mt in range(N_RT):
                    mcol = slice(mt * P, (mt + 1) * P)
                    ps = ps_sm(f"f{ch}_{mt}", cols=MLP_CC)
                    for kf in range(N_MF):
                        nc.tensor.matmul(
                            ps[:],
                            gT[kf][:, mcol],
                            w_t[:, kf * MLP_CC:(kf + 1) * MLP_CC],
                            start=(kf == 0), stop=(kf == N_MF - 1),
                        )
                    o_t = p7o.tile([P, MLP_CC], F32, name=f"o{ch}_{mt}", tag="o")
                    nc.vector.tensor_add(
                        o_t[:],
                        x2_res[mt][:, ch * MLP_CC:(ch + 1) * MLP_CC],
                        ps[:],
                    )
                    nc.scalar.dma_start(
                        out=out_loc[
                            mt * P:(mt + 1) * P,
                            ch * MLP_CC:(ch + 1) * MLP_CC,
                        ],
                        in_=o_t[:],
                    )
        p7w_ctx.close()
        gT_ctx.close()
        n2T_ctx.close()
        x2_ctx.close()
        stk.close()

    return nc


_NC_CACHE = None


def _get_nc():
    global _NC_CACHE
    if _NC_CACHE is None:
        _NC_CACHE = build_nc()
    return _NC_CACHE


def _prep_inputs(x, cos, sin, attention_bias, norm1_w, norm2_w, attn_w, proj_w,
                 fc_w, mlp_proj_w):
    bf = ml_dtypes.bfloat16
    xf = np.asarray(x, np.float32).reshape(R, C)
    w1 = np.asarray(norm1_w, np.float32)
    w2 = np.asarray(norm2_w, np.float32)
    aw = np.asarray(attn_w, np.float32) * w1[None, :]      # [3C, C] (norm folded)
    pwf = np.asarray(proj_w, np.float32)                   # [C, C]
    fwf = np.asarray(fc_w, np.float32) * w2[None, :]       # [F, C]
    mwf = np.asarray(mlp_proj_w, np.float32)               # [C, F]
    cosf = np.asarray(cos, np.float32)                     # [T, 64]
    sinf = np.asarray(sin, np.float32)

    awr = aw.reshape(H, 3, HD, C)
    # qkw[j<H] = K-weights of head j; qkw[j>=H] = Q-weights of head j-H.
    # qkw[j, p, k*128+m] = awr[h, comp, m, k*128+p]
    qk = np.empty((2 * H, P, C), np.float32)
    for h in range(H):
        qk[h] = awr[h, 1].T.reshape(N_KC, P, HD).transpose(1, 0, 2).reshape(P, C)
        qk[H + h] = awr[h, 0].T.reshape(N_KC, P, HD).transpose(1, 0, 2).reshape(P, C)
    # vw[half, k, p, (h-8*half)*128+d] = awr[h, 2, d, k*128+p]
    vwt = (
        awr[:, 2].reshape(H * HD, C).T.reshape(N_KC, P, 2, C // 2)
        .transpose(2, 0, 1, 3)
    )
    # pw[ont, p, kd*512+co] = proj_w[ont*512+co, kd*128+p]
    pwt = np.ascontiguousarray(
        pwf.reshape(4, 512, N_KC, P).transpose(0, 3, 2, 1)
    ).reshape(4, P, N_KC * 512)
    # fcw[mf, p, k*128+f] = fwf[mf*128+f, k*128+p]
    fct = np.ascontiguousarray(
        fwf.reshape(N_MF, P, N_KC, P).transpose(0, 3, 2, 1)
    ).reshape(N_MF, P, C)
    # mww[ch, p, kf*CC+c] = mwf[ch*CC+c, kf*128+p]
    mwt = np.ascontiguousarray(
        mwf.reshape(N_MCH, MLP_CC, N_MF, P).transpose(0, 3, 2, 1)
    ).reshape(N_MCH, P, N_MF * MLP_CC)

    qk_b = np.ascontiguousarray(qk).astype(bf)
    vw_b = np.ascontiguousarray(vwt).astype(bf)
    pw_b = np.ascontiguousarray(pwt).astype(bf)
    fc_b = fct.astype(bf)
    mw_b = mwt.astype(bf)
    # mask[s, t] = 1 iff s <= t  (transposed causal tril)
    maskT = np.triu(np.ones((P, P), np.float32))

    in_maps = []
    for c in range(N_CORES):
        t0 = (c % (N_CORES // B)) * R_LOC
        sm = np.zeros((P, N_CORES), np.float32)
        for s in range(N_CORES):
            if s // (N_CORES // B) == c // (N_CORES // B) and s > c:
                sm[:, s] = 1.0
        in_maps.append({
            "x_loc": np.ascontiguousarray(xf[R_LOC * c:R_LOC * (c + 1)]),
            "cosr": np.ascontiguousarray(cosf[t0:t0 + R_LOC].T).astype(bf),
            "sinr": np.ascontiguousarray(sinf[t0:t0 + R_LOC].T).astype(bf),
            "maskT": maskT,
            "smask": sm,
            "qkw": qk_b,
            "vw": vw_b,
            "pw": pw_b,
            "fcw": fc_b,
            "mww": mw_b,
        })
    return in_maps


def kernel(**inputs):
    nc = _get_nc()
    in_maps = _prep_inputs(**inputs)
    res = run_bass_kernel_spmd(nc, in_maps, list(range(N_CORES)))
    out = np.concatenate(
        [np.asarray(res.results[c]["out_loc"], np.float32) for c in range(N_CORES)],
        axis=0,
    )
    return out.reshape(B, T, C)


# revision 11
# speedup vs baseline: 1.9113x; 1.0020x over previous
"""Trainium2 Bass kernel for nn_Block_42460046688864 (dense transformer block).

Reference math (B=2, T=2048, C=2048, H=16, HD=128):
    n1  = rmsnorm(x) * norm1_w
    qkv = n1 @ attn_w.T ; q,k,v per head ; q,k = rope(q,k) ; phi = elu(.)+1
    w   = (phi_q . phi_k) * scale * tril ; w /= sum(w) ; y = w @ v
    h   = y @ proj_w.T ; x2 = x + h
    ffn = gelu(rmsnorm(x2)*norm2_w @ fc_w.T) @ mlp_proj_w.T ; out = x2 + ffn

Distribution (8 NeuronCores, one NEFF, fully data-parallel):
  - rows (b*T+t, 4096 total) sharded 512/core; every core streams the FULL
    weights from its own HBM (no activation collectives at all).
  - attention is chunked linear attention (causal tril + positive elu+1
    features == prefix-state form; scale and eps cancel to ~1e-9 rel).
    The only cross-core dependency is the causal prefix state: each core's
    segment state S_seg[h] = sum_t k_t (x) [v_t | 1] is exchanged with ONE
    small ReduceScatter. Core j writes S_seg * mask[j<s, same-seq] into
    slot s, so after the add-RS core s holds exactly the sum of its
    same-sequence predecessors' states (its causal init state). The RS is
    issued right after K/V are ready and overlaps the Q projection; the
    correction q @ S_init is fused into each chunk's PSUM accumulation.
  - V is computed directly in [token, dim] layout by using n1^T chunks as
    the stationary matmul operand (no V transposes); K additionally needs
    [token, dim] for the state outer products -> 64 small DMA transposes.

Notes:
  - norm weights are folded into attn_w / fc_w on the host (exact algebra).
  - matmul operands are bf16 (fp32 PSUM accumulation); norms, residuals and
    attention numerators/denominators stay fp32 (psum) end to end.
  - weights are pre-tiled on the host into [128 x N] DMA slabs so every
    weight DMA is one contiguous >=2KB-per-partition block.
  - SBUF pools are strict LIFO per side; long-lived attention tiles live on
    the left stack, y^T on the right stack so lifetimes nest.
  - TileContext's tail drain is patched to split its semaphore waits:
    this walrus build rejects >2 sync waits on one TPB_CTRL instruction.
"""

from contextlib import ExitStack

import numpy as np
import ml_dtypes

import concourse.bass as bass
import concourse.mybir as mybir
import concourse.tile as tile
from concourse.bass_utils import run_bass_kernel_spmd
from concourse.masks import make_identity
from bass_rust import ScopedClock

F32 = mybir.dt.float32
BF16 = mybir.dt.bfloat16
AF = mybir.ActivationFunctionType

N_CORES = 8
B, T, C, H, HD = 2, 2048, 2048, 16, 128
F = 4 * C                  # 8192 mlp hidden
R = B * T                  # 4096 flattened rows (b-major)
R_LOC = R // N_CORES       # 512 rows per core
P = 128
EPS_NORM = 1e-5
N_RT = R_LOC // P          # 4 local row tiles
N_KC = C // P              # 16 contraction tiles over C
N_CH = N_RT                # 4 local causal chunks
N_MF = F // P              # 64 mlp-hidden tiles
HD1 = HD + 1               # state cols: [v dims | 1]
SAW = H * HD1              # 2064 = all-head state cols
MLP_CC = 256               # mlp output col-chunk
N_MCH = C // MLP_CC        # 8 col chunks

_MAX_WAITS = 1  # this walrus build rejects multi-wait instructions


def _split_excess_waits(nc):
    """Move excess semaphore waits onto same-engine NoOps ahead of the op."""
    for fn in nc.m.functions:
        for bb in fn.blocks:
            insts = list(bb.instructions)
            out = []
            for ins in insts:
                si = getattr(ins, "sync_info", None)
                waits = list(si.on_wait) if si and si.on_wait else []
                sem_waits = [w for w in waits if w.sync_type == "semaphore"]
                if len(sem_waits) > _MAX_WAITS:
                    keep = [w for w in waits if w.sync_type != "semaphore"]
                    keep += sem_waits[: _MAX_WAITS - 1] if _MAX_WAITS > 1 else []
                    extra = sem_waits[_MAX_WAITS - 1:] if _MAX_WAITS > 1 else sem_waits
                    for j in range(0, len(extra), _MAX_WAITS):
                        chunk = extra[j:j + _MAX_WAITS]
                        nop = mybir.InstNoOp(
                            name=nc.get_next_instruction_name(), ins=[], outs=[]
                        )
                        nop.engine = ins.engine
                        nop.sync_info = mybir.SyncInfo(on_wait=chunk, on_update=[])
                        out.append(nop)
                    si.on_wait[:] = keep
                out.append(ins)
            if len(out) != len(insts):
                bb.instructions[:] = out


class _TC(tile.TileContext):
    """TileContext whose tail drain splits sem waits one-per-NOP."""

    def schedule_and_allocate(self):
        ret = super().schedule_and_allocate()
        _split_excess_waits(self.nc)
        return ret

    def _drain_and_barrier(self, tick_clock, wait_clock):
        probe = self.nc.sync.nop(nofuse=True, hint="drain_waits")
        wait_clock.add_sem_waits(
            probe.ins, ScopedClock({None: tick_clock.global_clock})
        )
        si = probe.ins.sync_info
        waits = list(si.on_wait) if si and si.on_wait else []
        if len(waits) > 1:
            si.on_wait[:] = waits[:1]
            for w in waits[1:]:
                extra = self.nc.sync.nop(nofuse=True, hint="drain_waits")
                extra.ins.sync_info = mybir.SyncInfo(on_wait=[w], on_update=[])
        self.nc.sync.drain()
        self.nc.all_engine_barrier()
        popped = self.nc._tile_sem_poison_stack.pop()
        assert popped is self._sem_poison
        self.nc.clear_and_free_semaphores(list(self.sems.allocated().values()))
        self.nc.all_engine_barrier()


def build_nc():
    nc = bass.Bass(target_bir_lowering=False)

    x_loc = nc.declare_dram_parameter("x_loc", [R_LOC, C], F32, isOutput=False)
    cosr = nc.declare_dram_parameter("cosr", [HD // 2, R_LOC], BF16, isOutput=False)
    sinr = nc.declare_dram_parameter("sinr", [HD // 2, R_LOC], BF16, isOutput=False)
    maskT = nc.declare_dram_parameter("maskT", [P, P], F32, isOutput=False)
    smask = nc.declare_dram_parameter("smask", [P, N_CORES], F32, isOutput=False)
    # pre-tiled weight slabs (see _prep_inputs for layouts)
    qkw = nc.declare_dram_parameter("qkw", [2 * H, P, C], BF16, isOutput=False)
    vw = nc.declare_dram_parameter("vw", [2, N_KC, P, C // 2], BF16, isOutput=False)
    pw = nc.declare_dram_parameter("pw", [4, P, N_KC * 512], BF16, isOutput=False)
    fcw = nc.declare_dram_parameter("fcw", [N_MF, P, C], BF16, isOutput=False)
    mww = nc.declare_dram_parameter(
        "mww", [N_MCH, P, N_MF * MLP_CC], BF16, isOutput=False
    )
    out_loc = nc.declare_dram_parameter("out_loc", [R_LOC, C], F32, isOutput=True)

    rs_in = nc.dram_tensor("rs_in", [N_CORES, P, SAW], BF16)
    rs_out = nc.dram_tensor("rs_out", [P, SAW], BF16)

    groups = [list(range(N_CORES))]

    with _TC(nc) as tc:
        stk = ExitStack()
        const = stk.enter_context(tc.tile_pool(name="const", bufs=1))
        psum = stk.enter_context(tc.tile_pool(name="psum", bufs=1, space="PSUM"))
        def ps_t(name, tag, bufs, cols=512, dtype=F32):
            return psum.tile([P, cols], dtype, name=name, tag=tag, bufs=bufs)
        ident_f32 = const.tile([P, P], F32)
        make_identity(nc, ident_f32)
        ident_bf = const.tile([P, P], BF16)
        make_identity(nc, ident_bf)
        mask_sb = const.tile([P, P], F32)
        nc.sync.dma_start(out=mask_sb[:], in_=maskT[:, :])
        smask_sb = const.tile([P, N_CORES], F32)
        nc.sync.dma_start(out=smask_sb[:], in_=smask[:, :])
        eps_t = const.tile([P, 1], F32)
        nc.vector.memset(eps_t[:], EPS_NORM)
        cos_sb = const.tile([HD // 2, R_LOC], BF16)
        sin_sb = const.tile([HD // 2, R_LOC], BF16)
        nc.sync.dma_start(out=cos_sb[:], in_=cosr[:, :])
        nc.sync.dma_start(out=sin_sb[:], in_=sinr[:, :])

        # y^T on the right stack: outlives the attention residents (left).
        yT_ctx = ExitStack()
        yT_pool = yT_ctx.enter_context(tc.tile_pool(name="yT", bufs=1, side="right"))
        yT = [
            [yT_pool.tile([P, P], BF16, name=f"yT{h}_{i}") for i in range(N_CH)]
            for h in range(H)
        ]

        # attention residents (left): released together after phase 4.
        att_ctx = ExitStack()
        n1T_pool = att_ctx.enter_context(tc.tile_pool(name="n1T", bufs=1))
        qk_pool = att_ctx.enter_context(tc.tile_pool(name="qkres", bufs=1))
        vp_pool = att_ctx.enter_context(tc.tile_pool(name="vpres", bufs=1))
        sbf_pool = att_ctx.enter_context(tc.tile_pool(name="sbfres", bufs=1))
        n1T = [n1T_pool.tile([P, R_LOC], BF16, name=f"n1T{k}") for k in range(N_KC)]
        kres = [qk_pool.tile([P, R_LOC], BF16, name=f"k{h}") for h in range(H)]
        qres = [qk_pool.tile([P, R_LOC], BF16, name=f"q{h}") for h in range(H)]
        vp = [
            [vp_pool.tile([P, HD1], BF16, name=f"vp{h}_{i}") for i in range(N_CH)]
            for h in range(H)
        ]

        # ---- phase 0: rmsnorm(x) -> n1T (SBUF-resident, transposed) ----
        with (
            tc.tile_pool(name="p0x", bufs=2) as p0x,
            tc.tile_pool(name="p0sq", bufs=1) as p0sq,
            tc.tile_pool(name="p0st", bufs=8) as p0st,
            tc.tile_pool(name="p0n", bufs=1) as p0n,
        ):
            n_ts = []
            for i in range(N_RT):
                x_t = p0x.tile([P, C], F32, name=f"x{i}", tag="x")
                nc.sync.dma_start(out=x_t[:], in_=x_loc[i * P:(i + 1) * P, :])
                sq = p0sq.tile([P, C], F32, name=f"sq{i}", tag="sq")
                ss = p0st.tile([P, 1], F32, name=f"ss{i}", tag="ss")
                nc.scalar.activation(sq[:], x_t[:], AF.Square, accum_out=ss[:])
                rms = p0st.tile([P, 1], F32, name=f"rms{i}", tag="rms")
                nc.scalar.activation(
                    rms[:], ss[:], AF.Sqrt, bias=eps_t[:], scale=1.0 / C
                )
                inv = p0st.tile([P, 1], F32, name=f"inv{i}", tag="inv")
                nc.vector.reciprocal(inv[:], rms[:])
                n_t = p0n.tile([P, C], F32, name=f"n{i}", tag=f"n{i}")
                nc.vector.tensor_scalar_mul(n_t[:], x_t[:], inv[:])
                n_ts.append(n_t)
            for k in range(N_KC):
                ps = ps_t(f"tr{k}", "v", 2)
                for i in range(N_RT):
                    nc.tensor.transpose(
                        ps[:, i * P:(i + 1) * P],
                        n_ts[i][:, k * P:(k + 1) * P], ident_f32[:],
                    )
                nc.scalar.copy(n1T[k][:], ps[:])

        # ---- phase 1: K then V then Q projections --------------------
        HF = HD // 2

        def rope_elu(dst, ps, pool, uid):
            """psum [128 x 512] (d-major head tile) -> phi(rope(.)) bf16."""
            raw = pool.tile([P, R_LOC], BF16, name=f"raw{uid}", tag="raw")
            nc.scalar.copy(raw[:], ps[:])
            s1 = pool.tile([HF, R_LOC], BF16, name=f"s1{uid}", tag="s1")
            s2 = pool.tile([HF, R_LOC], BF16, name=f"s2{uid}", tag="s2")
            ro = pool.tile([P, R_LOC], BF16, name=f"ro{uid}", tag="ro")
            nc.vector.tensor_mul(s1[:], raw[0:HF, :], cos_sb[:])
            nc.vector.tensor_mul(s2[:], raw[HF:P, :], sin_sb[:])
            nc.vector.tensor_sub(ro[0:HF, :], s1[:], s2[:])
            nc.vector.tensor_mul(s1[:], raw[0:HF, :], sin_sb[:])
            nc.vector.tensor_mul(s2[:], raw[HF:P, :], cos_sb[:])
            nc.vector.tensor_add(ro[HF:P, :], s1[:], s2[:])
            # phi = elu(ro)+1 = relu(ro) + exp(ro - relu(ro))
            rl = pool.tile([P, R_LOC], BF16, name=f"rl{uid}", tag="rl")
            nc.scalar.activation(rl[:], ro[:], AF.Relu)
            dm = pool.tile([P, R_LOC], BF16, name=f"dm{uid}", tag="dm")
            nc.vector.tensor_sub(dm[:], ro[:], rl[:])
            ex = pool.tile([P, R_LOC], BF16, name=f"ex{uid}", tag="ex")
            nc.scalar.activation(ex[:], dm[:], AF.Exp)
            nc.vector.tensor_add(dst[:], rl[:], ex[:])

        with (
            tc.tile_pool(name="p1w", bufs=5) as p1w,
            tc.tile_pool(name="p1vw", bufs=1) as p1vw,
            tc.tile_pool(name="p1r", bufs=4) as p1r,
        ):
            # K projections (j = 0..15), rope+elu on DVE as tiles land
            for j in range(H):
                w_t = p1w.tile([P, C], BF16, name=f"kw{j}", tag="qkw")
                nc.sync.dma_start(out=w_t[:], in_=qkw[j, :, :])
                ps = ps_t(f"kps{j}", "kq", 4)
                for k in range(N_KC):
                    nc.tensor.matmul(
                        ps[:], w_t[:, k * P:(k + 1) * P], n1T[k][:],
                        start=(k == 0), stop=(k == N_KC - 1),
                    )
                rope_elu(kres[j], ps, p1r, f"k{j}")

            # V projections, directly transposed: [t x hv] chunks
            if True:
                for half in range(2):
                    vw_sb = []
                    for k in range(N_KC):
                        w_t = p1vw.tile(
                            [P, C // 2], BF16, name=f"vw{half}_{k}", tag=f"vw{k}"
                        )
                        nc.sync.dma_start(out=w_t[:], in_=vw[half, k, :, :])
                        vw_sb.append(w_t)
                    for vb in range(2):
                        vcol = slice(vb * 512, (vb + 1) * 512)
                        for i in range(N_CH):
                            icol = slice(i * P, (i + 1) * P)
                            ps = ps_t(f"v{half}_{vb}_{i}", "v", 2)
                            for k in range(N_KC):
                                nc.tensor.matmul(
                                    ps[:], n1T[k][:, icol], vw_sb[k][:, vcol],
                                    start=(k == 0), stop=(k == N_KC - 1),
                                )
                            for hs in range(4):
                                h = half * 8 + vb * 4 + hs
                                nc.scalar.copy(
                                    vp[h][i][:, 0:HD], ps[:, hs * P:(hs + 1) * P]
                                )
                                nc.vector.memset(vp[h][i][:, HD:HD1], 1.0)

            # ---- phase 2: segment states + masked RS exchange --------
            # (emitted before Q so the collective overlaps Q/scores)
            s_bf = [[None] * N_CH for _ in range(H)]
            with (
                tc.high_priority(),
                tc.tile_pool(name="p2kp", bufs=4) as p2kp,
                tc.tile_pool(name="p2run", bufs=1) as p2run,
                tc.tile_pool(name="p2all", bufs=1) as p2all,
                tc.tile_pool(name="p2msk", bufs=2) as p2msk,
            ):
                s_all = p2all.tile([P, SAW], BF16, name="s_all")
                for h in range(H):
                    s_run = p2run.tile([P, HD1], F32, name=f"srun{h}")
                    for i in range(N_CH):
                        tcol = slice(i * P, (i + 1) * P)
                        kp = p2kp.tile([P, P], BF16, name=f"kp{h}_{i}", tag="kp")
                        nc.sync.dma_start_transpose(kp[:], kres[h][:, tcol])
                        sd = ps_t(f"sd{h}_{i}", "sd", 1, cols=HD1)
                        nc.tensor.matmul(sd[:], kp[:], vp[h][i][:], start=True, stop=True)
                        if i == 0:
                            nc.scalar.copy(s_run[:], sd[:])
                        else:
                            nc.vector.tensor_add(s_run[:], s_run[:], sd[:])
                        if i < N_CH - 1:
                            sb = sbf_pool.tile([P, HD1], BF16, name=f"sbf{h}_{i}")
                            nc.scalar.copy(sb[:], s_run[:])
                            s_bf[h][i + 1] = sb
                    nc.scalar.copy(s_all[:, h * HD1:(h + 1) * HD1], s_run[:])
                for s in range(N_CORES):
                    ms = p2msk.tile([P, SAW], BF16, name=f"ms{s}", tag="ms")
                    nc.vector.tensor_scalar_mul(ms[:], s_all[:], smask_sb[:, s:s + 1])
                    nc.sync.dma_start(out=rs_in[s, :, :], in_=ms[:])
                nc.gpsimd.collective_compute(
                    "ReduceScatter",
                    mybir.AluOpType.add,
                    ins=[rs_in.ap().opt()],
                    outs=[rs_out.ap().opt()],
                    replica_groups=groups,
                )

            # Q projections (j = 16..31), overlap the collective
            for j in range(H):
                w_t = p1w.tile([P, C], BF16, name=f"qw{j}", tag="qkw")
                nc.sync.dma_start(out=w_t[:], in_=qkw[H + j, :, :])
                ps = ps_t(f"qps{j}", "kq", 4)
                for k in range(N_KC):
                    nc.tensor.matmul(
                        ps[:], w_t[:, k * P:(k + 1) * P], n1T[k][:],
                        start=(k == 0), stop=(k == N_KC - 1),
                    )
                rope_elu(qres[j], ps, p1r, f"q{j}")

        # ---- phase 3: local masked scores (no state dependency) ------
        am_ctx = ExitStack()
        am_pool = am_ctx.enter_context(tc.tile_pool(name="amres", bufs=1))
        am = [[None] * N_CH for _ in range(H)]
        if True:
            for h in range(H):
                for i in range(N_CH):
                    tcol = slice(i * P, (i + 1) * P)
                    a_ps = ps_t(f"a{h}_{i}", "a", 1, cols=P)
                    nc.tensor.matmul(
                        a_ps[:], kres[h][:, tcol], qres[h][:, tcol],
                        start=True, stop=True,
                    )
                    am_t = am_pool.tile([P, P], BF16, name=f"am{h}_{i}")
                    nc.vector.tensor_mul(am_t[:], a_ps[:], mask_sb[:])
                    am[h][i] = am_t

        # ---- phase 4: y = (q@S_loc + Am^T@V' + q@S_init) / den; y^T --
        with (
            tc.tile_pool(name="p4si", bufs=1) as p4si,
            tc.tile_pool(name="p4y", bufs=4) as p4y,
        ):
            sinit = p4si.tile([P, SAW], BF16, name="sinit")
            with tc.high_priority():
                nc.sync.dma_start(out=sinit[:], in_=rs_out[:, :])
            for h in range(H):
                hcol = slice(h * HD1, (h + 1) * HD1)
                for i in range(N_CH):
                    tcol = slice(i * P, (i + 1) * P)
                    y_ps = ps_t(f"y{h}_{i}", "kq", 4, cols=HD1)
                    if i > 0:
                        nc.tensor.matmul(
                            y_ps[:], qres[h][:, tcol], s_bf[h][i][:],
                            start=True, stop=False,
                        )
                    nc.tensor.matmul(
                        y_ps[:], am[h][i][:], vp[h][i][:],
                        start=(i == 0), stop=False,
                    )
                    nc.tensor.matmul(
                        y_ps[:], qres[h][:, tcol], sinit[:, hcol],
                        start=False, stop=True,
                    )
                    rec = p4y.tile([P, 1], F32, name=f"rec{h}_{i}", tag="rec")
                    nc.vector.reciprocal(rec[:], y_ps[:, HD:HD1])
                    yb = p4y.tile([P, HD], BF16, name=f"yb{h}_{i}", tag="yb")
                    nc.scalar.activation(
                        yb[:], y_ps[:, 0:HD], AF.Identity, scale=rec[:]
                    )
                    tr = ps_t(f"ytr{h}_{i}", "v", 2, cols=P, dtype=BF16)
                    nc.tensor.transpose(tr[:], yb[:], ident_bf[:])
                    nc.scalar.copy(yT[h][i][:], tr[:])
        am_ctx.close()
        att_ctx.close()

        # ---- phase 5: proj, residual, rmsnorm2 -> n2T ----------------
        x2_ctx = ExitStack()
        x2_pool = x2_ctx.enter_context(tc.tile_pool(name="x2res", bufs=1))
        x2_res = [x2_pool.tile([P, C], F32, name=f"x2_{i}") for i in range(N_RT)]
        n2T_ctx = ExitStack()
        n2T_pool = n2T_ctx.enter_context(tc.tile_pool(name="n2T", bufs=1))
        n2T = [n2T_pool.tile([P, R_LOC], BF16, name=f"n2T{k}") for k in range(N_KC)]
        with (
            tc.tile_pool(name="p5w", bufs=2) as p5w,
            tc.tile_pool(name="p5sq", bufs=1) as p5sq,
            tc.tile_pool(name="p5st", bufs=8) as p5st,
            tc.tile_pool(name="p5n", bufs=1) as p5n,
        ):
            for mt in range(N_RT):
                nc.sync.dma_start(
                    out=x2_res[mt][:], in_=x_loc[mt * P:(mt + 1) * P, :]
                )
            for ont in range(4):
                ocol = slice(ont * 512, (ont + 1) * 512)
                w_t = p5w.tile([P, N_KC * 512], BF16, name=f"pw{ont}", tag="pw")
                nc.sync.dma_start(out=w_t[:], in_=pw[ont, :, :])
                ps_mt = [ps_t(f"h{ont}_{mt}", "kq", 4) for mt in range(N_RT)]
                for kd in range(N_KC):
                    for mt in range(N_RT):
                        nc.tensor.matmul(
                            ps_mt[mt][:], yT[kd][mt][:],
                            w_t[:, kd * 512:(kd + 1) * 512],
                            start=(kd == 0), stop=(kd == N_KC - 1),
                        )
                for mt in range(N_RT):
                    nc.vector.tensor_add(
                        x2_res[mt][:, ocol], x2_res[mt][:, ocol], ps_mt[mt][:]
                    )
            n2_ts = []
            for mt in range(N_RT):
                sq = p5sq.tile([P, C], F32, name=f"sq2_{mt}", tag="sq2")
                ss = p5st.tile([P, 1], F32, name=f"ss2_{mt}", tag="ss2")
                nc.scalar.activation(sq[:], x2_res[mt][:], AF.Square, accum_out=ss[:])
                rms = p5st.tile([P, 1], F32, name=f"rms2_{mt}", tag="rms2")
                nc.scalar.activation(
                    rms[:], ss[:], AF.Sqrt, bias=eps_t[:], scale=1.0 / C
                )
                inv = p5st.tile([P, 1], F32, name=f"inv2_{mt}", tag="inv2")
                nc.vector.reciprocal(inv[:], rms[:])
                n_t = p5n.tile([P, C], F32, name=f"n2_{mt}", tag=f"n2_{mt}")
                nc.vector.tensor_scalar_mul(n_t[:], x2_res[mt][:], inv[:])
                n2_ts.append(n_t)
            for k in range(N_KC):
                ps = ps_t(f"tr2_{k}", "v", 2)
                for mt in range(N_RT):
                    nc.tensor.transpose(
                        ps[:, mt * P:(mt + 1) * P],
                        n2_ts[mt][:, k * P:(k + 1) * P], ident_f32[:],
                    )
                nc.scalar.copy(n2T[k][:], ps[:])
        yT_ctx.close()

        # ---- phase 6: fc + gelu -> gT (resident) ---------------------
        gT_ctx = ExitStack()
        gT_pool = gT_ctx.enter_context(tc.tile_pool(name="gT", bufs=1))
        gT = [gT_pool.tile([P, R_LOC], BF16, name=f"gT{mf}") for mf in range(N_MF)]
        p7w_ctx = ExitStack()
        p7w = p7w_ctx.enter_context(tc.tile_pool(name="p7w", bufs=2))
        with (
            tc.tile_pool(name="p6w", bufs=3) as p6w,
        ):
            for mf in range(N_MF):
                w_t = p6w.tile([P, C], BF16, name=f"fcw{mf}", tag="fcw")
                nc.sync.dma_start(out=w_t[:], in_=fcw[mf, :, :])
                ps = ps_t(f"g{mf}", "kq", 4)
                for k in range(N_KC):
                    nc.tensor.matmul(
                        ps[:], w_t[:, k * P:(k + 1) * P], n2T[k][:],
                        start=(k == 0), stop=(k == N_KC - 1),
                    )
                nc.scalar.activation(gT[mf][:], ps[:], AF.Gelu)

        # ---- phase 7: mlp proj + residual -> out ---------------------
        with (
            tc.tile_pool(name="p7o", bufs=4) as p7o,
        ):
            for ch in range(N_MCH):
                w_t = p7w.tile([P, N_MF * MLP_CC], BF16, name=f"mw{ch}", tag="mw")
                nc.sync.dma_start(out=w_t[:], in_=mww[ch, :, :])
                for mt in range(N_RT):
                    mcol = slice(mt * P, (mt + 1) * P)
                    ps = ps_t(f"f{ch}_{mt}", "kq", 4, cols=MLP_CC)
                    for kf in range(N_MF):
                        nc.tensor.matmul(
                            ps[:],
                            gT[kf][:, mcol],
                            w_t[:, kf * MLP_CC:(kf + 1) * MLP_CC],
                            start=(kf == 0), stop=(kf == N_MF - 1),
                        )
                    o_t = p7o.tile([P, MLP_CC], F32, name=f"o{ch}_{mt}", tag="o")
                    nc.vector.tensor_add(
                        o_t[:],
                        x2_res[mt][:, ch * MLP_CC:(ch + 1) * MLP_CC],
                        ps[:],
                    )
                    nc.scalar.dma_start(
                        out=out_loc[
                            mt * P:(mt + 1) * P,
                            ch * MLP_CC:(ch + 1) * MLP_CC,
                        ],
                        in_=o_t[:],
                    )
        p7w_ctx.close()
        gT_ctx.close()
        n2T_ctx.close()
        x2_ctx.close()
        stk.close()

    return nc


_NC_CACHE = None


def _get_nc():
    global _NC_CACHE
    if _NC_CACHE is None:
        _NC_CACHE = build_nc()
    return _NC_CACHE


def _prep_inputs(x, cos, sin, attention_bias, norm1_w, norm2_w, attn_w, proj_w,
                 fc_w, mlp_proj_w):
    bf = ml_dtypes.bfloat16
    xf = np.asarray(x, np.float32).reshape(R, C)
    w1 = np.asarray(norm1_w, np.float32)
    w2 = np.asarray(norm2_w, np.float32)
    aw = np.asarray(attn_w, np.float32) * w1[None, :]      # [3C, C] (norm folded)
    pwf = np.asarray(proj_w, np.float32)                   # [C, C]
    fwf = np.asarray(fc_w, np.float32) * w2[None, :]       # [F, C]
    mwf = np.asarray(mlp_proj_w, np.float32)               # [C, F]
    cosf = np.asarray(cos, np.float32)                     # [T, 64]
    sinf = np.asarray(sin, np.float32)

    awr = aw.reshape(H, 3, HD, C)
    # qkw[j<H] = K-weights of head j; qkw[j>=H] = Q-weights of head j-H.
    # qkw[j, p, k*128+m] = awr[h, comp, m, k*128+p]
    qk = np.empty((2 * H, P, C), np.float32)
    for h in range(H):
        qk[h] = awr[h, 1].T.reshape(N_KC, P, HD).transpose(1, 0, 2).reshape(P, C)
        qk[H + h] = awr[h, 0].T.reshape(N_KC, P, HD).transpose(1, 0, 2).reshape(P, C)
    # vw[half, k, p, (h-8*half)*128+d] = awr[h, 2, d, k*128+p]
    vwt = (
        awr[:, 2].reshape(H * HD, C).T.reshape(N_KC, P, 2, C // 2)
        .transpose(2, 0, 1, 3)
    )
    # pw[ont, p, kd*512+co] = proj_w[ont*512+co, kd*128+p]
    pwt = np.ascontiguousarray(
        pwf.reshape(4, 512, N_KC, P).transpose(0, 3, 2, 1)
    ).reshape(4, P, N_KC * 512)
    # fcw[mf, p, k*128+f] = fwf[mf*128+f, k*128+p]
    fct = np.ascontiguousarray(
        fwf.reshape(N_MF, P, N_KC, P).transpose(0, 3, 2, 1)
    ).reshape(N_MF, P, C)
    # mww[ch, p, kf*CC+c] = mwf[ch*CC+c, kf*128+p]
    mwt = np.ascontiguousarray(
        mwf.reshape(N_MCH, MLP_CC, N_MF, P).transpose(0, 3, 2, 1)
    ).reshape(N_MCH, P, N_MF * MLP_CC)

    qk_b = np.ascontiguousarray(qk).astype(bf)
    vw_b = np.ascontiguousarray(vwt).astype(bf)
    pw_b = np.ascontiguousarray(pwt).astype(bf)
    fc_b = fct.astype(bf)
    mw_b = mwt.astype(bf)
    # mask[s, t] = 1 iff s <= t  (transposed causal tril)
    maskT = np.triu(np.ones((P, P), np.float32))

    in_maps = []
    for c in range(N_CORES):
        t0 = (c % (N_CORES // B)) * R_LOC
        sm = np.zeros((P, N_CORES), np.float32)
        for s in range(N_CORES):
            if s // (N_CORES // B) == c // (N_CORES // B) and s > c:
                sm[:, s] = 1.0
        in_maps.append({
            "x_loc": np.ascontiguousarray(xf[R_LOC * c:R_LOC * (c + 1)]),
            "cosr": np.ascontiguousarray(cosf[t0:t0 + R_LOC].T).astype(bf),
            "sinr": np.ascontiguousarray(sinf[t0:t0 + R_LOC].T).astype(bf),
            "maskT": maskT,
            "smask": sm,
            "qkw": qk_b,
            "vw": vw_b,
            "pw": pw_b,
            "fcw": fc_b,
            "mww": mw_b,
        })
    return in_maps


def kernel(**inputs):
    nc = _get_nc()
    in_maps = _prep_inputs(**inputs)
    res = run_bass_kernel_spmd(nc, in_maps, list(range(N_CORES)))
    out = np.concatenate(
        [np.asarray(res.results[c]["out_loc"], np.float32) for c in range(N_CORES)],
        axis=0,
    )
    return out.reshape(B, T, C)


# revision 12
# speedup vs baseline: 1.9839x; 1.0380x over previous
"""Trainium2 Bass kernel for nn_Block_42460046688864 (dense transformer block).

Reference math (B=2, T=2048, C=2048, H=16, HD=128):
    n1  = rmsnorm(x) * norm1_w
    qkv = n1 @ attn_w.T ; q,k,v per head ; q,k = rope(q,k) ; phi = elu(.)+1
    w   = (phi_q . phi_k) * scale * tril ; w /= sum(w) ; y = w @ v
    h   = y @ proj_w.T ; x2 = x + h
    ffn = gelu(rmsnorm(x2)*norm2_w @ fc_w.T) @ mlp_proj_w.T ; out = x2 + ffn

Distribution (8 NeuronCores, one NEFF, fully data-parallel):
  - rows (b*T+t, 4096 total) sharded 512/core; every core streams the FULL
    weights from its own HBM (no activation collectives at all).
  - attention is chunked linear attention (causal tril + positive elu+1
    features == prefix-state form; scale and eps cancel to ~1e-9 rel).
    The only cross-core dependency is the causal prefix state: each core's
    segment state S_seg[h] = sum_t k_t (x) [v_t | 1] is exchanged with ONE
    small ReduceScatter. Core j writes S_seg * mask[j<s, same-seq] into
    slot s, so after the add-RS core s holds exactly the sum of its
    same-sequence predecessors' states (its causal init state). The RS is
    issued right after K/V are ready and overlaps the Q projection; the
    correction q @ S_init is fused into each chunk's PSUM accumulation.
  - V is computed directly in [token, dim] layout by using n1^T chunks as
    the stationary matmul operand (no V transposes); K additionally needs
    [token, dim] for the state outer products -> 64 small DMA transposes.

Notes:
  - norm weights are folded into attn_w / fc_w on the host (exact algebra).
  - matmul operands are bf16 (fp32 PSUM accumulation); norms, residuals and
    attention numerators/denominators stay fp32 (psum) end to end.
  - weights are pre-tiled on the host into [128 x N] DMA slabs so every
    weight DMA is one contiguous >=2KB-per-partition block.
  - SBUF pools are strict LIFO per side; long-lived attention tiles live on
    the left stack, y^T on the right stack so lifetimes nest.
  - TileContext's tail drain is patched to split its semaphore waits:
    this walrus build rejects >2 sync waits on one TPB_CTRL instruction.
"""

from contextlib import ExitStack

import numpy as np
import ml_dtypes

import concourse.bass as bass
import concourse.mybir as mybir
import concourse.tile as tile
from concourse.bass_utils import run_bass_kernel_spmd
from concourse.masks import make_identity
from bass_rust import ScopedClock

F32 = mybir.dt.float32
BF16 = mybir.dt.bfloat16
AF = mybir.ActivationFunctionType

N_CORES = 8
B, T, C, H, HD = 2, 2048, 2048, 16, 128
F = 4 * C                  # 8192 mlp hidden
R = B * T                  # 4096 flattened rows (b-major)
R_LOC = R // N_CORES       # 512 rows per core
P = 128
EPS_NORM = 1e-5
N_RT = R_LOC // P          # 4 local row tiles
N_KC = C // P              # 16 contraction tiles over C
N_CH = N_RT                # 4 local causal chunks
N_MF = F // P              # 64 mlp-hidden tiles
HD1 = HD + 1               # state cols: [v dims | 1]
SAW = H * HD1              # 2064 = all-head state cols
MLP_CC = 256               # mlp output col-chunk
N_MCH = C // MLP_CC        # 8 col chunks

_MAX_WAITS = 2  # this walrus build rejects >2 sync waits per instruction


def _split_excess_waits(nc):
    """Move excess semaphore waits onto same-engine NoOps ahead of the op."""
    for fn in nc.m.functions:
        for bb in fn.blocks:
            insts = list(bb.instructions)
            out = []
            for ins in insts:
                si = getattr(ins, "sync_info", None)
                waits = list(si.on_wait) if si and si.on_wait else []
                sem_waits = [w for w in waits if w.sync_type == "semaphore"]
                if len(sem_waits) > _MAX_WAITS:
                    keep = [w for w in waits if w.sync_type != "semaphore"]
                    keep += sem_waits[: _MAX_WAITS - 1] if _MAX_WAITS > 1 else []
                    extra = sem_waits[_MAX_WAITS - 1:] if _MAX_WAITS > 1 else sem_waits
                    for j in range(0, len(extra), _MAX_WAITS):
                        chunk = extra[j:j + _MAX_WAITS]
                        nop = mybir.InstNoOp(
                            name=nc.get_next_instruction_name(), ins=[], outs=[]
                        )
                        nop.engine = ins.engine
                        nop.sync_info = mybir.SyncInfo(on_wait=chunk, on_update=[])
                        out.append(nop)
                    si.on_wait[:] = keep
                out.append(ins)
            if len(out) != len(insts):
                bb.instructions[:] = out


class _TC(tile.TileContext):
    """TileContext whose tail drain splits sem waits one-per-NOP."""

    def schedule_and_allocate(self):
        ret = super().schedule_and_allocate()
        _split_excess_waits(self.nc)
        return ret

    def _drain_and_barrier(self, tick_clock, wait_clock):
        probe = self.nc.sync.nop(nofuse=True, hint="drain_waits")
        wait_clock.add_sem_waits(
            probe.ins, ScopedClock({None: tick_clock.global_clock})
        )
        si = probe.ins.sync_info
        waits = list(si.on_wait) if si and si.on_wait else []
        if len(waits) > 1:
            si.on_wait[:] = waits[:1]
            for w in waits[1:]:
                extra = self.nc.sync.nop(nofuse=True, hint="drain_waits")
                extra.ins.sync_info = mybir.SyncInfo(on_wait=[w], on_update=[])
        self.nc.sync.drain()
        self.nc.all_engine_barrier()
        popped = self.nc._tile_sem_poison_stack.pop()
        assert popped is self._sem_poison
        self.nc.clear_and_free_semaphores(list(self.sems.allocated().values()))
        self.nc.all_engine_barrier()


def build_nc():
    nc = bass.Bass(target_bir_lowering=False)

    x_loc = nc.declare_dram_parameter("x_loc", [R_LOC, C], F32, isOutput=False)
    cosr = nc.declare_dram_parameter("cosr", [HD // 2, R_LOC], BF16, isOutput=False)
    sinr = nc.declare_dram_parameter("sinr", [HD // 2, R_LOC], BF16, isOutput=False)
    maskT = nc.declare_dram_parameter("maskT", [P, P], F32, isOutput=False)
    smask = nc.declare_dram_parameter("smask", [P, N_CORES], F32, isOutput=False)
    # pre-tiled weight slabs (see _prep_inputs for layouts)
    qkw = nc.declare_dram_parameter("qkw", [2 * H, P, C], BF16, isOutput=False)
    vw = nc.declare_dram_parameter("vw", [2, N_KC, P, C // 2], BF16, isOutput=False)
    pw = nc.declare_dram_parameter("pw", [4, P, N_KC * 512], BF16, isOutput=False)
    fcw = nc.declare_dram_parameter("fcw", [N_MF, P, C], BF16, isOutput=False)
    mww = nc.declare_dram_parameter(
        "mww", [N_MCH, P, N_MF * MLP_CC], BF16, isOutput=False
    )
    out_loc = nc.declare_dram_parameter("out_loc", [R_LOC, C], F32, isOutput=True)

    rs_in = nc.dram_tensor("rs_in", [N_CORES, P, SAW], BF16)
    rs_out = nc.dram_tensor("rs_out", [P, SAW], BF16)

    groups = [list(range(N_CORES))]

    with _TC(nc) as tc:
        stk = ExitStack()
        const = stk.enter_context(tc.tile_pool(name="const", bufs=1))
        psum = stk.enter_context(tc.tile_pool(name="psum", bufs=1, space="PSUM"))
        def ps_t(name, tag, bufs, cols=512, dtype=F32):
            return psum.tile([P, cols], dtype, name=name, tag=tag, bufs=bufs)
        ident_f32 = const.tile([P, P], F32)
        make_identity(nc, ident_f32)
        ident_bf = const.tile([P, P], BF16)
        make_identity(nc, ident_bf)
        mask_sb = const.tile([P, P], F32)
        nc.sync.dma_start(out=mask_sb[:], in_=maskT[:, :])
        smask_sb = const.tile([P, N_CORES], F32)
        nc.sync.dma_start(out=smask_sb[:], in_=smask[:, :])
        eps_t = const.tile([P, 1], F32)
        nc.vector.memset(eps_t[:], EPS_NORM)
        cos_sb = const.tile([HD // 2, R_LOC], BF16)
        sin_sb = const.tile([HD // 2, R_LOC], BF16)
        nc.sync.dma_start(out=cos_sb[:], in_=cosr[:, :])
        nc.sync.dma_start(out=sin_sb[:], in_=sinr[:, :])

        # y^T on the right stack: outlives the attention residents (left).
        yT_ctx = ExitStack()
        yT_pool = yT_ctx.enter_context(tc.tile_pool(name="yT", bufs=1, side="right"))
        yT = [
            [yT_pool.tile([P, P], BF16, name=f"yT{h}_{i}") for i in range(N_CH)]
            for h in range(H)
        ]

        # attention residents (left): released together after phase 4.
        att_ctx = ExitStack()
        n1T_pool = att_ctx.enter_context(tc.tile_pool(name="n1T", bufs=1))
        qk_pool = att_ctx.enter_context(tc.tile_pool(name="qkres", bufs=1))
        vp_pool = att_ctx.enter_context(tc.tile_pool(name="vpres", bufs=1))
        sbf_pool = att_ctx.enter_context(tc.tile_pool(name="sbfres", bufs=1))
        n1T = [n1T_pool.tile([P, R_LOC], BF16, name=f"n1T{k}") for k in range(N_KC)]
        kres = [qk_pool.tile([P, R_LOC], BF16, name=f"k{h}") for h in range(H)]
        qres = [qk_pool.tile([P, R_LOC], BF16, name=f"q{h}") for h in range(H)]
        vp = [
            [vp_pool.tile([P, HD1], BF16, name=f"vp{h}_{i}") for i in range(N_CH)]
            for h in range(H)
        ]

        # ---- phase 0: rmsnorm(x) -> n1T (SBUF-resident, transposed) ----
        with (
            tc.tile_pool(name="p0x", bufs=2) as p0x,
            tc.tile_pool(name="p0sq", bufs=1) as p0sq,
            tc.tile_pool(name="p0st", bufs=8) as p0st,
            tc.tile_pool(name="p0n", bufs=1) as p0n,
        ):
            n_ts = []
            for i in range(N_RT):
                x_t = p0x.tile([P, C], F32, name=f"x{i}", tag="x")
                nc.sync.dma_start(out=x_t[:], in_=x_loc[i * P:(i + 1) * P, :])
                sq = p0sq.tile([P, C], F32, name=f"sq{i}", tag="sq")
                ss = p0st.tile([P, 1], F32, name=f"ss{i}", tag="ss")
                nc.scalar.activation(sq[:], x_t[:], AF.Square, accum_out=ss[:])
                rms = p0st.tile([P, 1], F32, name=f"rms{i}", tag="rms")
                nc.scalar.activation(
                    rms[:], ss[:], AF.Sqrt, bias=eps_t[:], scale=1.0 / C
                )
                inv = p0st.tile([P, 1], F32, name=f"inv{i}", tag="inv")
                nc.vector.reciprocal(inv[:], rms[:])
                n_t = p0n.tile([P, C], F32, name=f"n{i}", tag=f"n{i}")
                nc.vector.tensor_scalar_mul(n_t[:], x_t[:], inv[:])
                n_ts.append(n_t)
            for k in range(N_KC):
                ps = ps_t(f"tr{k}", "v", 2)
                for i in range(N_RT):
                    nc.tensor.transpose(
                        ps[:, i * P:(i + 1) * P],
                        n_ts[i][:, k * P:(k + 1) * P], ident_f32[:],
                    )
                nc.scalar.copy(n1T[k][:], ps[:])

        # ---- phase 1: K then V then Q projections --------------------
        HF = HD // 2

        def rope_elu(dst, ps, pool, uid):
            """psum [128 x 512] (d-major head tile) -> phi(rope(.)) bf16."""
            raw = pool.tile([P, R_LOC], BF16, name=f"raw{uid}", tag="raw")
            nc.scalar.copy(raw[:], ps[:])
            s1 = pool.tile([HF, R_LOC], BF16, name=f"s1{uid}", tag="s1")
            s2 = pool.tile([HF, R_LOC], BF16, name=f"s2{uid}", tag="s2")
            ro = pool.tile([P, R_LOC], BF16, name=f"ro{uid}", tag="ro")
            nc.vector.tensor_mul(s1[:], raw[0:HF, :], cos_sb[:])
            nc.vector.tensor_mul(s2[:], raw[HF:P, :], sin_sb[:])
            nc.vector.tensor_sub(ro[0:HF, :], s1[:], s2[:])
            nc.vector.tensor_mul(s1[:], raw[0:HF, :], sin_sb[:])
            nc.vector.tensor_mul(s2[:], raw[HF:P, :], cos_sb[:])
            nc.vector.tensor_add(ro[HF:P, :], s1[:], s2[:])
            # phi = elu(ro)+1 = max(ro,0) + exp(min(ro,0))
            rl = pool.tile([P, R_LOC], BF16, name=f"rl{uid}", tag="rl")
            nc.vector.tensor_scalar_max(rl[:], ro[:], 0.0)
            dm = pool.tile([P, R_LOC], BF16, name=f"dm{uid}", tag="dm")
            nc.vector.tensor_scalar_min(dm[:], ro[:], 0.0)
            ex = pool.tile([P, R_LOC], BF16, name=f"ex{uid}", tag="ex")
            nc.scalar.activation(ex[:], dm[:], AF.Exp)
            nc.vector.tensor_add(dst[:], rl[:], ex[:])

        with (
            tc.tile_pool(name="p1w", bufs=5) as p1w,
            tc.tile_pool(name="p1vw", bufs=1) as p1vw,
            tc.tile_pool(name="p1r", bufs=6) as p1r,
        ):
            # K projections (j = 0..15), rope+elu on DVE as tiles land
            for j in range(H):
                w_t = p1w.tile([P, C], BF16, name=f"kw{j}", tag="qkw")
                nc.sync.dma_start(out=w_t[:], in_=qkw[j, :, :])
                ps = ps_t(f"kps{j}", "kq", 4)
                for k in range(N_KC):
                    nc.tensor.matmul(
                        ps[:], w_t[:, k * P:(k + 1) * P], n1T[k][:],
                        start=(k == 0), stop=(k == N_KC - 1),
                    )
                rope_elu(kres[j], ps, p1r, f"k{j}")

            # V projections, directly transposed: [t x hv] chunks
            if True:
                for half in range(2):
                    vw_sb = []
                    for k in range(N_KC):
                        w_t = p1vw.tile(
                            [P, C // 2], BF16, name=f"vw{half}_{k}", tag=f"vw{k}"
                        )
                        nc.sync.dma_start(out=w_t[:], in_=vw[half, k, :, :])
                        vw_sb.append(w_t)
                    for vb in range(2):
                        vcol = slice(vb * 512, (vb + 1) * 512)
                        for i in range(N_CH):
                            icol = slice(i * P, (i + 1) * P)
                            ps = ps_t(f"v{half}_{vb}_{i}", "v", 2)
                            for k in range(N_KC):
                                nc.tensor.matmul(
                                    ps[:], n1T[k][:, icol], vw_sb[k][:, vcol],
                                    start=(k == 0), stop=(k == N_KC - 1),
                                )
                            for hs in range(4):
                                h = half * 8 + vb * 4 + hs
                                nc.scalar.copy(
                                    vp[h][i][:, 0:HD], ps[:, hs * P:(hs + 1) * P]
                                )
                                nc.vector.memset(vp[h][i][:, HD:HD1], 1.0)

            # ---- phase 2: segment states + masked RS exchange --------
            # (emitted before Q so the collective overlaps Q/scores)
            s_bf = [[None] * N_CH for _ in range(H)]
            with (
                tc.high_priority(),
                tc.tile_pool(name="p2kp", bufs=8) as p2kp,
                tc.tile_pool(name="p2all", bufs=1) as p2all,
                tc.tile_pool(name="p2msk", bufs=2) as p2msk,
            ):
                s_all = p2all.tile([P, SAW], BF16, name="s_all")
                for h in range(H):
                    kp_t = []
                    for i in range(N_CH):
                        tcol = slice(i * P, (i + 1) * P)
                        kp = p2kp.tile([P, P], BF16, name=f"kp{h}_{i}", tag="kp")
                        nc.sync.dma_start_transpose(kp[:], kres[h][:, tcol])
                        kp_t.append(kp)
                    # local prefix states P_m = sum_{i<m} kp_i^T @ [v_i | 1]
                    # (redundant accumulations: no cross-engine scan chain)
                    for m in range(1, N_CH + 1):
                        ps = ps_t(f"pfx{h}_{m}", "sd", 2, cols=HD1)
                        for i in range(m):
                            nc.tensor.matmul(
                                ps[:], kp_t[i][:], vp[h][i][:],
                                start=(i == 0), stop=(i == m - 1),
                            )
                        if m < N_CH:
                            sb = sbf_pool.tile([P, HD1], BF16, name=f"sbf{h}_{m}")
                            nc.scalar.copy(sb[:], ps[:])
                            s_bf[h][m] = sb
                        else:
                            nc.scalar.copy(s_all[:, h * HD1:(h + 1) * HD1], ps[:])
                for s in range(N_CORES):
                    ms = p2msk.tile([P, SAW], BF16, name=f"ms{s}", tag="ms")
                    nc.vector.tensor_scalar_mul(ms[:], s_all[:], smask_sb[:, s:s + 1])
                    nc.sync.dma_start(out=rs_in[s, :, :], in_=ms[:])
                nc.gpsimd.collective_compute(
                    "ReduceScatter",
                    mybir.AluOpType.add,
                    ins=[rs_in.ap().opt()],
                    outs=[rs_out.ap().opt()],
                    replica_groups=groups,
                )

            # Q projections (j = 16..31), overlap the collective
            for j in range(H):
                w_t = p1w.tile([P, C], BF16, name=f"qw{j}", tag="qkw")
                nc.sync.dma_start(out=w_t[:], in_=qkw[H + j, :, :])
                ps = ps_t(f"qps{j}", "kq", 4)
                for k in range(N_KC):
                    nc.tensor.matmul(
                        ps[:], w_t[:, k * P:(k + 1) * P], n1T[k][:],
                        start=(k == 0), stop=(k == N_KC - 1),
                    )
                rope_elu(qres[j], ps, p1r, f"q{j}")

        # ---- phase 3: local masked scores (no state dependency) ------
        am_ctx = ExitStack()
        am_pool = am_ctx.enter_context(tc.tile_pool(name="amres", bufs=1))
        am = [[None] * N_CH for _ in range(H)]
        if True:
            for h in range(H):
                for i in range(N_CH):
                    tcol = slice(i * P, (i + 1) * P)
                    a_ps = ps_t(f"a{h}_{i}", "sd", 2, cols=P)
                    nc.tensor.matmul(
                        a_ps[:], kres[h][:, tcol], qres[h][:, tcol],
                        start=True, stop=True,
                    )
                    am_t = am_pool.tile([P, P], BF16, name=f"am{h}_{i}")
                    nc.vector.tensor_mul(am_t[:], a_ps[:], mask_sb[:])
                    am[h][i] = am_t

        # ---- phase 4: y = (q@S_loc + Am^T@V' + q@S_init) / den; y^T --
        with (
            tc.tile_pool(name="p4si", bufs=1) as p4si,
            tc.tile_pool(name="p4y", bufs=4) as p4y,
        ):
            sinit = p4si.tile([P, SAW], BF16, name="sinit")
            with tc.high_priority():
                nc.sync.dma_start(out=sinit[:], in_=rs_out[:, :])
            for h in range(H):
                hcol = slice(h * HD1, (h + 1) * HD1)
                for i in range(N_CH):
                    tcol = slice(i * P, (i + 1) * P)
                    y_ps = ps_t(f"y{h}_{i}", "kq", 4, cols=HD1)
                    if i > 0:
                        nc.tensor.matmul(
                            y_ps[:], qres[h][:, tcol], s_bf[h][i][:],
                            start=True, stop=False,
                        )
                    nc.tensor.matmul(
                        y_ps[:], am[h][i][:], vp[h][i][:],
                        start=(i == 0), stop=False,
                    )
                    nc.tensor.matmul(
                        y_ps[:], qres[h][:, tcol], sinit[:, hcol],
                        start=False, stop=True,
                    )
                    rec = p4y.tile([P, 1], F32, name=f"rec{h}_{i}", tag="rec")
                    nc.vector.reciprocal(rec[:], y_ps[:, HD:HD1])
                    yb = p4y.tile([P, HD], BF16, name=f"yb{h}_{i}", tag="yb")
                    nc.scalar.activation(
                        yb[:], y_ps[:, 0:HD], AF.Identity, scale=rec[:]
                    )
                    tr = ps_t(f"ytr{h}_{i}", "v", 2, cols=P, dtype=BF16)
                    nc.tensor.transpose(tr[:], yb[:], ident_bf[:])
                    nc.scalar.copy(yT[h][i][:], tr[:])
        am_ctx.close()
        att_ctx.close()

        # ---- phase 5: proj, residual, rmsnorm2 -> n2T ----------------
        x2_ctx = ExitStack()
        x2_pool = x2_ctx.enter_context(tc.tile_pool(name="x2res", bufs=1))
        x2_res = [x2_pool.tile([P, C], F32, name=f"x2_{i}") for i in range(N_RT)]
        n2T_ctx = ExitStack()
        n2T_pool = n2T_ctx.enter_context(tc.tile_pool(name="n2T", bufs=1))
        n2T = [n2T_pool.tile([P, R_LOC], BF16, name=f"n2T{k}") for k in range(N_KC)]
        with (
            tc.tile_pool(name="p5w", bufs=2) as p5w,
            tc.tile_pool(name="p5sq", bufs=1) as p5sq,
            tc.tile_pool(name="p5st", bufs=8) as p5st,
            tc.tile_pool(name="p5n", bufs=1) as p5n,
        ):
            for mt in range(N_RT):
                nc.sync.dma_start(
                    out=x2_res[mt][:], in_=x_loc[mt * P:(mt + 1) * P, :]
                )
            for ont in range(4):
                ocol = slice(ont * 512, (ont + 1) * 512)
                w_t = p5w.tile([P, N_KC * 512], BF16, name=f"pw{ont}", tag="pw")
                nc.sync.dma_start(out=w_t[:], in_=pw[ont, :, :])
                ps_mt = [ps_t(f"h{ont}_{mt}", "kq", 4) for mt in range(N_RT)]
                for kd in range(N_KC):
                    for mt in range(N_RT):
                        nc.tensor.matmul(
                            ps_mt[mt][:], yT[kd][mt][:],
                            w_t[:, kd * 512:(kd + 1) * 512],
                            start=(kd == 0), stop=(kd == N_KC - 1),
                        )
                for mt in range(N_RT):
                    nc.vector.tensor_add(
                        x2_res[mt][:, ocol], x2_res[mt][:, ocol], ps_mt[mt][:]
                    )
            n2_ts = []
            for mt in range(N_RT):
                sq = p5sq.tile([P, C], F32, name=f"sq2_{mt}", tag="sq2")
                ss = p5st.tile([P, 1], F32, name=f"ss2_{mt}", tag="ss2")
                nc.scalar.activation(sq[:], x2_res[mt][:], AF.Square, accum_out=ss[:])
                rms = p5st.tile([P, 1], F32, name=f"rms2_{mt}", tag="rms2")
                nc.scalar.activation(
                    rms[:], ss[:], AF.Sqrt, bias=eps_t[:], scale=1.0 / C
                )
                inv = p5st.tile([P, 1], F32, name=f"inv2_{mt}", tag="inv2")
                nc.vector.reciprocal(inv[:], rms[:])
                n_t = p5n.tile([P, C], F32, name=f"n2_{mt}", tag=f"n2_{mt}")
                nc.vector.tensor_scalar_mul(n_t[:], x2_res[mt][:], inv[:])
                n2_ts.append(n_t)
            for k in range(N_KC):
                ps = ps_t(f"tr2_{k}", "v", 2)
                for mt in range(N_RT):
                    nc.tensor.transpose(
                        ps[:, mt * P:(mt + 1) * P],
                        n2_ts[mt][:, k * P:(k + 1) * P], ident_f32[:],
                    )
                nc.scalar.copy(n2T[k][:], ps[:])
        yT_ctx.close()

        # ---- phase 6: fc + gelu -> gT (resident) ---------------------
        gT_ctx = ExitStack()
        gT_pool = gT_ctx.enter_context(tc.tile_pool(name="gT", bufs=1))
        gT = [gT_pool.tile([P, R_LOC], BF16, name=f"gT{mf}") for mf in range(N_MF)]
        p7w_ctx = ExitStack()
        p7w = p7w_ctx.enter_context(tc.tile_pool(name="p7w", bufs=2))
        with (
            tc.tile_pool(name="p6w", bufs=3) as p6w,
        ):
            for mf in range(N_MF):
                w_t = p6w.tile([P, C], BF16, name=f"fcw{mf}", tag="fcw")
                nc.sync.dma_start(out=w_t[:], in_=fcw[mf, :, :])
                ps = ps_t(f"g{mf}", "kq", 4)
                for k in range(N_KC):
                    nc.tensor.matmul(
                        ps[:], w_t[:, k * P:(k + 1) * P], n2T[k][:],
                        start=(k == 0), stop=(k == N_KC - 1),
                    )
                nc.scalar.activation(gT[mf][:], ps[:], AF.Gelu)

        # ---- phase 7: mlp proj + residual -> out ---------------------
        with (
            tc.tile_pool(name="p7o", bufs=4) as p7o,
        ):
            for ch in range(N_MCH):
                w_t = p7w.tile([P, N_MF * MLP_CC], BF16, name=f"mw{ch}", tag="mw")
                nc.sync.dma_start(out=w_t[:], in_=mww[ch, :, :])
                for mt in range(N_RT):
                    mcol = slice(mt * P, (mt + 1) * P)
                    ps = ps_t(f"f{ch}_{mt}", "kq", 4, cols=MLP_CC)
                    for kf in range(N_MF):
                        nc.tensor.matmul(
                            ps[:],
                            gT[kf][:, mcol],
                            w_t[:, kf * MLP_CC:(kf + 1) * MLP_CC],
                            start=(kf == 0), stop=(kf == N_MF - 1),
                        )
                    o_t = p7o.tile([P, MLP_CC], F32, name=f"o{ch}_{mt}", tag="o")
                    nc.vector.tensor_add(
                        o_t[:],
                        x2_res[mt][:, ch * MLP_CC:(ch + 1) * MLP_CC],
                        ps[:],
                    )
                    nc.scalar.dma_start(
                        out=out_loc[
                            mt * P:(mt + 1) * P,
                            ch * MLP_CC:(ch + 1) * MLP_CC,
                        ],
                        in_=o_t[:],
                    )
        p7w_ctx.close()
        gT_ctx.close()
        n2T_ctx.close()
        x2_ctx.close()
        stk.close()

    return nc


_NC_CACHE = None


def _get_nc():
    global _NC_CACHE
    if _NC_CACHE is None:
        _NC_CACHE = build_nc()
    return _NC_CACHE


def _prep_inputs(x, cos, sin, attention_bias, norm1_w, norm2_w, attn_w, proj_w,
                 fc_w, mlp_proj_w):
    bf = ml_dtypes.bfloat16
    xf = np.asarray(x, np.float32).reshape(R, C)
    w1 = np.asarray(norm1_w, np.float32)
    w2 = np.asarray(norm2_w, np.float32)
    aw = np.asarray(attn_w, np.float32) * w1[None, :]      # [3C, C] (norm folded)
    pwf = np.asarray(proj_w, np.float32)                   # [C, C]
    fwf = np.asarray(fc_w, np.float32) * w2[None, :]       # [F, C]
    mwf = np.asarray(mlp_proj_w, np.float32)               # [C, F]
    cosf = np.asarray(cos, np.float32)                     # [T, 64]
    sinf = np.asarray(sin, np.float32)

    awr = aw.reshape(H, 3, HD, C)
    # qkw[j<H] = K-weights of head j; qkw[j>=H] = Q-weights of head j-H.
    # qkw[j, p, k*128+m] = awr[h, comp, m, k*128+p]
    qk = np.empty((2 * H, P, C), np.float32)
    for h in range(H):
        qk[h] = awr[h, 1].T.reshape(N_KC, P, HD).transpose(1, 0, 2).reshape(P, C)
        qk[H + h] = awr[h, 0].T.reshape(N_KC, P, HD).transpose(1, 0, 2).reshape(P, C)
    # vw[half, k, p, (h-8*half)*128+d] = awr[h, 2, d, k*128+p]
    vwt = (
        awr[:, 2].reshape(H * HD, C).T.reshape(N_KC, P, 2, C // 2)
        .transpose(2, 0, 1, 3)
    )
    # pw[ont, p, kd*512+co] = proj_w[ont*512+co, kd*128+p]
    pwt = np.ascontiguousarray(
        pwf.reshape(4, 512, N_KC, P).transpose(0, 3, 2, 1)
    ).reshape(4, P, N_KC * 512)
    # fcw[mf, p, k*128+f] = fwf[mf*128+f, k*128+p]
    fct = np.ascontiguousarray(
        fwf.reshape(N_MF, P, N_KC, P).transpose(0, 3, 2, 1)
    ).reshape(N_MF, P, C)
    # mww[ch, p, kf*CC+c] = mwf[ch*CC+c, kf*128+p]
    mwt = np.ascontiguousarray(
        mwf.reshape(N_MCH, MLP_CC, N_MF, P).transpose(0, 3, 2, 1)
    ).reshape(N_MCH, P, N_MF * MLP_CC)

    qk_b = np.ascontiguousarray(qk).astype(bf)
    vw_b = np.ascontiguousarray(vwt).astype(bf)
    pw_b = np.ascontiguousarray(pwt).astype(bf)
    fc_b = fct.astype(bf)
    mw_b = mwt.astype(bf)
    # mask[s, t] = 1 iff s <= t  (transposed causal tril)
    maskT = np.triu(np.ones((P, P), np.float32))

    in_maps = []
    for c in range(N_CORES):
        t0 = (c % (N_CORES // B)) * R_LOC
        sm = np.zeros((P, N_CORES), np.float32)
        for s in range(N_CORES):
            if s // (N_CORES // B) == c // (N_CORES // B) and s > c:
                sm[:, s] = 1.0
        in_maps.append({
            "x_loc": np.ascontiguousarray(xf[R_LOC * c:R_LOC * (c + 1)]),
            "cosr": np.ascontiguousarray(cosf[t0:t0 + R_LOC].T).astype(bf),
            "sinr": np.ascontiguousarray(sinf[t0:t0 + R_LOC].T).astype(bf),
            "maskT": maskT,
            "smask": sm,
            "qkw": qk_b,
            "vw": vw_b,
            "pw": pw_b,
            "fcw": fc_b,
            "mww": mw_b,
        })
    return in_maps


def kernel(**inputs):
    nc = _get_nc()
    in_maps = _prep_inputs(**inputs)
    res = run_bass_kernel_spmd(nc, in_maps, list(range(N_CORES)))
    out = np.concatenate(
        [np.asarray(res.results[c]["out_loc"], np.float32) for c in range(N_CORES)],
        axis=0,
    )
    return out.reshape(B, T, C)


# revision 13
# speedup vs baseline: 2.1149x; 1.0660x over previous
"""Trainium2 Bass kernel for nn_Block_42460046688864 (dense transformer block).

Reference math (B=2, T=2048, C=2048, H=16, HD=128):
    n1  = rmsnorm(x) * norm1_w
    qkv = n1 @ attn_w.T ; q,k,v per head ; q,k = rope(q,k) ; phi = elu(.)+1
    w   = (phi_q . phi_k) * scale * tril ; w /= sum(w) ; y = w @ v
    h   = y @ proj_w.T ; x2 = x + h
    ffn = gelu(rmsnorm(x2)*norm2_w @ fc_w.T) @ mlp_proj_w.T ; out = x2 + ffn

Distribution (8 NeuronCores, one NEFF, fully data-parallel):
  - rows (b*T+t, 4096 total) sharded 512/core; every core streams the FULL
    weights from its own HBM (no activation collectives at all).
  - attention is chunked linear attention (causal tril + positive elu+1
    features == prefix-state form; scale and eps cancel to ~1e-9 rel).
    The only cross-core dependency is the causal prefix state: each core's
    segment state S_seg[h] = sum_t k_t (x) [v_t | 1] is exchanged with ONE
    small ReduceScatter. Core j writes S_seg * mask[j<s, same-seq] into
    slot s, so after the add-RS core s holds exactly the sum of its
    same-sequence predecessors' states (its causal init state). The RS is
    issued right after K/V are ready and overlaps the Q projection; the
    correction q @ S_init is fused into each chunk's PSUM accumulation.
  - V is computed directly in [token, dim] layout by using n1^T chunks as
    the stationary matmul operand (no V transposes); K additionally needs
    [token, dim] for the state outer products -> 64 small DMA transposes.

Notes:
  - norm weights are folded into attn_w / fc_w on the host (exact algebra).
  - matmul operands are bf16 (fp32 PSUM accumulation); norms, residuals and
    attention numerators/denominators stay fp32 (psum) end to end.
  - weights are pre-tiled on the host into [128 x N] DMA slabs so every
    weight DMA is one contiguous >=2KB-per-partition block.
  - SBUF pools are strict LIFO per side; long-lived attention tiles live on
    the left stack, y^T on the right stack so lifetimes nest.
  - TileContext's tail drain is patched to split its semaphore waits:
    this walrus build rejects >2 sync waits on one TPB_CTRL instruction.
"""

from contextlib import ExitStack

import numpy as np
import ml_dtypes

import concourse.bass as bass
import concourse.mybir as mybir
import concourse.tile as tile
from concourse.bass_utils import run_bass_kernel_spmd
from concourse.masks import make_identity
from bass_rust import ScopedClock

F32 = mybir.dt.float32
BF16 = mybir.dt.bfloat16
AF = mybir.ActivationFunctionType

N_CORES = 8
B, T, C, H, HD = 2, 2048, 2048, 16, 128
F = 4 * C                  # 8192 mlp hidden
R = B * T                  # 4096 flattened rows (b-major)
R_LOC = R // N_CORES       # 512 rows per core
P = 128
EPS_NORM = 1e-5
N_RT = R_LOC // P          # 4 local row tiles
N_KC = C // P              # 16 contraction tiles over C
N_CH = N_RT                # 4 local causal chunks
N_MF = F // P              # 64 mlp-hidden tiles
HD1 = HD + 1               # state cols: [v dims | 1]
SAW = H * HD1              # 2064 = all-head state cols
MLP_CC = 256               # mlp output col-chunk
N_MCH = C // MLP_CC        # 8 col chunks

_MAX_WAITS = 2  # this walrus build rejects >2 sync waits per instruction


def _split_excess_waits(nc):
    """Move excess semaphore waits onto same-engine NoOps ahead of the op."""
    for fn in nc.m.functions:
        for bb in fn.blocks:
            insts = list(bb.instructions)
            out = []
            for ins in insts:
                si = getattr(ins, "sync_info", None)
                waits = list(si.on_wait) if si and si.on_wait else []
                sem_waits = [w for w in waits if w.sync_type == "semaphore"]
                if len(sem_waits) > _MAX_WAITS:
                    keep = [w for w in waits if w.sync_type != "semaphore"]
                    keep += sem_waits[: _MAX_WAITS - 1] if _MAX_WAITS > 1 else []
                    extra = sem_waits[_MAX_WAITS - 1:] if _MAX_WAITS > 1 else sem_waits
                    for j in range(0, len(extra), _MAX_WAITS):
                        chunk = extra[j:j + _MAX_WAITS]
                        nop = mybir.InstNoOp(
                            name=nc.get_next_instruction_name(), ins=[], outs=[]
                        )
                        nop.engine = ins.engine
                        nop.sync_info = mybir.SyncInfo(on_wait=chunk, on_update=[])
                        out.append(nop)
                    si.on_wait[:] = keep
                out.append(ins)
            if len(out) != len(insts):
                bb.instructions[:] = out


class _TC(tile.TileContext):
    """TileContext whose tail drain splits sem waits one-per-NOP."""

    def schedule_and_allocate(self):
        ret = super().schedule_and_allocate()
        _split_excess_waits(self.nc)
        return ret

    def _drain_and_barrier(self, tick_clock, wait_clock):
        probe = self.nc.sync.nop(nofuse=True, hint="drain_waits")
        wait_clock.add_sem_waits(
            probe.ins, ScopedClock({None: tick_clock.global_clock})
        )
        si = probe.ins.sync_info
        waits = list(si.on_wait) if si and si.on_wait else []
        if len(waits) > 1:
            si.on_wait[:] = waits[:1]
            for w in waits[1:]:
                extra = self.nc.sync.nop(nofuse=True, hint="drain_waits")
                extra.ins.sync_info = mybir.SyncInfo(on_wait=[w], on_update=[])
        self.nc.sync.drain()
        self.nc.all_engine_barrier()
        popped = self.nc._tile_sem_poison_stack.pop()
        assert popped is self._sem_poison
        self.nc.clear_and_free_semaphores(list(self.sems.allocated().values()))
        self.nc.all_engine_barrier()


def build_nc():
    nc = bass.Bass(target_bir_lowering=False)

    x_loc = nc.declare_dram_parameter("x_loc", [R_LOC, C], F32, isOutput=False)
    cosr = nc.declare_dram_parameter("cosr", [HD // 2, R_LOC], BF16, isOutput=False)
    sinr = nc.declare_dram_parameter("sinr", [HD // 2, R_LOC], BF16, isOutput=False)
    maskT = nc.declare_dram_parameter("maskT", [P, P], F32, isOutput=False)
    smask = nc.declare_dram_parameter("smask", [P, N_CORES], F32, isOutput=False)
    # pre-tiled weight slabs (see _prep_inputs for layouts)
    qkw = nc.declare_dram_parameter("qkw", [2 * H, P, C], BF16, isOutput=False)
    vw = nc.declare_dram_parameter("vw", [2, N_KC, P, C // 2], BF16, isOutput=False)
    pw = nc.declare_dram_parameter("pw", [4, P, N_KC * 512], BF16, isOutput=False)
    fcw = nc.declare_dram_parameter("fcw", [N_MF, P, C], BF16, isOutput=False)
    mww = nc.declare_dram_parameter(
        "mww", [N_MCH, P, N_MF * MLP_CC], BF16, isOutput=False
    )
    out_loc = nc.declare_dram_parameter("out_loc", [R_LOC, C], F32, isOutput=True)

    rs_in = nc.dram_tensor("rs_in", [N_CORES, P, SAW], BF16)
    rs_out = nc.dram_tensor("rs_out", [P, SAW], BF16)

    groups = [list(range(N_CORES))]

    with _TC(nc) as tc:
        stk = ExitStack()
        const = stk.enter_context(tc.tile_pool(name="const", bufs=1))
        psum = stk.enter_context(tc.tile_pool(name="psum", bufs=1, space="PSUM"))
        def ps_t(name, tag, bufs, cols=512, dtype=F32):
            return psum.tile([P, cols], dtype, name=name, tag=tag, bufs=bufs)
        ident_f32 = const.tile([P, P], F32)
        make_identity(nc, ident_f32)
        ident_bf = const.tile([P, P], BF16)
        make_identity(nc, ident_bf)
        mask_sb = const.tile([P, P], F32)
        nc.sync.dma_start(out=mask_sb[:], in_=maskT[:, :])
        smask_sb = const.tile([P, N_CORES], F32)
        nc.sync.dma_start(out=smask_sb[:], in_=smask[:, :])
        eps_t = const.tile([P, 1], F32)
        nc.vector.memset(eps_t[:], EPS_NORM)
        cos_sb = const.tile([HD // 2, R_LOC], BF16)
        sin_sb = const.tile([HD // 2, R_LOC], BF16)
        nc.sync.dma_start(out=cos_sb[:], in_=cosr[:, :])
        nc.sync.dma_start(out=sin_sb[:], in_=sinr[:, :])

        # y^T on the right stack: outlives the attention residents (left).
        yT_ctx = ExitStack()
        yT_pool = yT_ctx.enter_context(tc.tile_pool(name="yT", bufs=1, side="right"))
        yT = [
            [yT_pool.tile([P, P], BF16, name=f"yT{h}_{i}") for i in range(N_CH)]
            for h in range(H)
        ]

        # attention residents (left): released together after phase 4.
        att_ctx = ExitStack()
        n1T_pool = att_ctx.enter_context(tc.tile_pool(name="n1T", bufs=1))
        qk_pool = att_ctx.enter_context(tc.tile_pool(name="qkres", bufs=1))
        vp_pool = att_ctx.enter_context(tc.tile_pool(name="vpres", bufs=1))
        sbf_pool = att_ctx.enter_context(tc.tile_pool(name="sbfres", bufs=1))
        n1T = [n1T_pool.tile([P, R_LOC], BF16, name=f"n1T{k}") for k in range(N_KC)]
        kres = [qk_pool.tile([P, R_LOC], BF16, name=f"k{h}") for h in range(H)]
        qres = [qk_pool.tile([P, R_LOC], BF16, name=f"q{h}") for h in range(H)]
        vp = [
            [vp_pool.tile([P, HD1], BF16, name=f"vp{h}_{i}") for i in range(N_CH)]
            for h in range(H)
        ]

        # ---- phase 0: rmsnorm(x) -> n1T (SBUF-resident, transposed) ----
        with (
            tc.tile_pool(name="p0x", bufs=2) as p0x,
            tc.tile_pool(name="p0sq", bufs=1) as p0sq,
            tc.tile_pool(name="p0st", bufs=8) as p0st,
            tc.tile_pool(name="p0n", bufs=1) as p0n,
        ):
            n_ts = []
            for i in range(N_RT):
                x_t = p0x.tile([P, C], F32, name=f"x{i}", tag="x")
                nc.sync.dma_start(out=x_t[:], in_=x_loc[i * P:(i + 1) * P, :])
                sq = p0sq.tile([P, C], F32, name=f"sq{i}", tag="sq")
                ss = p0st.tile([P, 1], F32, name=f"ss{i}", tag="ss")
                nc.scalar.activation(sq[:], x_t[:], AF.Square, accum_out=ss[:])
                rms = p0st.tile([P, 1], F32, name=f"rms{i}", tag="rms")
                nc.scalar.activation(
                    rms[:], ss[:], AF.Sqrt, bias=eps_t[:], scale=1.0 / C
                )
                inv = p0st.tile([P, 1], F32, name=f"inv{i}", tag="inv")
                nc.vector.reciprocal(inv[:], rms[:])
                n_t = p0n.tile([P, C], F32, name=f"n{i}", tag=f"n{i}")
                nc.vector.tensor_scalar_mul(n_t[:], x_t[:], inv[:])
                n_ts.append(n_t)
            for k in range(N_KC):
                ps = ps_t(f"tr{k}", "v", 2)
                for i in range(N_RT):
                    nc.tensor.transpose(
                        ps[:, i * P:(i + 1) * P],
                        n_ts[i][:, k * P:(k + 1) * P], ident_f32[:],
                    )
                nc.scalar.copy(n1T[k][:], ps[:])

        # ---- phase 1: K then V then Q projections --------------------
        HF = HD // 2

        def rope_elu(dst, ps, pool, uid):
            """psum [128 x 512] (d-major head tile) -> phi(rope(.)) bf16."""
            raw = pool.tile([P, R_LOC], BF16, name=f"raw{uid}", tag="raw")
            nc.scalar.copy(raw[:], ps[:])
            s1 = pool.tile([HF, R_LOC], BF16, name=f"s1{uid}", tag="s1")
            s2 = pool.tile([HF, R_LOC], BF16, name=f"s2{uid}", tag="s2")
            ro = pool.tile([P, R_LOC], BF16, name=f"ro{uid}", tag="ro")
            nc.vector.tensor_mul(s1[:], raw[0:HF, :], cos_sb[:])
            nc.vector.tensor_mul(s2[:], raw[HF:P, :], sin_sb[:])
            nc.vector.tensor_sub(ro[0:HF, :], s1[:], s2[:])
            nc.vector.tensor_mul(s1[:], raw[0:HF, :], sin_sb[:])
            nc.vector.tensor_mul(s2[:], raw[HF:P, :], cos_sb[:])
            nc.vector.tensor_add(ro[HF:P, :], s1[:], s2[:])
            # phi = elu(ro)+1 = max(ro,0) + exp(min(ro,0))
            rl = pool.tile([P, R_LOC], BF16, name=f"rl{uid}", tag="rl")
            nc.vector.tensor_scalar_max(rl[:], ro[:], 0.0)
            dm = pool.tile([P, R_LOC], BF16, name=f"dm{uid}", tag="dm")
            nc.vector.tensor_scalar_min(dm[:], ro[:], 0.0)
            ex = pool.tile([P, R_LOC], BF16, name=f"ex{uid}", tag="ex")
            nc.scalar.activation(ex[:], dm[:], AF.Exp)
            nc.vector.tensor_add(dst[:], rl[:], ex[:])

        with (
            tc.tile_pool(name="p1w", bufs=5) as p1w,
            tc.tile_pool(name="p1vw", bufs=1) as p1vw,
            tc.tile_pool(name="p1r", bufs=6) as p1r,
        ):
            # K projections (j = 0..15), rope+elu on DVE as tiles land
            for j in range(H):
                w_t = p1w.tile([P, C], BF16, name=f"kw{j}", tag="qkw")
                nc.sync.dma_start(out=w_t[:], in_=qkw[j, :, :])
                ps = ps_t(f"kps{j}", "kq", 3)
                for k in range(N_KC):
                    nc.tensor.matmul(
                        ps[:], w_t[:, k * P:(k + 1) * P], n1T[k][:],
                        start=(k == 0), stop=(k == N_KC - 1),
                    )
                rope_elu(kres[j], ps, p1r, f"k{j}")

            # V projections, directly transposed: [t x hv] chunks
            if True:
                for half in range(2):
                    vw_sb = []
                    for k in range(N_KC):
                        w_t = p1vw.tile(
                            [P, C // 2], BF16, name=f"vw{half}_{k}", tag=f"vw{k}"
                        )
                        nc.sync.dma_start(out=w_t[:], in_=vw[half, k, :, :])
                        vw_sb.append(w_t)
                    for vb in range(2):
                        vcol = slice(vb * 512, (vb + 1) * 512)
                        for i in range(N_CH):
                            icol = slice(i * P, (i + 1) * P)
                            ps = ps_t(f"v{half}_{vb}_{i}", "v", 2)
                            for k in range(N_KC):
                                nc.tensor.matmul(
                                    ps[:], n1T[k][:, icol], vw_sb[k][:, vcol],
                                    start=(k == 0), stop=(k == N_KC - 1),
                                )
                            for hs in range(4):
                                h = half * 8 + vb * 4 + hs
                                nc.scalar.copy(
                                    vp[h][i][:, 0:HD], ps[:, hs * P:(hs + 1) * P]
                                )
                                nc.vector.memset(vp[h][i][:, HD:HD1], 1.0)

            # ---- phase 2: segment states + masked RS exchange --------
            # (emitted before Q so the collective overlaps Q/scores)
            s_bf = [[None] * N_CH for _ in range(H)]
            with (
                tc.high_priority(),
                tc.tile_pool(name="p2kp", bufs=8) as p2kp,
                tc.tile_pool(name="p2all", bufs=1) as p2all,
                tc.tile_pool(name="p2msk", bufs=2) as p2msk,
            ):
                s_all = p2all.tile([P, SAW], BF16, name="s_all")
                for h in range(H):
                    kp_t = []
                    for i in range(N_CH):
                        tcol = slice(i * P, (i + 1) * P)
                        kps = ps_t(f"kptr{h}_{i}", "kptr", 1, cols=P, dtype=BF16)
                        nc.tensor.transpose(kps[:], kres[h][:, tcol], ident_bf[:])
                        kp = p2kp.tile([P, P], BF16, name=f"kp{h}_{i}", tag="kp")
                        nc.scalar.copy(kp[:], kps[:])
                        kp_t.append(kp)
                    # local prefix states P_m = sum_{i<m} kp_i^T @ [v_i | 1]
                    # (redundant accumulations: no cross-engine scan chain)
                    for m in range(1, N_CH + 1):
                        ps = ps_t(f"pfx{h}_{m}", "sd", 2, cols=HD1)
                        for i in range(m):
                            nc.tensor.matmul(
                                ps[:], kp_t[i][:], vp[h][i][:],
                                start=(i == 0), stop=(i == m - 1),
                            )
                        if m < N_CH:
                            sb = sbf_pool.tile([P, HD1], BF16, name=f"sbf{h}_{m}")
                            nc.scalar.copy(sb[:], ps[:])
                            s_bf[h][m] = sb
                        else:
                            nc.scalar.copy(s_all[:, h * HD1:(h + 1) * HD1], ps[:])
                for s in range(N_CORES):
                    ms = p2msk.tile([P, SAW], BF16, name=f"ms{s}", tag="ms")
                    nc.vector.tensor_scalar_mul(ms[:], s_all[:], smask_sb[:, s:s + 1])
                    nc.sync.dma_start(out=rs_in[s, :, :], in_=ms[:])
                nc.gpsimd.collective_compute(
                    "ReduceScatter",
                    mybir.AluOpType.add,
                    ins=[rs_in.ap().opt()],
                    outs=[rs_out.ap().opt()],
                    replica_groups=groups,
                )

            # Q projections (j = 16..31), overlap the collective
            for j in range(H):
                w_t = p1w.tile([P, C], BF16, name=f"qw{j}", tag="qkw")
                nc.sync.dma_start(out=w_t[:], in_=qkw[H + j, :, :])
                ps = ps_t(f"qps{j}", "kq", 3)
                for k in range(N_KC):
                    nc.tensor.matmul(
                        ps[:], w_t[:, k * P:(k + 1) * P], n1T[k][:],
                        start=(k == 0), stop=(k == N_KC - 1),
                    )
                rope_elu(qres[j], ps, p1r, f"q{j}")

        # ---- phase 3: local masked scores (no state dependency) ------
        am_ctx = ExitStack()
        am_pool = am_ctx.enter_context(tc.tile_pool(name="amres", bufs=1))
        am = [[None] * N_CH for _ in range(H)]
        if True:
            for h in range(H):
                for i in range(N_CH):
                    tcol = slice(i * P, (i + 1) * P)
                    a_ps = ps_t(f"a{h}_{i}", "sd", 2, cols=P)
                    nc.tensor.matmul(
                        a_ps[:], kres[h][:, tcol], qres[h][:, tcol],
                        start=True, stop=True,
                    )
                    am_t = am_pool.tile([P, P], BF16, name=f"am{h}_{i}")
                    nc.vector.tensor_mul(am_t[:], a_ps[:], mask_sb[:])
                    am[h][i] = am_t

        # ---- phase 4: y = (q@S_loc + Am^T@V' + q@S_init) / den; y^T --
        with (
            tc.tile_pool(name="p4si", bufs=1) as p4si,
            tc.tile_pool(name="p4y", bufs=4) as p4y,
        ):
            sinit = p4si.tile([P, SAW], BF16, name="sinit")
            with tc.high_priority():
                nc.sync.dma_start(out=sinit[:], in_=rs_out[:, :])
            for h in range(H):
                hcol = slice(h * HD1, (h + 1) * HD1)
                for i in range(N_CH):
                    tcol = slice(i * P, (i + 1) * P)
                    y_ps = ps_t(f"y{h}_{i}", "kq", 3, cols=HD1)
                    if i > 0:
                        nc.tensor.matmul(
                            y_ps[:], qres[h][:, tcol], s_bf[h][i][:],
                            start=True, stop=False,
                        )
                    nc.tensor.matmul(
                        y_ps[:], am[h][i][:], vp[h][i][:],
                        start=(i == 0), stop=False,
                    )
                    nc.tensor.matmul(
                        y_ps[:], qres[h][:, tcol], sinit[:, hcol],
                        start=False, stop=True,
                    )
                    rec = p4y.tile([P, 1], F32, name=f"rec{h}_{i}", tag="rec")
                    nc.vector.reciprocal(rec[:], y_ps[:, HD:HD1])
                    yb = p4y.tile([P, HD], BF16, name=f"yb{h}_{i}", tag="yb")
                    nc.vector.tensor_scalar_mul(yb[:], y_ps[:, 0:HD], rec[:])
                    tr = ps_t(f"ytr{h}_{i}", "v", 2, cols=P, dtype=BF16)
                    nc.tensor.transpose(tr[:], yb[:], ident_bf[:])
                    nc.scalar.copy(yT[h][i][:], tr[:])
        am_ctx.close()
        att_ctx.close()

        # ---- phase 5: proj, residual, rmsnorm2 -> n2T ----------------
        x2_ctx = ExitStack()
        x2_pool = x2_ctx.enter_context(tc.tile_pool(name="x2res", bufs=1))
        x2_res = [x2_pool.tile([P, C], F32, name=f"x2_{i}") for i in range(N_RT)]
        n2T_ctx = ExitStack()
        n2T_pool = n2T_ctx.enter_context(tc.tile_pool(name="n2T", bufs=1))
        n2T = [n2T_pool.tile([P, R_LOC], BF16, name=f"n2T{k}") for k in range(N_KC)]
        with (
            tc.tile_pool(name="p5w", bufs=2) as p5w,
            tc.tile_pool(name="p5sq", bufs=1) as p5sq,
            tc.tile_pool(name="p5st", bufs=8) as p5st,
            tc.tile_pool(name="p5n", bufs=1) as p5n,
        ):
            for mt in range(N_RT):
                nc.sync.dma_start(
                    out=x2_res[mt][:], in_=x_loc[mt * P:(mt + 1) * P, :]
                )
            for ont in range(4):
                ocol = slice(ont * 512, (ont + 1) * 512)
                w_t = p5w.tile([P, N_KC * 512], BF16, name=f"pw{ont}", tag="pw")
                nc.sync.dma_start(out=w_t[:], in_=pw[ont, :, :])
                for mg in range(2):
                    mts = (2 * mg, 2 * mg + 1)
                    ps_mt = {mt: ps_t(f"h{ont}_{mt}", "kq", 3) for mt in mts}
                    for kd in range(N_KC):
                        for mt in mts:
                            nc.tensor.matmul(
                                ps_mt[mt][:], yT[kd][mt][:],
                                w_t[:, kd * 512:(kd + 1) * 512],
                                start=(kd == 0), stop=(kd == N_KC - 1),
                            )
                    for mt in mts:
                        nc.vector.tensor_add(
                            x2_res[mt][:, ocol], x2_res[mt][:, ocol], ps_mt[mt][:]
                        )
            n2_ts = []
            for mt in range(N_RT):
                sq = p5sq.tile([P, C], F32, name=f"sq2_{mt}", tag="sq2")
                ss = p5st.tile([P, 1], F32, name=f"ss2_{mt}", tag="ss2")
                nc.scalar.activation(sq[:], x2_res[mt][:], AF.Square, accum_out=ss[:])
                rms = p5st.tile([P, 1], F32, name=f"rms2_{mt}", tag="rms2")
                nc.scalar.activation(
                    rms[:], ss[:], AF.Sqrt, bias=eps_t[:], scale=1.0 / C
                )
                inv = p5st.tile([P, 1], F32, name=f"inv2_{mt}", tag="inv2")
                nc.vector.reciprocal(inv[:], rms[:])
                n_t = p5n.tile([P, C], F32, name=f"n2_{mt}", tag=f"n2_{mt}")
                nc.vector.tensor_scalar_mul(n_t[:], x2_res[mt][:], inv[:])
                n2_ts.append(n_t)
            for k in range(N_KC):
                ps = ps_t(f"tr2_{k}", "v", 2)
                for mt in range(N_RT):
                    nc.tensor.transpose(
                        ps[:, mt * P:(mt + 1) * P],
                        n2_ts[mt][:, k * P:(k + 1) * P], ident_f32[:],
                    )
                nc.scalar.copy(n2T[k][:], ps[:])
        yT_ctx.close()

        # ---- phase 6: fc + gelu -> gT (resident) ---------------------
        gT_ctx = ExitStack()
        gT_pool = gT_ctx.enter_context(tc.tile_pool(name="gT", bufs=1))
        gT = [gT_pool.tile([P, R_LOC], BF16, name=f"gT{mf}") for mf in range(N_MF)]
        p7w_ctx = ExitStack()
        p7w = p7w_ctx.enter_context(tc.tile_pool(name="p7w", bufs=2))
        with (
            tc.tile_pool(name="p6w", bufs=3) as p6w,
        ):
            for mf in range(N_MF):
                w_t = p6w.tile([P, C], BF16, name=f"fcw{mf}", tag="fcw")
                nc.sync.dma_start(out=w_t[:], in_=fcw[mf, :, :])
                ps = ps_t(f"g{mf}", "kq", 3)
                for k in range(N_KC):
                    nc.tensor.matmul(
                        ps[:], w_t[:, k * P:(k + 1) * P], n2T[k][:],
                        start=(k == 0), stop=(k == N_KC - 1),
                    )
                nc.scalar.activation(gT[mf][:], ps[:], AF.Gelu)

        # ---- phase 7: mlp proj + residual -> out ---------------------
        with (
            tc.tile_pool(name="p7o", bufs=4) as p7o,
        ):
            for ch in range(N_MCH):
                w_t = p7w.tile([P, N_MF * MLP_CC], BF16, name=f"mw{ch}", tag="mw")
                nc.sync.dma_start(out=w_t[:], in_=mww[ch, :, :])
                for mt in range(N_RT):
                    mcol = slice(mt * P, (mt + 1) * P)
                    ps = ps_t(f"f{ch}_{mt}", "kq", 3, cols=MLP_CC)
                    for kf in range(N_MF):
                        nc.tensor.matmul(
                            ps[:],
                            gT[kf][:, mcol],
                            w_t[:, kf * MLP_CC:(kf + 1) * MLP_CC],
                            start=(kf == 0), stop=(kf == N_MF - 1),
                        )
                    o_t = p7o.tile([P, MLP_CC], F32, name=f"o{ch}_{mt}", tag="o")
                    nc.vector.tensor_add(
                        o_t[:],
                        x2_res[mt][:, ch * MLP_CC:(ch + 1) * MLP_CC],
                        ps[:],
                    )
                    nc.scalar.dma_start(
                        out=out_loc[
                            mt * P:(mt + 1) * P,
                            ch * MLP_CC:(ch + 1) * MLP_CC,
                        ],
                        in_=o_t[:],
                    )
        p7w_ctx.close()
        gT_ctx.close()
        n2T_ctx.close()
        x2_ctx.close()
        stk.close()

    return nc


_NC_CACHE = None


def _get_nc():
    global _NC_CACHE
    if _NC_CACHE is None:
        _NC_CACHE = build_nc()
    return _NC_CACHE


def _prep_inputs(x, cos, sin, attention_bias, norm1_w, norm2_w, attn_w, proj_w,
                 fc_w, mlp_proj_w):
    bf = ml_dtypes.bfloat16
    xf = np.asarray(x, np.float32).reshape(R, C)
    w1 = np.asarray(norm1_w, np.float32)
    w2 = np.asarray(norm2_w, np.float32)
    aw = np.asarray(attn_w, np.float32) * w1[None, :]      # [3C, C] (norm folded)
    pwf = np.asarray(proj_w, np.float32)                   # [C, C]
    fwf = np.asarray(fc_w, np.float32) * w2[None, :]       # [F, C]
    mwf = np.asarray(mlp_proj_w, np.float32)               # [C, F]
    cosf = np.asarray(cos, np.float32)                     # [T, 64]
    sinf = np.asarray(sin, np.float32)

    awr = aw.reshape(H, 3, HD, C)
    # qkw[j<H] = K-weights of head j; qkw[j>=H] = Q-weights of head j-H.
    # qkw[j, p, k*128+m] = awr[h, comp, m, k*128+p]
    qk = np.empty((2 * H, P, C), np.float32)
    for h in range(H):
        qk[h] = awr[h, 1].T.reshape(N_KC, P, HD).transpose(1, 0, 2).reshape(P, C)
        qk[H + h] = awr[h, 0].T.reshape(N_KC, P, HD).transpose(1, 0, 2).reshape(P, C)
    # vw[half, k, p, (h-8*half)*128+d] = awr[h, 2, d, k*128+p]
    vwt = (
        awr[:, 2].reshape(H * HD, C).T.reshape(N_KC, P, 2, C // 2)
        .transpose(2, 0, 1, 3)
    )
    # pw[ont, p, kd*512+co] = proj_w[ont*512+co, kd*128+p]
    pwt = np.ascontiguousarray(
        pwf.reshape(4, 512, N_KC, P).transpose(0, 3, 2, 1)
    ).reshape(4, P, N_KC * 512)
    # fcw[mf, p, k*128+f] = fwf[mf*128+f, k*128+p]
    fct = np.ascontiguousarray(
        fwf.reshape(N_MF, P, N_KC, P).transpose(0, 3, 2, 1)
    ).reshape(N_MF, P, C)
    # mww[ch, p, kf*CC+c] = mwf[ch*CC+c, kf*128+p]
    mwt = np.ascontiguousarray(
        mwf.reshape(N_MCH, MLP_CC, N_MF, P).transpose(0, 3, 2, 1)
    ).reshape(N_MCH, P, N_MF * MLP_CC)

    qk_b = np.ascontiguousarray(qk).astype(bf)
    vw_b = np.ascontiguousarray(vwt).astype(bf)
    pw_b = np.ascontiguousarray(pwt).astype(bf)
    fc_b = fct.astype(bf)
    mw_b = mwt.astype(bf)
    # mask[s, t] = 1 iff s <= t  (transposed causal tril)
    maskT = np.triu(np.ones((P, P), np.float32))

    in_maps = []
    for c in range(N_CORES):
        t0 = (c % (N_CORES // B)) * R_LOC
        sm = np.zeros((P, N_CORES), np.float32)
        for s in range(N_CORES):
            if s // (N_CORES // B) == c // (N_CORES // B) and s > c:
                sm[:, s] = 1.0
        in_maps.append({
            "x_loc": np.ascontiguousarray(xf[R_LOC * c:R_LOC * (c + 1)]),
            "cosr": np.ascontiguousarray(cosf[t0:t0 + R_LOC].T).astype(bf),
            "sinr": np.ascontiguousarray(sinf[t0:t0 + R_LOC].T).astype(bf),
            "maskT": maskT,
            "smask": sm,
            "qkw": qk_b,
            "vw": vw_b,
            "pw": pw_b,
            "fcw": fc_b,
            "mww": mw_b,
        })
    return in_maps


def kernel(**inputs):
    nc = _get_nc()
    in_maps = _prep_inputs(**inputs)
    res = run_bass_kernel_spmd(nc, in_maps, list(range(N_CORES)))
    out = np.concatenate(
        [np.asarray(res.results[c]["out_loc"], np.float32) for c in range(N_CORES)],
        axis=0,
    )
    return out.reshape(B, T, C)
